# revision 1
# baseline (speedup 1.0000x reference)
"""MetaNetImageEncoder Trainium2 kernel.

Data-parallel over batch: 8 samples per NeuronCore x 8 cores.

Per core (sample-local b in 0..7, D=768, N=196 patches, T=8 tasks):
  1. base pass:   A = P @ W1 (bf16 matmuls, patches pre-transposed on host)
                  pooled_b = mean_n relu(A + b1)   (ACT accum_out)
  2. MetaNet:     coefs[t,b] via two small matmul chains
  3. mixing:      M_b = sum_t c[t,b] dW1[t] via a block-diagonal coefficient
                  stationary (one [128,128] lhsT for all 96 matmuls), PSUM
                  partitions = (sample, i%16); SBUF->SBUF DMA de-interleave
                  into clean per-sample [i, j] tiles; += W1 on DVE
  4. final pass:  H = relu(P @ (W1+M_b) + nb1), pooled_new (ACT accum_out)
  5. layer 2:     out = pooled@W2 + sum_t c (pooled@dW2[t]) + b2 + c@db2,
                  all accumulated in one PSUM chain per 384-wide half
"""
import numpy as np
import ml_dtypes

import concourse.bass as bass
import concourse.mybir as mybir
import concourse.tile as tile
from concourse.vector_clock import ScopedClock
from concourse.bass_utils import run_bass_kernel_spmd

F32 = mybir.dt.float32
BF16 = mybir.dt.bfloat16
RELU = mybir.ActivationFunctionType.Relu

P = 16
D = 768
T = 8
HM = 192
NPAT = 196          # 14*14 patches
B = 64
NCORES = 8
BC = B // NCORES    # 8 samples per core
NB = BC * NPAT      # 1568
KT = D // 128       # 6 k-tiles

_PATCHED = False


def _apply_tile_patch():
    """This container's walrus allows only one sem wait per instruction;
    TileContext's exit drain attaches one wait per live semaphore. Split
    them onto standalone single-wait nops."""
    global _PATCHED
    if _PATCHED:
        return
    _PATCHED = True

    def _patched(self, tick_clock, wait_clock):
        carrier = self.nc.sync.nop(nofuse=True, hint="drain_waits")
        wait_clock.add_sem_waits(
            carrier.ins, ScopedClock({None: tick_clock.global_clock})
        )
        si = carrier.ins.sync_info
        waits = list(si.on_wait) if si else []
        if len(waits) > 1:
            carrier.ins.sync_info = mybir.SyncInfo(on_wait=[waits[0]], on_update=[])
            for w in waits[1:]:
                extra = self.nc.sync.nop(nofuse=True, hint="drain_waits")
                extra.ins.sync_info = mybir.SyncInfo(on_wait=[w], on_update=[])
        self.nc.sync.drain()
        self.nc.all_engine_barrier()
        popped = self.nc._tile_sem_poison_stack.pop()
        assert popped is self._sem_poison
        self.nc.clear_and_free_semaphores(list(self.sems.allocated().values()))
        self.nc.all_engine_barrier()

    tile.TileContext._drain_and_barrier = _patched


def _split_multi_waits(nc, max_waits: int = 1):
    """Hoist extra sem waits onto same-engine InstNoOp carriers."""
    for f in nc.m.functions:
        for blk in f.blocks:
            out = []
            for inst in blk.instructions:
                si = inst.sync_info
                if si is not None and len(si.on_wait) > max_waits:
                    waits = list(si.on_wait)
                    for i, w in enumerate(waits[:-max_waits]):
                        out.append(mybir.InstNoOp(
                            name=f"{inst.name}-w{i}",
                            sync_info=mybir.SyncInfo(on_wait=[w], on_update=[]),
                            bass_nofuse=True,
                            engine=inst.engine,
                        ))
                    inst.sync_info = mybir.SyncInfo(
                        on_wait=waits[-max_waits:], on_update=list(si.on_update)
                    )
                out.append(inst)
            blk.instructions = out


def build_kernel():
    nc = bass.Bass(target_bir_lowering=False, trn_type="TRN2")

    din = {}
    def inp(name, shape, dt):
        din[name] = nc.dram_tensor(name, shape, dt, kind="ExternalInput")
        return din[name]

    xt = inp("xt", (128, KT, NB), BF16)          # patches^T  [i_local, kt, (b,n)]
    w1 = inp("w1", (128, KT, D), BF16)           # W1 [i_local, kt, j]
    w2 = inp("w2", (128, KT, D), BF16)           # W2 [j_local, kt, e]
    dw1 = inp("dw1", (48, 128, D), BF16)         # [icl*6+it, (t,s16), j]
    dw2 = inp("dw2", (T, KT, 128, D), BF16)      # [t, kt, j_local, e]
    db1 = inp("db1", (T, D), BF16)
    db2 = inp("db2", (T, D), BF16)
    b1t = inp("b1t", (128, KT), F32)             # b1 [j_local, jt]
    b2t = inp("b2t", (128, KT), F32)             # b2 [e_local, et]
    b2r = inp("b2r", (BC, D), F32)               # b2 replicated over samples
    mw1 = inp("mw1", (128, KT, HM), BF16)
    mb1t = inp("mb1t", (128, 2), F32)
    mw2 = inp("mw2", (128, 2, T), BF16)          # [h_local, g, t], g=1 padded
    mb2t = inp("mb2t", (T, 1), F32)
    iexp = inp("iexp", (T, 128), F32)            # repeat(eye(8),16,axis=1)
    mask16 = inp("mask16", (128, P), BF16)       # [p, s'] = (p%16==s')
    i8 = inp("i8", (T, T), F32)                  # eye(8)
    i8bf = inp("i8bf", (T, T), BF16)             # eye(8) bf16

    out = nc.dram_tensor("out", (BC, D), F32, kind="ExternalOutput")

    with tile.TileContext(nc) as tc:
        with (
            tc.tile_pool(name="big", bufs=1) as big,
            tc.tile_pool(name="sm", bufs=1) as sm,
            tc.tile_pool(name="dwp", bufs=2) as dwp,
            tc.tile_pool(name="dw2p", bufs=3) as dw2p,
            tc.tile_pool(name="mxcp", bufs=2) as mxcp,
            tc.tile_pool(name="scr", bufs=3) as scr,
            tc.tile_pool(name="pst", bufs=2, space="PSUM") as pst,
        ):
            # ---------- persistent loads ----------
            xt_sb = big.tile([128, KT, NB], BF16, tag="xt")
            nc.sync.dma_start(xt_sb[:], xt[:])
            w1_sb = big.tile([128, KT, D], BF16, tag="w1")
            nc.sync.dma_start(w1_sb[:], w1[:])
            w2_sb = big.tile([128, KT, D], BF16, tag="w2")
            nc.sync.dma_start(w2_sb[:], w2[:])
            mxiall = big.tile([128, 8, KT, D], BF16, tag="mxiall")

            b1t_sb = sm.tile([128, KT], F32, tag="b1t")
            nc.sync.dma_start(b1t_sb[:], b1t[:])
            b2t_sb = sm.tile([128, KT], F32, tag="b2t")
            nc.sync.dma_start(b2t_sb[:], b2t[:])
            b2r_sb = sm.tile([BC, D], F32, tag="b2r")
            nc.sync.dma_start(b2r_sb[:], b2r[:])
            mw1_sb = sm.tile([128, KT, HM], BF16, tag="mw1")
            nc.sync.dma_start(mw1_sb[:], mw1[:])
            mb1t_sb = sm.tile([128, 2], F32, tag="mb1t")
            nc.sync.dma_start(mb1t_sb[:], mb1t[:])
            mw2_sb = sm.tile([128, 2, T], BF16, tag="mw2")
            nc.sync.dma_start(mw2_sb[:], mw2[:])
            mb2t_sb = sm.tile([T, 1], F32, tag="mb2t")
            nc.sync.dma_start(mb2t_sb[:], mb2t[:])
            iexp_sb = sm.tile([T, 128], F32, tag="iexp")
            nc.sync.dma_start(iexp_sb[:], iexp[:])
            mask16_sb = sm.tile([128, P], BF16, tag="mask16")
            nc.sync.dma_start(mask16_sb[:], mask16[:])
            i8_sb = sm.tile([T, T], F32, tag="i8")
            nc.sync.dma_start(i8_sb[:], i8[:])
            i8bf_sb = sm.tile([T, T], BF16, tag="i8bf")
            nc.sync.dma_start(i8bf_sb[:], i8bf[:])
            db1_sb = sm.tile([T, D], BF16, tag="db1")
            nc.sync.dma_start(db1_sb[:], db1[:])
            db2_sb = sm.tile([T, D], BF16, tag="db2")
            nc.sync.dma_start(db2_sb[:], db2[:])

            poolb = sm.tile([128, KT * BC], F32, tag="poolb")
            pooln = sm.tile([128, KT * BC], F32, tag="pooln")

            # ---------- phase 1: base pass ----------
            with tc.tile_pool(name="psA", bufs=4, space="PSUM") as psA:
                for jt in range(KT):
                    for ch in range(4):      # 4 chunks of 392 = 2 samples
                        pa = psA.tile([128, 392], F32, tag="a")
                        for kt in range(KT):
                            nc.tensor.matmul(
                                pa[:],
                                w1_sb[:, kt, jt * 128:(jt + 1) * 128],
                                xt_sb[:, kt, ch * 392:(ch + 1) * 392],
                                start=(kt == 0), stop=(kt == KT - 1))
                        for bi in range(2):
                            b = ch * 2 + bi
                            ro = scr.tile([128, NPAT], BF16, tag="ro")
                            nc.scalar.activation(
                                ro[:], pa[:, bi * NPAT:(bi + 1) * NPAT], RELU,
                                bias=b1t_sb[:, jt:jt + 1],
                                accum_out=poolb[:, jt * BC + b:jt * BC + b + 1])

            # ---------- phase 2: MetaNet ----------
            poolb_bf = sm.tile([128, KT * BC], BF16, tag="poolbbf")
            nc.scalar.mul(poolb_bf[:], poolb[:], 1.0 / NPAT)

            # base2^T[e, b] = W2.T @ pooled + b2  (input to MetaNet)
            base2_bf = sm.tile([128, KT * BC], BF16, tag="base2bf")
            for et in range(KT):
                p2 = pst.tile([128, T], F32, tag="tiny")
                for kt in range(KT):
                    nc.tensor.matmul(
                        p2[:], w2_sb[:, kt, et * 128:(et + 1) * 128],
                        poolb_bf[:, kt * BC:(kt + 1) * BC],
                        start=(kt == 0), stop=(kt == KT - 1))
                nc.vector.tensor_scalar_add(
                    base2_bf[:, et * BC:(et + 1) * BC], p2[:],
                    b2t_sb[:, et:et + 1])

            mh0 = sm.tile([128, T], BF16, tag="mh0")
            mh1 = sm.tile([64, T], BF16, tag="mh1")
            for g, mh_g in ((0, mh0), (1, mh1)):
                cols = 128 if g == 0 else 64
                pm = pst.tile([cols, T], F32, tag="tiny")
                for kt in range(KT):
                    nc.tensor.matmul(
                        pm[:], mw1_sb[:, kt, g * 128:g * 128 + cols],
                        base2_bf[:, kt * BC:(kt + 1) * BC],
                        start=(kt == 0), stop=(kt == KT - 1))
                nc.scalar.activation(mh_g[:], pm[:], RELU,
                                     bias=mb1t_sb[:cols, g:g + 1])

            pc = pst.tile([T, T], F32, tag="tiny")
            nc.tensor.matmul(pc[:], mw2_sb[:, 0, :], mh0[:], start=True, stop=False)
            nc.tensor.matmul(pc[:], mw2_sb[0:64, 1, :], mh1[:], start=False, stop=True)
            coefsT = sm.tile([T, T], F32, tag="coefsT")
            nc.vector.tensor_scalar_add(coefsT[:], pc[:], mb2t_sb[:])
            coefsT_bf = sm.tile([T, T], BF16, tag="coefsTbf")
            nc.vector.tensor_copy(coefsT_bf[:], coefsT[:])

            # coefficient replication [128, 8]: cRep[(t,s), b] = c[t, b]
            pr = pst.tile([128, T], F32, tag="tiny")
            nc.tensor.matmul(pr[:], iexp_sb[:], coefsT[:], start=True, stop=True)
            crep = sm.tile([128, T], F32, tag="crep")
            nc.vector.tensor_copy(crep[:], pr[:])

            # block-diagonal mixing stationary Cb[(t,s), (b,s')]
            cb_sb = sm.tile([128, 128], BF16, tag="cb")
            for b in range(BC):
                nc.vector.tensor_scalar_mul(
                    cb_sb[:, b * P:(b + 1) * P], mask16_sb[:],
                    crep[:, b:b + 1])

            # coefsB[b, t] = c[t, b]; Cdiag_t = diag(coefsB[:, t]) for layer 2
            pe2 = pst.tile([T, T], F32, tag="tiny")
            nc.tensor.matmul(pe2[:], coefsT[:], i8_sb[:], start=True, stop=True)
            coefsB = sm.tile([T, T], F32, tag="coefsB")
            nc.vector.tensor_copy(coefsB[:], pe2[:])
            cdiag = sm.tile([T, T, T], BF16, tag="cdiag")   # [b', t, b]
            for t in range(T):
                nc.vector.tensor_scalar_mul(
                    cdiag[:, t, :], i8bf_sb[:], coefsB[:, t:t + 1])

            # nb1t[j_local, jt, b] = b1 + coefs @ db1
            nb1t = sm.tile([128, KT, BC], F32, tag="nb1t")
            for jt in range(KT):
                pb = pst.tile([128, T], F32, tag="tiny")
                nc.tensor.matmul(pb[:], db1_sb[:, jt * 128:(jt + 1) * 128],
                                 coefsT_bf[:], start=True, stop=True)
                nc.vector.tensor_scalar_add(
                    nb1t[:, jt, :], pb[:], b1t_sb[:, jt:jt + 1])

            # ---------- phase 3: mixing ----------
            with tc.tile_pool(name="psM", bufs=3, space="PSUM") as psM:
                for icl in range(8):
                    dwt6 = dwp.tile([128, KT, D], BF16, tag="dw")
                    nc.sync.dma_start(
                        dwt6[:],
                        dw1[icl * KT:(icl + 1) * KT].rearrange("k p j -> p k j"))
                    for it in range(KT):
                        pm2 = psM.tile([128, 2, 512], F32, tag="m")  # 2 banks
                        for jh in range(2):
                            nc.tensor.matmul(
                                pm2[:, jh, 0:384], cb_sb[:],
                                dwt6[:, it, jh * 384:(jh + 1) * 384],
                                start=True, stop=True)
                        dst = mxiall[:, icl, it, :].rearrange(
                            "p (a b) -> p a b", a=2, b=384)
                        if (icl * KT + it) % 2 == 0:
                            nc.vector.tensor_copy(dst, pm2[:, :, 0:384])
                        else:
                            nc.scalar.copy(dst, pm2[:, :, 0:384])

            # ---------- phase 4: final per-sample pass ----------
            with tc.tile_pool(name="psF", bufs=4, space="PSUM") as psF:
                for b in range(BC):
                    mxcb = mxcp.tile([128, KT, D], BF16, tag="mxcb")
                    deint_eng = nc.scalar if b % 2 else nc.sync
                    for icl in range(8):
                        deint_eng.dma_start(
                            mxcb[icl * P:(icl + 1) * P, :, :],
                            mxiall[b * P:(b + 1) * P, icl, :, :])
                    nc.vector.tensor_tensor(
                        mxcb[:], mxcb[:], w1_sb[:], op=mybir.AluOpType.add)
                    for jt in range(KT):
                        pf = psF.tile([128, NPAT], F32, tag="f")
                        for it in range(KT):
                            nc.tensor.matmul(
                                pf[:],
                                mxcb[:, it, jt * 128:(jt + 1) * 128],
                                xt_sb[:, it, b * NPAT:(b + 1) * NPAT],
                                start=(it == 0), stop=(it == KT - 1))
                        ro = scr.tile([128, NPAT], BF16, tag="ro")
                        nc.scalar.activation(
                            ro[:], pf[:], RELU,
                            bias=nb1t[:, jt, b:b + 1],
                            accum_out=pooln[:, jt * BC + b:jt * BC + b + 1])

            # ---------- phase 5: layer 2 ----------
            pooln_bf = sm.tile([128, KT * BC], BF16, tag="poolnbf")
            nc.scalar.mul(pooln_bf[:], pooln[:], 1.0 / NPAT)

            vst = sm.tile([BC, T, D], BF16, tag="vst")
            psV = tc.alloc_tile_pool(name="psV", bufs=4, space="PSUM")
            for t in range(T):
                dwt2 = dw2p.tile([128, KT, D], BF16, tag="dw2")
                nc.sync.dma_start(
                    dwt2[:], dw2[t].rearrange("k p e -> p k e"))
                for eh in range(2):
                    pv = psV.tile([8, 384], F32, tag="v")
                    for kt in range(KT):
                        nc.tensor.matmul(
                            pv[:], pooln_bf[:, kt * BC:(kt + 1) * BC],
                            dwt2[:, kt, eh * 384:(eh + 1) * 384],
                            start=(kt == 0), stop=(kt == KT - 1))
                    nc.vector.tensor_copy(
                        vst[:, t, eh * 384:(eh + 1) * 384], pv[:])

            out_sb = sm.tile([BC, D], F32, tag="out")
            for eh in range(2):
                po = psV.tile([8, 384], F32, tag="v")
                for kt in range(KT):
                    nc.tensor.matmul(
                        po[:], pooln_bf[:, kt * BC:(kt + 1) * BC],
                        w2_sb[:, kt, eh * 384:(eh + 1) * 384],
                        start=(kt == 0), stop=False)
                for t in range(T):
                    nc.tensor.matmul(po[:], cdiag[:, t, :],
                                     vst[:, t, eh * 384:(eh + 1) * 384],
                                     start=False, stop=False)
                nc.tensor.matmul(po[:], coefsT_bf[:],
                                 db2_sb[:, eh * 384:(eh + 1) * 384],
                                 start=False, stop=True)
                nc.vector.tensor_tensor(
                    out_sb[:, eh * 384:(eh + 1) * 384], po[:],
                    b2r_sb[:, eh * 384:(eh + 1) * 384],
                    op=mybir.AluOpType.add)
            nc.sync.dma_start(out[:], out_sb[:])
            psV.release()

    _split_multi_waits(nc)
    return nc


def prep_inputs(x, W1, b1, W2, b2, dW1, db1, dW2, db2, mw1, mb1, mw2, mb2):
    """Host-side layout prep. Returns per-core in_maps."""
    bf = ml_dtypes.bfloat16
    x = np.asarray(x); W1 = np.asarray(W1); W2 = np.asarray(W2)
    b1 = np.asarray(b1); b2 = np.asarray(b2)
    dW1 = np.asarray(dW1); dW2 = np.asarray(dW2)
    db1 = np.asarray(db1); db2 = np.asarray(db2)
    mw1 = np.asarray(mw1); mb1 = np.asarray(mb1)
    mw2 = np.asarray(mw2); mb2 = np.asarray(mb2)

    # patches^T: [B, D, NPAT]
    pt = x.reshape(B, 3, 14, P, 14, P).transpose(0, 1, 3, 5, 2, 4)
    pt = np.ascontiguousarray(pt).reshape(B, D, NPAT)

    # shared (replicated) tensors
    w1_c = np.ascontiguousarray(
        W1.reshape(KT, 128, D).transpose(1, 0, 2)).astype(bf)
    w2_c = np.ascontiguousarray(
        W2.reshape(KT, 128, D).transpose(1, 0, 2)).astype(bf)
    # dw1[icl*6+it, (t,s16), j] = dW1[t, (it*8+icl)*16+s, j]
    d = dW1.reshape(T, KT, 8, P, D)            # [t, it, icl, s, j]
    dw1_c = np.ascontiguousarray(
        d.transpose(2, 1, 0, 3, 4).reshape(8 * KT, 128, D)).astype(bf)
    dw2_c = np.ascontiguousarray(dW2.reshape(T, KT, 128, D)).astype(bf)
    db1_c = db1.astype(bf)
    db2_c = db2.astype(bf)
    b1t_c = np.ascontiguousarray(b1.reshape(KT, 128).T).astype(np.float32)
    b2t_c = np.ascontiguousarray(b2.reshape(KT, 128).T).astype(np.float32)
    b2r_c = np.tile(b2.astype(np.float32), (BC, 1))
    mw1_c = np.ascontiguousarray(
        mw1.reshape(KT, 128, HM).transpose(1, 0, 2)).astype(bf)
    mb1t_c = np.zeros((128, 2), np.float32)
    mb1t_c[:, 0] = mb1[:128]
    mb1t_c[:64, 1] = mb1[128:]
    mw2_c = np.zeros((128, 2, T), np.float32)
    mw2_c[:, 0, :] = mw2[:128]
    mw2_c[:64, 1, :] = mw2[128:]
    mw2_c = mw2_c.astype(bf)
    mb2t_c = mb2.reshape(T, 1).astype(np.float32)
    iexp_c = np.repeat(np.eye(T, dtype=np.float32), P, axis=1)
    mask16_c = np.tile(np.eye(P, dtype=np.float32), (8, 1)).astype(bf)
    i8_c = np.eye(T, dtype=np.float32)

    shared = dict(
        w1=w1_c, w2=w2_c, dw1=dw1_c, dw2=dw2_c, db1=db1_c, db2=db2_c,
        b1t=b1t_c, b2t=b2t_c, b2r=b2r_c, mw1=mw1_c, mb1t=mb1t_c,
        mw2=mw2_c, mb2t=mb2t_c,
        iexp=iexp_c, mask16=mask16_c, i8=i8_c, i8bf=i8_c.astype(bf),
    )

    in_maps = []
    for c in range(NCORES):
        ptc = pt[c * BC:(c + 1) * BC]                  # [BC, D, NPAT]
        # xt[p, kt, (b,n)] = ptc[b, kt*128+p, n]
        xt_c = np.ascontiguousarray(
            ptc.reshape(BC, KT, 128, NPAT).transpose(2, 1, 0, 3)
        ).reshape(128, KT, NB).astype(bf)
        m = dict(shared)
        m["xt"] = xt_c
        in_maps.append(m)
    return in_maps


_NC_CACHE = {}


def kernel(**inputs) -> np.ndarray:
    _apply_tile_patch()
    if "nc" not in _NC_CACHE:
        _NC_CACHE["nc"] = build_kernel()
    nc = _NC_CACHE["nc"]
    in_maps = prep_inputs(**inputs)
    res = run_bass_kernel_spmd(nc, in_maps, core_ids=list(range(NCORES)))
    return np.concatenate([r["out"] for r in res.results], axis=0)



# revision 13
# speedup vs baseline: 1.4689x; 1.4689x over previous
"""MetaNetImageEncoder Trainium2 kernel — fp8 DoubleRow edition.

Data-parallel over batch: 8 samples per NeuronCore x 8 cores.

Per core (sample-local b in 0..7, D=768, N=196 patches, T=8 tasks):
  1. base pass:   A = P @ W1 as fp8 DoubleRow matmuls (K=256 per instr),
                  pooled_b = sum_n relu(A + b1') via ACT/DVE accum_out
                  (b1' = b1/(SX*SW); the fp8 scale folds into later muls)
  2. MetaNet:     coefs via small-stationary matmuls + PE transposes
  3. mixing:      M_b = sum_t c[t,b] dW1[t] with a (t,s32)-packed fp8
                  DoubleRow stationary; 4 samples x 32 i-rows per pass;
                  PSUM evacuated by DVE/GpSimd/ACT round-robin (x1/16)
  4. de-interleave: wide 32-partition DMAs regroup mixing output into
                  per-sample DR-layout stationary tiles
  5. final pass:  pf = P@W1 + P@M_b (6 fp8 DR matmuls, one PSUM chain),
                  relu-pool split between ACT and DVE
  6. layer 2:     out = pooled@W2 (bf16) + sum_t cdiag (pooled@dW2 fp8 DR)
                  + coefs@db2 + b2
"""
import numpy as np
import ml_dtypes

import concourse.bass as bass
import concourse.mybir as mybir
import concourse.tile as tile
from concourse.vector_clock import ScopedClock
from concourse.bass_utils import run_bass_kernel_spmd

F32 = mybir.dt.float32
BF16 = mybir.dt.bfloat16
F8 = mybir.dt.float8e4
RELU = mybir.ActivationFunctionType.Relu
DR = mybir.MatmulPerfMode.DoubleRow
ADD = mybir.AluOpType.add
MAX = mybir.AluOpType.max
MULT = mybir.AluOpType.mult

P = 16
D = 768
T = 8
HM = 192
NPAT = 196          # 14*14 patches
B = 64
NCORES = 8
BC = B // NCORES    # 8 samples per core
NB = BC * NPAT      # 1568
KT = D // 128       # 6 k-tiles
KTD = 3             # 3 double k-tiles

SX = 3.0            # patch fp8 scale
SW = 0.06           # weight fp8 scale
SXW = SX * SW

_PATCHED = False


def _apply_tile_patch():
    """This container's walrus allows only one sem wait per instruction;
    TileContext's exit drain attaches one wait per live semaphore. Split
    them onto standalone single-wait nops."""
    global _PATCHED
    if _PATCHED:
        return
    _PATCHED = True

    def _patched(self, tick_clock, wait_clock):
        carrier = self.nc.sync.nop(nofuse=True, hint="drain_waits")
        wait_clock.add_sem_waits(
            carrier.ins, ScopedClock({None: tick_clock.global_clock})
        )
        si = carrier.ins.sync_info
        waits = list(si.on_wait) if si else []
        if len(waits) > 1:
            carrier.ins.sync_info = mybir.SyncInfo(on_wait=[waits[0]], on_update=[])
            for w in waits[1:]:
                extra = self.nc.sync.nop(nofuse=True, hint="drain_waits")
                extra.ins.sync_info = mybir.SyncInfo(on_wait=[w], on_update=[])
        self.nc.sync.drain()
        self.nc.all_engine_barrier()
        popped = self.nc._tile_sem_poison_stack.pop()
        assert popped is self._sem_poison
        self.nc.clear_and_free_semaphores(list(self.sems.allocated().values()))
        self.nc.all_engine_barrier()

    tile.TileContext._drain_and_barrier = _patched


def _split_multi_waits(nc, max_waits: int = 1):
    """Hoist extra sem waits onto same-engine InstNoOp carriers."""
    for f in nc.m.functions:
        for blk in f.blocks:
            out = []
            for inst in blk.instructions:
                si = inst.sync_info
                if si is not None and len(si.on_wait) > max_waits:
                    waits = list(si.on_wait)
                    for i, w in enumerate(waits[:-max_waits]):
                        out.append(mybir.InstNoOp(
                            name=f"{inst.name}-w{i}",
                            sync_info=mybir.SyncInfo(on_wait=[w], on_update=[]),
                            bass_nofuse=True,
                            engine=inst.engine,
                        ))
                    inst.sync_info = mybir.SyncInfo(
                        on_wait=waits[-max_waits:], on_update=list(si.on_update)
                    )
                out.append(inst)
            blk.instructions = out


def build_kernel(split_waits=True, debug=False):
    nc = bass.Bass(target_bir_lowering=False, trn_type="TRN2")

    din = {}
    def inp(name, shape, dt):
        din[name] = nc.dram_tensor(name, shape, dt, kind="ExternalInput")
        return din[name]

    xt = inp("xt", (128, KTD, 2, NB), F8)        # patches^T/SX, DR layout
    w1 = inp("w1", (128, KTD, 2, D), F8)         # W1/SW, DR layout
    w2 = inp("w2", (128, KT, D), BF16)           # W2 [k_local, kt, e]
    dw1 = inp("dw1", (128, 24, 2, D), F8)        # [(t,slo), iblk, shi, j]
    dw2 = inp("dw2", (128, T, KTD, 2, D), F8)    # [k_local, t, ktd, hi, e]
    db1 = inp("db1", (T, D), BF16)               # db1/(SX*SW)
    db2 = inp("db2", (T, D), BF16)
    b1t = inp("b1t", (128, KT), F32)             # b1/(SX*SW) [j_local, jt]
    b2t = inp("b2t", (128, KT), F32)             # b2 [e_local, et]
    b2r = inp("b2r", (BC, D), F32)               # b2 replicated over samples
    mw1 = inp("mw1", (128, KT, HM), BF16)
    mb1r = inp("mb1r", (BC, HM), BF16)           # mb1 replicated over samples
    mw2 = inp("mw2", (128, 2, T), BF16)          # [h_local, g, t], g=1 padded
    mb2r = inp("mb2r", (BC, T), F32)             # mb2 replicated over samples
    iexp16 = inp("iexp16", (T, 128), F32)        # 16*repeat(eye(8),16,axis=1)
    mask32 = inp("mask32", (128, 2, 32), BF16)   # [(t,slo), shi, s'32]
    i8 = inp("i8", (T, T), F32)                  # eye(8)
    i8bf = inp("i8bf", (T, T), BF16)             # eye(8) bf16

    out = nc.dram_tensor("out", (BC, D), F32, kind="ExternalOutput")
    if debug:
        for nm, shp, dt in [
                ("dbg_poolb", (128, KT, BC), F32), ("dbg_coefsB", (BC, T), F32),
                ("dbg_crep", (128, T), F32),
                ("dbg_mxcb0", (128, 4, KTD, 2, D), F8),
                ("dbg_pooln", (128, KT, BC), F32),
                ("dbg_vst", (BC, T, D), BF16)]:
            din[nm] = nc.dram_tensor(nm, shp, dt, kind="ExternalOutput")

    with tile.TileContext(nc) as tc:
        with (
            tc.tile_pool(name="big", bufs=1) as big,
            tc.tile_pool(name="sm", bufs=1) as sm,
            tc.tile_pool(name="scr", bufs=4) as scr,
        ):
            # ---------- persistent loads ----------
            # sync queue: ph1+mixing critical path
            xt_sb = big.tile([128, KTD, 2, NB], F8, tag="xt")
            nc.sync.dma_start(xt_sb[:], xt[:])
            w1_sb = big.tile([128, KTD, 2, D], F8, tag="w1")
            nc.sync.dma_start(w1_sb[:], w1[:])
            dw1_sb = big.tile([128, 24, 2, D], F8, tag="dw1")
            nc.sync.dma_start(dw1_sb[:], dw1[:])
            # scalar queue: ph2/ph5 tensors
            w2_sb = big.tile([128, KT, D], BF16, tag="w2")
            nc.scalar.dma_start(w2_sb[:], w2[:])
            dw2_sb = big.tile([128, T, KTD, 2, D], F8, tag="dw2")
            nc.scalar.dma_start(dw2_sb[:], dw2[:])

            b1t_sb = sm.tile([128, KT], F32, tag="b1t")
            nc.sync.dma_start(b1t_sb[:], b1t[:])
            b2t_sb = sm.tile([128, KT], F32, tag="b2t")
            nc.sync.dma_start(b2t_sb[:], b2t[:])
            b2r_sb = sm.tile([BC, D], F32, tag="b2r")
            nc.sync.dma_start(b2r_sb[:], b2r[:])
            mw1_sb = sm.tile([128, KT, HM], BF16, tag="mw1")
            nc.sync.dma_start(mw1_sb[:], mw1[:])
            mb1r_sb = sm.tile([BC, HM], BF16, tag="mb1r")
            nc.sync.dma_start(mb1r_sb[:], mb1r[:])
            mw2_sb = sm.tile([128, 2, T], BF16, tag="mw2")
            nc.sync.dma_start(mw2_sb[:], mw2[:])
            mb2r_sb = sm.tile([BC, T], F32, tag="mb2r")
            nc.sync.dma_start(mb2r_sb[:], mb2r[:])
            iexp16_sb = sm.tile([T, 128], F32, tag="iexp16")
            nc.sync.dma_start(iexp16_sb[:], iexp16[:])
            mask32_sb = sm.tile([128, 2, 32], BF16, tag="mask32")
            nc.sync.dma_start(mask32_sb[:], mask32[:])
            i8_sb = sm.tile([T, T], F32, tag="i8")
            nc.sync.dma_start(i8_sb[:], i8[:])
            i8bf_sb = sm.tile([T, T], BF16, tag="i8bf")
            nc.sync.dma_start(i8bf_sb[:], i8bf[:])
            db1_sb = sm.tile([T, D], BF16, tag="db1")
            nc.sync.dma_start(db1_sb[:], db1[:])
            db2_sb = sm.tile([T, D], BF16, tag="db2")
            nc.sync.dma_start(db2_sb[:], db2[:])

            poolb = sm.tile([128, KT, BC], F32, tag="poolb")
            pooln = sm.tile([128, KT, BC], F32, tag="pooln")
            zeros_sb = sm.tile([128, NPAT], BF16, tag="zeros")
            nc.vector.memset(zeros_sb[:], 0.0)

            def relu_pool(pa, bi, jt, b, dst):
                """relu(pa_slice + b1') summed into dst column; ACT or DVE."""
                sl = pa[:, bi * NPAT:(bi + 1) * NPAT]
                if b % 2 == 0:
                    ro = scr.tile([128, NPAT], BF16, tag="ro")
                    nc.scalar.activation(
                        ro[:], sl, RELU, bias=b1t_sb[:, jt:jt + 1],
                        accum_out=dst)
                else:
                    ro = scr.tile([128, NPAT], BF16, tag="ro2")
                    nc.vector.scalar_tensor_tensor(
                        ro[:], sl, b1t_sb[:, jt:jt + 1], zeros_sb[:],
                        op0=ADD, op1=MAX, accum_out=dst)

            # ---------- phase 1: base pass (fp8 DR) ----------
            with tc.tile_pool(name="psA", bufs=4, space="PSUM") as psA:
                for jt in range(KT):
                    for ch in range(4):      # chunks of 392 = 2 samples
                        pa = psA.tile([128, 392], F32, tag="a")
                        for ktd in range(KTD):
                            nc.tensor.matmul(
                                pa[:],
                                w1_sb[:, ktd, :, jt * 128:(jt + 1) * 128],
                                xt_sb[:, ktd, :, ch * 392:(ch + 1) * 392],
                                start=(ktd == 0), stop=(ktd == KTD - 1),
                                perf_mode=DR)
                        for bi in range(2):
                            b = ch * 2 + bi
                            relu_pool(pa, bi, jt, b, poolb[:, jt, b:b + 1])

            # ---------- phase 2: MetaNet ----------
            with tc.tile_pool(name="psB", bufs=1, space="PSUM") as psB:
                poolb_bf = sm.tile([128, KT, BC], BF16, tag="poolbbf")
                nc.scalar.mul(poolb_bf[:], poolb[:], SXW / NPAT)

                # base2 = pooled @ W2 : [8, 768] (small stationary)
                pb2 = psB.tile([8, 2, 512], F32, tag="b2big")
                for eh in range(2):
                    for kt in range(KT):
                        nc.tensor.matmul(
                            pb2[:, eh, 0:384], poolb_bf[:, kt, :],
                            w2_sb[:, kt, eh * 384:(eh + 1) * 384],
                            start=(kt == 0), stop=(kt == KT - 1))
                base2b = sm.tile([BC, D], BF16, tag="base2b")
                nc.vector.tensor_copy(
                    base2b[:].rearrange("b (eh e) -> b eh e", eh=2),
                    pb2[:, :, 0:384])

                # transpose to [e_local, et, b] and add b2 bias
                base2T = sm.tile([128, KT, BC], BF16, tag="base2T")
                for et in range(KT):
                    ptp = psB.tile([128, BC], BF16, tag="tp")
                    nc.tensor.transpose(
                        ptp[:], base2b[:, et * 128:(et + 1) * 128], i8bf_sb[:])
                    nc.vector.tensor_scalar_add(
                        base2T[:, et, :], ptp[:], b2t_sb[:, et:et + 1])

                # mh = relu(base2 @ mw1 + mb1) : [8, 192]
                pmh = psB.tile([BC, HM], F32, tag="mh")
                for et in range(KT):
                    nc.tensor.matmul(
                        pmh[:], base2T[:, et, :], mw1_sb[:, et, :],
                        start=(et == 0), stop=(et == KT - 1))
                mh_sb = sm.tile([BC, HM], BF16, tag="mhsb")
                nc.vector.scalar_tensor_tensor(
                    mh_sb[:], pmh[:], 1.0, mb1r_sb[:], op0=MULT, op1=ADD)
                nc.vector.tensor_scalar_max(mh_sb[:], mh_sb[:], 0.0)

                # transpose mh -> [h_local, b] chunks
                mhT0 = sm.tile([128, BC], BF16, tag="mhT0")
                mhT1 = sm.tile([64, BC], BF16, tag="mhT1")
                ptp0 = psB.tile([128, BC], BF16, tag="tp")
                nc.tensor.transpose(ptp0[:], mh_sb[:, 0:128], i8bf_sb[:])
                nc.vector.tensor_copy(mhT0[:], ptp0[:])
                ptp1 = psB.tile([64, BC], BF16, tag="tp")
                nc.tensor.transpose(ptp1[:], mh_sb[:, 128:192], i8bf_sb[:])
                nc.vector.tensor_copy(mhT1[:], ptp1[:])

                # coefs[b, t]
                pcB = psB.tile([BC, T], F32, tag="sm8")
                nc.tensor.matmul(pcB[:], mhT0[:], mw2_sb[:, 0, :],
                                 start=True, stop=False)
                nc.tensor.matmul(pcB[:], mhT1[:], mw2_sb[0:64, 1, :],
                                 start=False, stop=True)
                coefsB = sm.tile([BC, T], F32, tag="coefsB")
                nc.vector.tensor_tensor(coefsB[:], pcB[:], mb2r_sb[:], op=ADD)

                # coefsT[t, b] via PE transpose
                ptc = psB.tile([T, T], F32, tag="sm8")
                nc.tensor.transpose(ptc[:], coefsB[:], i8_sb[:])
                coefsT = sm.tile([T, T], F32, tag="coefsT")
                nc.vector.tensor_copy(coefsT[:], ptc[:])
                coefsT_bf = sm.tile([T, T], BF16, tag="coefsTbf")
                nc.vector.tensor_copy(coefsT_bf[:], ptc[:])

                # crep16[(t,slo), b] = 16*c[t,b]
                pcr = psB.tile([128, T], F32, tag="sm8")
                nc.tensor.matmul(pcr[:], iexp16_sb[:], coefsT[:],
                                 start=True, stop=True)
                crep = sm.tile([128, T], F32, tag="crepsb")
                nc.vector.tensor_copy(crep[:], pcr[:])

                # mixing stationaries cb2_g[(t,slo), shi, (bg,s'32)] fp8
                cb2_0 = sm.tile([128, 2, 128], F8, tag="cb2_0")
                cb2_1 = sm.tile([128, 2, 128], F8, tag="cb2_1")
                cb2 = [cb2_0, cb2_1]
                for g in range(2):
                    for bg in range(4):
                        nc.vector.tensor_scalar_mul(
                            cb2[g][:, :, bg * 32:(bg + 1) * 32],
                            mask32_sb[:], crep[:, g * 4 + bg:g * 4 + bg + 1])

                # cdiag[b', t, b] for layer-2 coef application
                cdiag = sm.tile([T, T, T], BF16, tag="cdiag")
                for t in range(T):
                    nc.vector.tensor_scalar_mul(
                        cdiag[:, t, :], i8bf_sb[:], coefsB[:, t:t + 1])

                # nb1t[j_local, jt, b] = (b1 + coefs @ db1)/(SX*SW)
                nb1t = sm.tile([128, KT, BC], F32, tag="nb1t")
                for jt in range(KT):
                    pb = psB.tile([128, T], F32, tag="sm8")
                    nc.tensor.matmul(pb[:], db1_sb[:, jt * 128:(jt + 1) * 128],
                                     coefsT_bf[:], start=True, stop=True)
                    nc.vector.tensor_scalar_add(
                        nb1t[:, jt, :], pb[:], b1t_sb[:, jt:jt + 1])

            # ---------- phase 3: mixing (fp8 DR, 4 samples x 32 rows) ----------
            # mxg[g][(bg,s'), ph, kh, j] = M[(g,bg), i=(kh*4+ph)*32+s', j]/SW
            mxg_0 = big.tile([128, 4, KT, D], F8, tag="mxg0")
            mxg_1 = big.tile([128, 4, KT, D], F8, tag="mxg1")
            mxg = [mxg_0, mxg_1]
            mxcb_0 = big.tile([128, 4, KTD, 2, D], F8, tag="mxcb0")
            mxcb_1 = big.tile([128, 4, KTD, 2, D], F8, tag="mxcb1")
            mxcb = [mxcb_0, mxcb_1]
            cp_eng = [nc.vector, nc.scalar]
            with tc.tile_pool(name="psM", bufs=2, space="PSUM") as psM:
                for g in range(2):
                    for iblk in range(24):
                        pm = psM.tile([128, 2, 512], F32, tag="m")
                        for jh in range(2):
                            nc.tensor.matmul(
                                pm[:, jh, 0:384], cb2[g][:],
                                dw1_sb[:, iblk, :, jh * 384:(jh + 1) * 384],
                                start=True, stop=True, perf_mode=DR)
                        eng = cp_eng[iblk % 2]
                        dst = mxg[g][:, iblk % 4, iblk // 4, :].rearrange(
                            "p (jh j) -> p jh j", jh=2)
                        if eng is nc.scalar:
                            nc.scalar.mul(dst, pm[:, :, 0:384], 1.0 / 16.0)
                        else:
                            eng.tensor_scalar_mul(
                                dst, pm[:, :, 0:384], 1.0 / 16.0)

                    # ---------- de-interleave gathers for this group ----------
                    # mxcb[g][ph*32+pl, bg, ktd, hi, j] =
                    #     mxg[g][bg*32+pl, ph, (ktd,hi), j]
                    for ph in range(4):
                        for bg in range(4):
                            deq = nc.sync if (bg % 2 == 0) else nc.scalar
                            deq.dma_start(
                                mxcb[g][ph * 32:(ph + 1) * 32, bg, :, :, :],
                                mxg[g][bg * 32:(bg + 1) * 32, ph, :, :])

                # ---------- phase 4: final per-sample pass ----------
                with tc.tile_pool(name="psF", bufs=4, space="PSUM") as psF:
                    for b in range(BC):
                        g, bg = b // 4, b % 4
                        for jt in range(KT):
                            pf = psF.tile([128, NPAT], F32, tag="f")
                            for ktd in range(KTD):
                                nc.tensor.matmul(
                                    pf[:],
                                    w1_sb[:, ktd, :, jt * 128:(jt + 1) * 128],
                                    xt_sb[:, ktd, :, b * NPAT:(b + 1) * NPAT],
                                    start=(ktd == 0), stop=False, perf_mode=DR)
                            for ktd in range(KTD):
                                nc.tensor.matmul(
                                    pf[:],
                                    mxcb[g][:, bg, ktd, :,
                                            jt * 128:(jt + 1) * 128],
                                    xt_sb[:, ktd, :, b * NPAT:(b + 1) * NPAT],
                                    start=False, stop=(ktd == KTD - 1),
                                    perf_mode=DR)
                            ro = scr.tile([128, NPAT], BF16, tag="ro4")
                            if (b + jt) % 2 == 0:
                                nc.scalar.activation(
                                    ro[:], pf[:], RELU,
                                    bias=nb1t[:, jt, b:b + 1],
                                    accum_out=pooln[:, jt, b:b + 1])
                            else:
                                nc.vector.scalar_tensor_tensor(
                                    ro[:], pf[:], nb1t[:, jt, b:b + 1],
                                    zeros_sb[:], op0=ADD, op1=MAX,
                                    accum_out=pooln[:, jt, b:b + 1])

            # ---------- phase 5: layer 2 ----------
            pooln_f8 = sm.tile([128, KTD, 2, 32], F8, tag="poolnf8")
            nc.gpsimd.memset(pooln_f8[:], 0.0)
            nc.scalar.mul(
                pooln_f8[:, :, :, 0:BC].rearrange("p k h b -> p (k h) b"),
                pooln[:], SXW / NPAT)
            pooln_bf = sm.tile([128, KT, BC], BF16, tag="poolnbf")
            nc.gpsimd.tensor_scalar_mul(pooln_bf[:], pooln[:], SXW / NPAT)

            vst = sm.tile([BC, T, D], BF16, tag="vst")
            psV = tc.alloc_tile_pool(name="psV", bufs=2, space="PSUM")
            for t in range(T):
                pv = psV.tile([32, 2, 512], F32, tag="v")
                for eh in range(2):
                    for ktd in range(KTD):
                        nc.tensor.matmul(
                            pv[:, eh, 0:384], pooln_f8[:, ktd, :, :],
                            dw2_sb[:, t, ktd, :, eh * 384:(eh + 1) * 384],
                            start=(ktd == 0), stop=(ktd == KTD - 1),
                            perf_mode=DR)
                if t % 2 == 0:
                    nc.vector.tensor_scalar_mul(
                        vst[:, t, :].rearrange("b (eh e) -> b eh e", eh=2),
                        pv[0:BC, :, 0:384], SW)
                else:
                    nc.scalar.mul(
                        vst[:, t, :].rearrange("b (eh e) -> b eh e", eh=2),
                        pv[0:BC, :, 0:384], SW)

            out_sb = sm.tile([BC, D], F32, tag="out")
            po = psV.tile([BC, 2, 512], F32, tag="v")
            for eh in range(2):
                for kt in range(KT):
                    nc.tensor.matmul(
                        po[:, eh, 0:384], pooln_bf[:, kt, :],
                        w2_sb[:, kt, eh * 384:(eh + 1) * 384],
                        start=(kt == 0), stop=False)
                for t in range(T):
                    nc.tensor.matmul(
                        po[:, eh, 0:384], cdiag[:, t, :],
                        vst[:, t, eh * 384:(eh + 1) * 384],
                        start=False, stop=False)
                nc.tensor.matmul(
                    po[:, eh, 0:384], coefsT_bf[:],
                    db2_sb[:, eh * 384:(eh + 1) * 384],
                    start=False, stop=True)
            nc.vector.tensor_tensor(
                out_sb[:].rearrange("b (eh e) -> b eh e", eh=2),
                po[:, :, 0:384], b2r_sb[:].rearrange("b (eh e) -> b eh e", eh=2),
                op=ADD)
            nc.sync.dma_start(out[:], out_sb[:])
            if debug:
                nc.sync.dma_start(din["dbg_poolb"][:], poolb[:])
                nc.sync.dma_start(din["dbg_coefsB"][:], coefsB[:])
                nc.sync.dma_start(din["dbg_crep"][:], crep[:])
                nc.sync.dma_start(din["dbg_mxcb0"][:], mxcb[0][:])
                nc.sync.dma_start(din["dbg_pooln"][:], pooln[:])
                nc.sync.dma_start(din["dbg_vst"][:], vst[:])
            psV.release()

    if split_waits:
        _split_multi_waits(nc)
    return nc


def prep_inputs(x, W1, b1, W2, b2, dW1, db1, dW2, db2, mw1, mb1, mw2, mb2):
    """Host-side layout prep. Returns per-core in_maps."""
    bf = ml_dtypes.bfloat16
    f8 = ml_dtypes.float8_e4m3
    f32 = np.float32
    x = np.asarray(x, f32); W1 = np.asarray(W1, f32); W2 = np.asarray(W2, f32)
    b1 = np.asarray(b1, f32); b2 = np.asarray(b2, f32)
    dW1 = np.asarray(dW1, f32); dW2 = np.asarray(dW2, f32)
    db1 = np.asarray(db1, f32); db2 = np.asarray(db2, f32)
    mw1 = np.asarray(mw1, f32); mb1 = np.asarray(mb1, f32)
    mw2 = np.asarray(mw2, f32); mb2 = np.asarray(mb2, f32)

    # patches^T: [B, D, NPAT]
    pt = x.reshape(B, 3, 14, P, 14, P).transpose(0, 1, 3, 5, 2, 4)
    pt = np.ascontiguousarray(pt).reshape(B, D, NPAT)

    # shared (replicated) tensors
    w1_c = np.ascontiguousarray(
        (W1 / SW).reshape(KTD, 2, 128, D).transpose(2, 0, 1, 3)).astype(f8)
    w2_c = np.ascontiguousarray(
        W2.reshape(KT, 128, D).transpose(1, 0, 2)).astype(bf)
    # dw1[(t,slo), iblk, shi, j] = dW1[t, iblk*32+shi*16+slo, j]/SW
    d = (dW1 / SW).reshape(T, 24, 2, P, D)       # [t, iblk, shi, slo, j]
    dw1_c = np.ascontiguousarray(
        d.transpose(0, 3, 1, 2, 4).reshape(128, 24, 2, D)).astype(f8)
    # dw2[k_local, t, ktd, hi, e] = dW2[t, ktd*256+hi*128+k_local, e]/SW
    dw2_c = np.ascontiguousarray(
        (dW2 / SW).reshape(T, KTD, 2, 128, D).transpose(3, 0, 1, 2, 4)
    ).astype(f8)
    db1_c = (db1 / SXW).astype(bf)
    db2_c = db2.astype(bf)
    b1t_c = np.ascontiguousarray((b1 / SXW).reshape(KT, 128).T).astype(f32)
    b2t_c = np.ascontiguousarray(b2.reshape(KT, 128).T).astype(f32)
    b2r_c = np.tile(b2, (BC, 1))
    mw1_c = np.ascontiguousarray(
        mw1.reshape(KT, 128, HM).transpose(1, 0, 2)).astype(bf)
    mb1r_c = np.tile(mb1, (BC, 1)).astype(bf)
    mw2_c = np.zeros((128, 2, T), f32)
    mw2_c[:, 0, :] = mw2[:128]
    mw2_c[:64, 1, :] = mw2[128:]
    mw2_c = mw2_c.astype(bf)
    mb2r_c = np.tile(mb2, (BC, 1)).astype(f32)
    iexp16_c = np.repeat(np.eye(T, dtype=f32) * 16.0, P, axis=1)
    # mask32[(t,slo), shi, s'] = (s' == shi*16+slo)
    m32 = np.zeros((P, 2, 32), f32)
    for slo in range(P):
        for shi in range(2):
            m32[slo, shi, shi * P + slo] = 1.0
    mask32_c = np.tile(m32, (T, 1, 1)).astype(bf)
    i8_c = np.eye(T, dtype=f32)

    shared = dict(
        w1=w1_c, w2=w2_c, dw1=dw1_c, dw2=dw2_c, db1=db1_c, db2=db2_c,
        b1t=b1t_c, b2t=b2t_c, b2r=b2r_c, mw1=mw1_c, mb1r=mb1r_c,
        mw2=mw2_c, mb2r=mb2r_c,
        iexp16=iexp16_c, mask32=mask32_c, i8=i8_c, i8bf=i8_c.astype(bf),
    )

    in_maps = []
    for c in range(NCORES):
        ptc = pt[c * BC:(c + 1) * BC]                  # [BC, D, NPAT]
        # xt[p, ktd, hi, (b,n)] = ptc[b, ktd*256+hi*128+p, n]/SX
        xt_c = np.ascontiguousarray(
            (ptc / SX).reshape(BC, KTD, 2, 128, NPAT).transpose(3, 1, 2, 0, 4)
        ).reshape(128, KTD, 2, NB).astype(f8)
        m = dict(shared)
        m["xt"] = xt_c
        in_maps.append(m)
    return in_maps


_NC_CACHE = {}


def kernel(**inputs) -> np.ndarray:
    _apply_tile_patch()
    if "nc" not in _NC_CACHE:
        _NC_CACHE["nc"] = build_kernel()
    nc = _NC_CACHE["nc"]
    in_maps = prep_inputs(**inputs)
    res = run_bass_kernel_spmd(nc, in_maps, core_ids=list(range(NCORES)))
    return np.concatenate([r["out"] for r in res.results], axis=0)


# revision 18
# speedup vs baseline: 1.4877x; 1.0128x over previous
"""MetaNetImageEncoder Trainium2 kernel — fp8 DoubleRow edition.

Data-parallel over batch: 8 samples per NeuronCore x 8 cores.

Per core (sample-local b in 0..7, D=768, N=196 patches, T=8 tasks):
  1. base pass:   A = P @ W1 as fp8 DoubleRow matmuls (K=256 per instr),
                  pooled_b = sum_n relu(A + b1') via ACT/DVE accum_out
                  (b1' = b1/(SX*SW); the fp8 scale folds into later muls)
  2. MetaNet:     coefs via small-stationary matmuls + PE transposes
  3. mixing:      M_b = sum_t c[t,b] dW1[t] with a (t,s32)-packed fp8
                  DoubleRow stationary; 4 samples x 32 i-rows per pass;
                  PSUM evacuated by DVE/GpSimd/ACT round-robin (x1/16)
  4. de-interleave: wide 32-partition DMAs regroup mixing output into
                  per-sample DR-layout stationary tiles
  5. final pass:  pf = P@W1 + P@M_b (6 fp8 DR matmuls, one PSUM chain),
                  relu-pool split between ACT and DVE
  6. layer 2:     out = pooled@W2 (bf16) + sum_t cdiag (pooled@dW2 fp8 DR)
                  + coefs@db2 + b2
"""
import numpy as np
import ml_dtypes

import concourse.bass as bass
import concourse.mybir as mybir
import concourse.tile as tile
from concourse.vector_clock import ScopedClock
from concourse.bass_utils import run_bass_kernel_spmd

F32 = mybir.dt.float32
BF16 = mybir.dt.bfloat16
F8 = mybir.dt.float8e4
RELU = mybir.ActivationFunctionType.Relu
DR = mybir.MatmulPerfMode.DoubleRow
ADD = mybir.AluOpType.add
MAX = mybir.AluOpType.max
MULT = mybir.AluOpType.mult

P = 16
D = 768
T = 8
HM = 192
NPAT = 196          # 14*14 patches
B = 64
NCORES = 8
BC = B // NCORES    # 8 samples per core
NB = BC * NPAT      # 1568
KT = D // 128       # 6 k-tiles
KTD = 3             # 3 double k-tiles

SX = 3.0            # patch fp8 scale
SW = 0.06           # weight fp8 scale
SXW = SX * SW

_PATCHED = False


def _apply_tile_patch():
    """This container's walrus allows only one sem wait per instruction;
    TileContext's exit drain attaches one wait per live semaphore. Split
    them onto standalone single-wait nops."""
    global _PATCHED
    if _PATCHED:
        return
    _PATCHED = True

    def _patched(self, tick_clock, wait_clock):
        carrier = self.nc.sync.nop(nofuse=True, hint="drain_waits")
        wait_clock.add_sem_waits(
            carrier.ins, ScopedClock({None: tick_clock.global_clock})
        )
        si = carrier.ins.sync_info
        waits = list(si.on_wait) if si else []
        if len(waits) > 1:
            carrier.ins.sync_info = mybir.SyncInfo(on_wait=[waits[0]], on_update=[])
            for w in waits[1:]:
                extra = self.nc.sync.nop(nofuse=True, hint="drain_waits")
                extra.ins.sync_info = mybir.SyncInfo(on_wait=[w], on_update=[])
        self.nc.sync.drain()
        self.nc.all_engine_barrier()
        popped = self.nc._tile_sem_poison_stack.pop()
        assert popped is self._sem_poison
        self.nc.clear_and_free_semaphores(list(self.sems.allocated().values()))
        self.nc.all_engine_barrier()

    tile.TileContext._drain_and_barrier = _patched


def _split_multi_waits(nc, max_waits: int = 1):
    """Hoist extra sem waits onto same-engine InstNoOp carriers."""
    for f in nc.m.functions:
        for blk in f.blocks:
            out = []
            for inst in blk.instructions:
                si = inst.sync_info
                if si is not None and len(si.on_wait) > max_waits:
                    waits = list(si.on_wait)
                    for i, w in enumerate(waits[:-max_waits]):
                        out.append(mybir.InstNoOp(
                            name=f"{inst.name}-w{i}",
                            sync_info=mybir.SyncInfo(on_wait=[w], on_update=[]),
                            bass_nofuse=True,
                            engine=inst.engine,
                        ))
                    inst.sync_info = mybir.SyncInfo(
                        on_wait=waits[-max_waits:], on_update=list(si.on_update)
                    )
                out.append(inst)
            blk.instructions = out


def build_kernel(split_waits=True, debug=False):
    nc = bass.Bass(target_bir_lowering=False, trn_type="TRN2")

    din = {}
    def inp(name, shape, dt):
        din[name] = nc.dram_tensor(name, shape, dt, kind="ExternalInput")
        return din[name]

    xt = inp("xt", (128, KTD, 2, NB), F8)        # patches^T/SX, DR layout
    w1 = inp("w1", (128, KTD, 2, D), F8)         # W1/SW, DR layout
    w2 = inp("w2", (128, KT, D), BF16)           # W2 [k_local, kt, e]
    dw1 = inp("dw1", (128, 24, 2, D), F8)        # [(t,slo), iblk, shi, j]
    w1i = inp("w1i", (128, 4, 4, D), F8)         # W1/SW mixing-layout, ktd0/1
    dw2 = inp("dw2", (128, T, KTD, 2, D), F8)    # [k_local, t, ktd, hi, e]
    db1 = inp("db1", (T, D), BF16)               # db1/(SX*SW)
    db2 = inp("db2", (T, D), BF16)
    b1t = inp("b1t", (128, KT), F32)             # b1/(SX*SW) [j_local, jt]
    b2t = inp("b2t", (128, KT), F32)             # b2 [e_local, et]
    b2r = inp("b2r", (BC, D), F32)               # b2 replicated over samples
    mw1 = inp("mw1", (128, KT, HM), BF16)
    mb1r = inp("mb1r", (BC, HM), BF16)           # mb1 replicated over samples
    mw2 = inp("mw2", (128, 2, T), BF16)          # [h_local, g, t], g=1 padded
    mb2r = inp("mb2r", (BC, T), F32)             # mb2 replicated over samples
    iexp16 = inp("iexp16", (T, 128), F32)        # 16*repeat(eye(8),16,axis=1)
    mask32 = inp("mask32", (128, 2, 32), BF16)   # [(t,slo), shi, s'32]
    i8 = inp("i8", (T, T), F32)                  # eye(8)
    i8bf = inp("i8bf", (T, T), BF16)             # eye(8) bf16

    out = nc.dram_tensor("out", (BC, D), F32, kind="ExternalOutput")
    if debug:
        for nm, shp, dt in [
                ("dbg_poolb", (128, KT, BC), F32), ("dbg_coefsB", (BC, T), F32),
                ("dbg_crep", (128, T), F32),
                ("dbg_mxcb0", (128, 4, KTD, 2, D), F8),
                ("dbg_pooln", (128, KT, BC), F32),
                ("dbg_vst", (BC, T, D), BF16)]:
            din[nm] = nc.dram_tensor(nm, shp, dt, kind="ExternalOutput")

    with tile.TileContext(nc) as tc:
        with (
            tc.tile_pool(name="big", bufs=1) as big,
            tc.tile_pool(name="sm", bufs=1) as sm,
            tc.tile_pool(name="scr", bufs=2) as scr,
        ):
            # ---------- persistent loads ----------
            b1t_sb = sm.tile([128, KT], F32, tag="b1t")
            nc.sync.dma_start(b1t_sb[:], b1t[:])
            b2t_sb = sm.tile([128, KT], F32, tag="b2t")
            nc.sync.dma_start(b2t_sb[:], b2t[:])
            b2r_sb = sm.tile([BC, D], F32, tag="b2r")
            nc.sync.dma_start(b2r_sb[:], b2r[:])
            mw1_sb = sm.tile([128, KT, HM], BF16, tag="mw1")
            nc.sync.dma_start(mw1_sb[:], mw1[:])
            mb1r_sb = sm.tile([BC, HM], BF16, tag="mb1r")
            nc.sync.dma_start(mb1r_sb[:], mb1r[:])
            mw2_sb = sm.tile([128, 2, T], BF16, tag="mw2")
            nc.sync.dma_start(mw2_sb[:], mw2[:])
            mb2r_sb = sm.tile([BC, T], F32, tag="mb2r")
            nc.sync.dma_start(mb2r_sb[:], mb2r[:])
            iexp16_sb = sm.tile([T, 128], F32, tag="iexp16")
            nc.sync.dma_start(iexp16_sb[:], iexp16[:])
            mask32_sb = sm.tile([128, 2, 32], BF16, tag="mask32")
            nc.sync.dma_start(mask32_sb[:], mask32[:])
            i8_sb = sm.tile([T, T], F32, tag="i8")
            nc.sync.dma_start(i8_sb[:], i8[:])
            i8bf_sb = sm.tile([T, T], BF16, tag="i8bf")
            nc.sync.dma_start(i8bf_sb[:], i8bf[:])
            db1_sb = sm.tile([T, D], BF16, tag="db1")
            nc.sync.dma_start(db1_sb[:], db1[:])
            db2_sb = sm.tile([T, D], BF16, tag="db2")
            nc.sync.dma_start(db2_sb[:], db2[:])
            # sync queue: ph1 critical path after the smalls
            xt_sb = big.tile([128, KTD, 2, NB], F8, tag="xt")
            nc.sync.dma_start(xt_sb[:], xt[:])
            w1_sb = big.tile([128, KTD, 2, D], F8, tag="w1")
            nc.sync.dma_start(w1_sb[:], w1[:])
            # scalar queue: w2 (metanet), then mixing/ph5 tensors
            w2_sb = big.tile([128, KT, D], BF16, tag="w2")
            nc.scalar.dma_start(w2_sb[:], w2[:])
            dw1_sb = big.tile([128, 24, 2, D], F8, tag="dw1")
            nc.scalar.dma_start(dw1_sb[:], dw1[:])
            w1i_sb = big.tile([128, 4, 4, D], F8, tag="w1i")
            nc.scalar.dma_start(w1i_sb[:], w1i[:])
            dw2_sb = big.tile([128, T, KTD, 2, D], F8, tag="dw2")
            nc.scalar.dma_start(dw2_sb[:], dw2[:])

            poolb = sm.tile([128, KT, BC], F32, tag="poolb")
            pooln = sm.tile([128, KT, BC], F32, tag="pooln")
            zeros_sb = sm.tile([128, NPAT], F8, tag="zeros")
            nc.vector.memset(zeros_sb[:], 0.0)

            def relu_pool(pa, bi, jt, b, dst):
                """relu(pa_slice + b1') summed into dst column; ACT or DVE."""
                sl = pa[:, bi * NPAT:(bi + 1) * NPAT]
                if b % 2 == 0:
                    ro = scr.tile([128, NPAT], F8, tag="ro")
                    nc.scalar.activation(
                        ro[:], sl, RELU, bias=b1t_sb[:, jt:jt + 1],
                        accum_out=dst)
                else:
                    ro = scr.tile([128, NPAT], F8, tag="ro")
                    nc.vector.scalar_tensor_tensor(
                        ro[:], sl, b1t_sb[:, jt:jt + 1], zeros_sb[:],
                        op0=ADD, op1=MAX, accum_out=dst)

            # ---------- phase 1: base pass (fp8 DR) ----------
            with tc.tile_pool(name="psA", bufs=4, space="PSUM") as psA:
                for jt in range(KT):
                    for ch in range(4):      # chunks of 392 = 2 samples
                        pa = psA.tile([128, 392], F32, tag="a")
                        for ktd in range(KTD):
                            nc.tensor.matmul(
                                pa[:],
                                w1_sb[:, ktd, :, jt * 128:(jt + 1) * 128],
                                xt_sb[:, ktd, :, ch * 392:(ch + 1) * 392],
                                start=(ktd == 0), stop=(ktd == KTD - 1),
                                perf_mode=DR)
                        for bi in range(2):
                            b = ch * 2 + bi
                            relu_pool(pa, bi, jt, b, poolb[:, jt, b:b + 1])

            # ---------- phase 2: MetaNet ----------
            with tc.tile_pool(name="psB", bufs=1, space="PSUM") as psB:
                poolb_bf = sm.tile([128, KT, BC], BF16, tag="poolbbf")
                nc.scalar.mul(poolb_bf[:], poolb[:], SXW / NPAT)

                # base2 = pooled @ W2 : [8, 768] (small stationary)
                pb2 = psB.tile([8, 2, 512], F32, tag="b2big")
                for eh in range(2):
                    for kt in range(KT):
                        nc.tensor.matmul(
                            pb2[:, eh, 0:384], poolb_bf[:, kt, :],
                            w2_sb[:, kt, eh * 384:(eh + 1) * 384],
                            start=(kt == 0), stop=(kt == KT - 1))
                base2b = sm.tile([BC, D], BF16, tag="base2b")
                nc.vector.tensor_copy(
                    base2b[:].rearrange("b (eh e) -> b eh e", eh=2),
                    pb2[:, :, 0:384])

                # transpose to [e_local, et, b] and add b2 bias
                base2T = sm.tile([128, KT, BC], BF16, tag="base2T")
                for et in range(KT):
                    ptp = psB.tile([128, BC], BF16, tag="tp")
                    nc.tensor.transpose(
                        ptp[:], base2b[:, et * 128:(et + 1) * 128], i8bf_sb[:])
                    nc.vector.tensor_scalar_add(
                        base2T[:, et, :], ptp[:], b2t_sb[:, et:et + 1])

                # mh = relu(base2 @ mw1 + mb1) : [8, 192]
                pmh = psB.tile([BC, HM], F32, tag="mh")
                for et in range(KT):
                    nc.tensor.matmul(
                        pmh[:], base2T[:, et, :], mw1_sb[:, et, :],
                        start=(et == 0), stop=(et == KT - 1))
                mh_sb = sm.tile([BC, HM], BF16, tag="mhsb")
                nc.vector.scalar_tensor_tensor(
                    mh_sb[:], pmh[:], 1.0, mb1r_sb[:], op0=MULT, op1=ADD)
                nc.vector.tensor_scalar_max(mh_sb[:], mh_sb[:], 0.0)

                # transpose mh -> [h_local, b] chunks
                mhT0 = sm.tile([128, BC], BF16, tag="mhT0")
                mhT1 = sm.tile([64, BC], BF16, tag="mhT1")
                ptp0 = psB.tile([128, BC], BF16, tag="tp")
                nc.tensor.transpose(ptp0[:], mh_sb[:, 0:128], i8bf_sb[:])
                nc.vector.tensor_copy(mhT0[:], ptp0[:])
                ptp1 = psB.tile([64, BC], BF16, tag="tp")
                nc.tensor.transpose(ptp1[:], mh_sb[:, 128:192], i8bf_sb[:])
                nc.vector.tensor_copy(mhT1[:], ptp1[:])

                # coefs[b, t]
                pcB = psB.tile([BC, T], F32, tag="sm8")
                nc.tensor.matmul(pcB[:], mhT0[:], mw2_sb[:, 0, :],
                                 start=True, stop=False)
                nc.tensor.matmul(pcB[:], mhT1[:], mw2_sb[0:64, 1, :],
                                 start=False, stop=True)
                coefsB = sm.tile([BC, T], F32, tag="coefsB")
                nc.vector.tensor_tensor(coefsB[:], pcB[:], mb2r_sb[:], op=ADD)

                # coefsT[t, b] via PE transpose
                ptc = psB.tile([T, T], F32, tag="sm8")
                nc.tensor.transpose(ptc[:], coefsB[:], i8_sb[:])
                coefsT = sm.tile([T, T], F32, tag="coefsT")
                nc.vector.tensor_copy(coefsT[:], ptc[:])
                coefsT_bf = sm.tile([T, T], BF16, tag="coefsTbf")
                nc.vector.tensor_copy(coefsT_bf[:], ptc[:])

                # crep16[(t,slo), b] = 16*c[t,b]
                pcr = psB.tile([128, T], F32, tag="sm8")
                nc.tensor.matmul(pcr[:], iexp16_sb[:], coefsT[:],
                                 start=True, stop=True)
                crep = sm.tile([128, T], F32, tag="crepsb")
                nc.vector.tensor_copy(crep[:], pcr[:])

                # mixing stationaries cb2_g[(t,slo), shi, (bg,s'32)] fp8
                cb2_0 = sm.tile([128, 2, 128], F8, tag="cb2_0")
                cb2_1 = sm.tile([128, 2, 128], F8, tag="cb2_1")
                cb2 = [cb2_0, cb2_1]
                for g in range(2):
                    for bg in range(4):
                        nc.vector.tensor_scalar_mul(
                            cb2[g][:, :, bg * 32:(bg + 1) * 32],
                            mask32_sb[:], crep[:, g * 4 + bg:g * 4 + bg + 1])

                # cdiag[b', t, b] for layer-2 coef application
                cdiag = sm.tile([T, T, T], BF16, tag="cdiag")
                for t in range(T):
                    nc.vector.tensor_scalar_mul(
                        cdiag[:, t, :], i8bf_sb[:], coefsB[:, t:t + 1])

                # nb1t[j_local, jt, b] = (b1 + coefs @ db1)/(SX*SW)
                nb1t = sm.tile([128, KT, BC], F32, tag="nb1t")
                for jt in range(KT):
                    pb = psB.tile([128, T], F32, tag="sm8")
                    nc.tensor.matmul(pb[:], db1_sb[:, jt * 128:(jt + 1) * 128],
                                     coefsT_bf[:], start=True, stop=True)
                    nc.vector.tensor_scalar_add(
                        nb1t[:, jt, :], pb[:], b1t_sb[:, jt:jt + 1])

            # ---------- phase 3: mixing (fp8 DR, 4 samples x 32 rows) ----------
            # mxg[g][(bg,s'), ph, kh, j] = M[(g,bg), i=(kh*4+ph)*32+s', j]/SW
            mxgp = tc.alloc_tile_pool(name="mxgp", bufs=1)
            mxg_0 = mxgp.tile([128, 4, KT, D], F8, tag="mxg0")
            mxg_1 = mxgp.tile([128, 4, KT, D], F8, tag="mxg1")
            mxg = [mxg_0, mxg_1]
            mxcb_0 = big.tile([128, 4, KTD, 2, D], F8, tag="mxcb0")
            mxcb_1 = big.tile([128, 4, KTD, 2, D], F8, tag="mxcb1")
            mxcb = [mxcb_0, mxcb_1]
            with tc.tile_pool(name="psM", bufs=3, space="PSUM") as psM:
                for g in range(2):
                    for ph in range(4):
                        for kh in range(KT):
                            iblk = kh * 4 + ph
                            pm = psM.tile([128, 2, 512], F32, tag="m")
                            for jh in range(2):
                                nc.tensor.matmul(
                                    pm[:, jh, 0:384], cb2[g][:],
                                    dw1_sb[:, iblk, :, jh * 384:(jh + 1) * 384],
                                    start=True, stop=True, perf_mode=DR)
                            dst = mxg[g][:, ph, kh, :].rearrange(
                                "p (jh j) -> p jh j", jh=2)
                            if kh < 4:
                                # fold W1 in: (pm/16) + W1/SW  (ktd 0,1)
                                nc.vector.scalar_tensor_tensor(
                                    dst, pm[:, :, 0:384], 1.0 / 16.0,
                                    w1i_sb[:, ph, kh, :].rearrange(
                                        "p (jh j) -> p jh j", jh=2),
                                    op0=MULT, op1=ADD)
                            else:
                                nc.scalar.mul(dst, pm[:, :, 0:384], 1.0 / 16.0)
                        # de-interleave gathers for this (g, ph) column
                        for bg in range(4):
                            deq = nc.sync if (bg % 2 == 0) else nc.scalar
                            deq.dma_start(
                                mxcb[g][ph * 32:(ph + 1) * 32, bg, :, :, :],
                                mxg[g][bg * 32:(bg + 1) * 32, ph, :, :])

            mxgp.release()

            # ---------- phase 4: final per-sample pass ----------
            if True:
                with tc.tile_pool(name="psF", bufs=6, space="PSUM") as psF:
                    for b in range(BC):
                        g, bg = b // 4, b % 4
                        for jt in range(KT):
                            pf = psF.tile([128, NPAT], F32, tag="f")
                            for ktd in range(KTD):
                                nc.tensor.matmul(
                                    pf[:],
                                    mxcb[g][:, bg, ktd, :,
                                            jt * 128:(jt + 1) * 128],
                                    xt_sb[:, ktd, :, b * NPAT:(b + 1) * NPAT],
                                    start=(ktd == 0), stop=False,
                                    perf_mode=DR)
                            nc.tensor.matmul(
                                pf[:],
                                w1_sb[:, 2, :, jt * 128:(jt + 1) * 128],
                                xt_sb[:, 2, :, b * NPAT:(b + 1) * NPAT],
                                start=False, stop=True, perf_mode=DR)
                            ro = scr.tile([128, NPAT], F8, tag="ro")
                            if (b + jt) % 2 == 0:
                                nc.scalar.activation(
                                    ro[:], pf[:], RELU,
                                    bias=nb1t[:, jt, b:b + 1],
                                    accum_out=pooln[:, jt, b:b + 1])
                            else:
                                nc.vector.scalar_tensor_tensor(
                                    ro[:], pf[:], nb1t[:, jt, b:b + 1],
                                    zeros_sb[:], op0=ADD, op1=MAX,
                                    accum_out=pooln[:, jt, b:b + 1])

            # ---------- phase 5: layer 2 ----------
            pooln_f8 = sm.tile([128, KTD, 2, 32], F8, tag="poolnf8")
            nc.gpsimd.memset(pooln_f8[:], 0.0)
            nc.scalar.mul(
                pooln_f8[:, :, :, 0:BC].rearrange("p k h b -> p (k h) b"),
                pooln[:], SXW / NPAT)
            pooln_bf = sm.tile([128, KT, BC], BF16, tag="poolnbf")
            nc.gpsimd.tensor_scalar_mul(pooln_bf[:], pooln[:], SXW / NPAT)

            vst = sm.tile([BC, T, D], BF16, tag="vst")
            psV = tc.alloc_tile_pool(name="psV", bufs=2, space="PSUM")
            for t in range(T):
                pv = psV.tile([32, 2, 512], F32, tag="v")
                for eh in range(2):
                    for ktd in range(KTD):
                        nc.tensor.matmul(
                            pv[:, eh, 0:384], pooln_f8[:, ktd, :, :],
                            dw2_sb[:, t, ktd, :, eh * 384:(eh + 1) * 384],
                            start=(ktd == 0), stop=(ktd == KTD - 1),
                            perf_mode=DR)
                if t % 2 == 0:
                    nc.vector.tensor_scalar_mul(
                        vst[:, t, :].rearrange("b (eh e) -> b eh e", eh=2),
                        pv[0:BC, :, 0:384], SW)
                else:
                    nc.scalar.mul(
                        vst[:, t, :].rearrange("b (eh e) -> b eh e", eh=2),
                        pv[0:BC, :, 0:384], SW)

            out_sb = sm.tile([BC, D], F32, tag="out")
            po = psV.tile([BC, 2, 512], F32, tag="v")
            for eh in range(2):
                for kt in range(KT):
                    nc.tensor.matmul(
                        po[:, eh, 0:384], pooln_bf[:, kt, :],
                        w2_sb[:, kt, eh * 384:(eh + 1) * 384],
                        start=(kt == 0), stop=False)
                for t in range(T):
                    nc.tensor.matmul(
                        po[:, eh, 0:384], cdiag[:, t, :],
                        vst[:, t, eh * 384:(eh + 1) * 384],
                        start=False, stop=False)
                nc.tensor.matmul(
                    po[:, eh, 0:384], coefsT_bf[:],
                    db2_sb[:, eh * 384:(eh + 1) * 384],
                    start=False, stop=True)
            nc.vector.tensor_tensor(
                out_sb[:].rearrange("b (eh e) -> b eh e", eh=2),
                po[:, :, 0:384], b2r_sb[:].rearrange("b (eh e) -> b eh e", eh=2),
                op=ADD)
            nc.sync.dma_start(out[:], out_sb[:])
            if debug:
                nc.sync.dma_start(din["dbg_poolb"][:], poolb[:])
                nc.sync.dma_start(din["dbg_coefsB"][:], coefsB[:])
                nc.sync.dma_start(din["dbg_crep"][:], crep[:])
                nc.sync.dma_start(din["dbg_mxcb0"][:], mxcb[0][:])
                nc.sync.dma_start(din["dbg_pooln"][:], pooln[:])
                nc.sync.dma_start(din["dbg_vst"][:], vst[:])
            psV.release()

    if split_waits:
        _split_multi_waits(nc)
    return nc


def prep_inputs(x, W1, b1, W2, b2, dW1, db1, dW2, db2, mw1, mb1, mw2, mb2):
    """Host-side layout prep. Returns per-core in_maps."""
    bf = ml_dtypes.bfloat16
    f8 = ml_dtypes.float8_e4m3
    f32 = np.float32
    x = np.asarray(x, f32); W1 = np.asarray(W1, f32); W2 = np.asarray(W2, f32)
    b1 = np.asarray(b1, f32); b2 = np.asarray(b2, f32)
    dW1 = np.asarray(dW1, f32); dW2 = np.asarray(dW2, f32)
    db1 = np.asarray(db1, f32); db2 = np.asarray(db2, f32)
    mw1 = np.asarray(mw1, f32); mb1 = np.asarray(mb1, f32)
    mw2 = np.asarray(mw2, f32); mb2 = np.asarray(mb2, f32)

    # patches^T: [B, D, NPAT]
    pt = x.reshape(B, 3, 14, P, 14, P).transpose(0, 1, 3, 5, 2, 4)
    pt = np.ascontiguousarray(pt).reshape(B, D, NPAT)

    # shared (replicated) tensors
    w1_c = np.ascontiguousarray(
        (W1 / SW).reshape(KTD, 2, 128, D).transpose(2, 0, 1, 3)).astype(f8)
    w2_c = np.ascontiguousarray(
        W2.reshape(KT, 128, D).transpose(1, 0, 2)).astype(bf)
    # dw1[(t,slo), iblk, shi, j] = dW1[t, iblk*32+shi*16+slo, j]/SW
    d = (dW1 / SW).reshape(T, 24, 2, P, D)       # [t, iblk, shi, slo, j]
    dw1_c = np.ascontiguousarray(
        d.transpose(0, 3, 1, 2, 4).reshape(128, 24, 2, D)).astype(f8)
    # w1i[(bg,s'), ph, kh, j] = W1[(kh*4+ph)*32+s', j]/SW  (kh<4: ktd 0,1)
    w1i_c = np.zeros((128, 4, 4, D), np.float32)
    for ph in range(4):
        for kh in range(4):
            blk = (W1 / SW)[(kh * 4 + ph) * 32:(kh * 4 + ph) * 32 + 32, :]
            for bg in range(4):
                w1i_c[bg * 32:(bg + 1) * 32, ph, kh, :] = blk
    w1i_c = w1i_c.astype(f8)
    # dw2[k_local, t, ktd, hi, e] = dW2[t, ktd*256+hi*128+k_local, e]/SW
    dw2_c = np.ascontiguousarray(
        (dW2 / SW).reshape(T, KTD, 2, 128, D).transpose(3, 0, 1, 2, 4)
    ).astype(f8)
    db1_c = (db1 / SXW).astype(bf)
    db2_c = db2.astype(bf)
    b1t_c = np.ascontiguousarray((b1 / SXW).reshape(KT, 128).T).astype(f32)
    b2t_c = np.ascontiguousarray(b2.reshape(KT, 128).T).astype(f32)
    b2r_c = np.tile(b2, (BC, 1))
    mw1_c = np.ascontiguousarray(
        mw1.reshape(KT, 128, HM).transpose(1, 0, 2)).astype(bf)
    mb1r_c = np.tile(mb1, (BC, 1)).astype(bf)
    mw2_c = np.zeros((128, 2, T), f32)
    mw2_c[:, 0, :] = mw2[:128]
    mw2_c[:64, 1, :] = mw2[128:]
    mw2_c = mw2_c.astype(bf)
    mb2r_c = np.tile(mb2, (BC, 1)).astype(f32)
    iexp16_c = np.repeat(np.eye(T, dtype=f32) * 16.0, P, axis=1)
    # mask32[(t,slo), shi, s'] = (s' == shi*16+slo)
    m32 = np.zeros((P, 2, 32), f32)
    for slo in range(P):
        for shi in range(2):
            m32[slo, shi, shi * P + slo] = 1.0
    mask32_c = np.tile(m32, (T, 1, 1)).astype(bf)
    i8_c = np.eye(T, dtype=f32)

    shared = dict(
        w1=w1_c, w1i=w1i_c, w2=w2_c, dw1=dw1_c, dw2=dw2_c, db1=db1_c,
        db2=db2_c,
        b1t=b1t_c, b2t=b2t_c, b2r=b2r_c, mw1=mw1_c, mb1r=mb1r_c,
        mw2=mw2_c, mb2r=mb2r_c,
        iexp16=iexp16_c, mask32=mask32_c, i8=i8_c, i8bf=i8_c.astype(bf),
    )

    in_maps = []
    for c in range(NCORES):
        ptc = pt[c * BC:(c + 1) * BC]                  # [BC, D, NPAT]
        # xt[p, ktd, hi, (b,n)] = ptc[b, ktd*256+hi*128+p, n]/SX
        xt_c = np.ascontiguousarray(
            (ptc / SX).reshape(BC, KTD, 2, 128, NPAT).transpose(3, 1, 2, 0, 4)
        ).reshape(128, KTD, 2, NB).astype(f8)
        m = dict(shared)
        m["xt"] = xt_c
        in_maps.append(m)
    return in_maps


_NC_CACHE = {}


def kernel(**inputs) -> np.ndarray:
    _apply_tile_patch()
    if "nc" not in _NC_CACHE:
        _NC_CACHE["nc"] = build_kernel()
    nc = _NC_CACHE["nc"]
    in_maps = prep_inputs(**inputs)
    res = run_bass_kernel_spmd(nc, in_maps, core_ids=list(range(NCORES)))
    return np.concatenate([r["out"] for r in res.results], axis=0)


# revision 19
# speedup vs baseline: 1.5379x; 1.0337x over previous
"""MetaNetImageEncoder Trainium2 kernel — fp8 DoubleRow edition.

Data-parallel over batch: 8 samples per NeuronCore x 8 cores.

Per core (sample-local b in 0..7, D=768, N=196 patches, T=8 tasks):
  1. base pass:   A = P @ W1 as fp8 DoubleRow matmuls (K=256 per instr),
                  pooled_b = sum_n relu(A + b1') via ACT/DVE accum_out
                  (b1' = b1/(SX*SW); the fp8 scale folds into later muls)
  2. MetaNet:     coefs via small-stationary matmuls + PE transposes
  3. mixing:      M_b = sum_t c[t,b] dW1[t] with a (t,s32)-packed fp8
                  DoubleRow stationary; 4 samples x 32 i-rows per pass;
                  PSUM evacuated by DVE/GpSimd/ACT round-robin (x1/16)
  4. de-interleave: wide 32-partition DMAs regroup mixing output into
                  per-sample DR-layout stationary tiles
  5. final pass:  pf = P@W1 + P@M_b (6 fp8 DR matmuls, one PSUM chain),
                  relu-pool split between ACT and DVE
  6. layer 2:     out = pooled@W2 (bf16) + sum_t cdiag (pooled@dW2 fp8 DR)
                  + coefs@db2 + b2
"""
import numpy as np
import ml_dtypes

import concourse.bass as bass
import concourse.mybir as mybir
import concourse.tile as tile
from concourse.vector_clock import ScopedClock
from concourse.bass_utils import run_bass_kernel_spmd

F32 = mybir.dt.float32
BF16 = mybir.dt.bfloat16
F8 = mybir.dt.float8e4
RELU = mybir.ActivationFunctionType.Relu
DR = mybir.MatmulPerfMode.DoubleRow
ADD = mybir.AluOpType.add
MAX = mybir.AluOpType.max
MULT = mybir.AluOpType.mult

P = 16
D = 768
T = 8
HM = 192
NPAT = 196          # 14*14 patches
B = 64
NCORES = 8
BC = B // NCORES    # 8 samples per core
NB = BC * NPAT      # 1568
KT = D // 128       # 6 k-tiles
KTD = 3             # 3 double k-tiles

SX = 3.0            # patch fp8 scale
SW = 0.06           # weight fp8 scale
SXW = SX * SW

_PATCHED = False


def _apply_tile_patch():
    """This container's walrus allows only one sem wait per instruction;
    TileContext's exit drain attaches one wait per live semaphore. Split
    them onto standalone single-wait nops."""
    global _PATCHED
    if _PATCHED:
        return
    _PATCHED = True

    def _patched(self, tick_clock, wait_clock):
        carrier = self.nc.sync.nop(nofuse=True, hint="drain_waits")
        wait_clock.add_sem_waits(
            carrier.ins, ScopedClock({None: tick_clock.global_clock})
        )
        si = carrier.ins.sync_info
        waits = list(si.on_wait) if si else []
        if len(waits) > 1:
            carrier.ins.sync_info = mybir.SyncInfo(on_wait=[waits[0]], on_update=[])
            for w in waits[1:]:
                extra = self.nc.sync.nop(nofuse=True, hint="drain_waits")
                extra.ins.sync_info = mybir.SyncInfo(on_wait=[w], on_update=[])
        self.nc.sync.drain()
        self.nc.all_engine_barrier()
        popped = self.nc._tile_sem_poison_stack.pop()
        assert popped is self._sem_poison
        self.nc.clear_and_free_semaphores(list(self.sems.allocated().values()))
        self.nc.all_engine_barrier()

    tile.TileContext._drain_and_barrier = _patched


def _split_multi_waits(nc, max_waits: int = 1):
    """Hoist extra sem waits onto same-engine InstNoOp carriers."""
    for f in nc.m.functions:
        for blk in f.blocks:
            out = []
            for inst in blk.instructions:
                si = inst.sync_info
                if si is not None and len(si.on_wait) > max_waits:
                    waits = list(si.on_wait)
                    for i, w in enumerate(waits[:-max_waits]):
                        out.append(mybir.InstNoOp(
                            name=f"{inst.name}-w{i}",
                            sync_info=mybir.SyncInfo(on_wait=[w], on_update=[]),
                            bass_nofuse=True,
                            engine=inst.engine,
                        ))
                    inst.sync_info = mybir.SyncInfo(
                        on_wait=waits[-max_waits:], on_update=list(si.on_update)
                    )
                out.append(inst)
            blk.instructions = out


def build_kernel(split_waits=True, debug=False):
    nc = bass.Bass(target_bir_lowering=False, trn_type="TRN2")

    din = {}
    def inp(name, shape, dt):
        din[name] = nc.dram_tensor(name, shape, dt, kind="ExternalInput")
        return din[name]

    xt = inp("xt", (128, KTD, 2, NB), F8)        # patches^T/SX, DR layout
    w1 = inp("w1", (128, KTD, 2, D), F8)         # W1/SW, DR layout
    w2 = inp("w2", (128, KT, D), BF16)           # W2 [k_local, kt, e]
    dw1 = inp("dw1", (128, 24, 2, D), F8)        # [(t,slo), iblk, shi, j]
    w1i = inp("w1i", (128, 4, 4, D), F8)         # W1/SW mixing-layout, ktd0/1
    dw2 = inp("dw2", (128, T, KTD, 2, D), F8)    # [k_local, t, ktd, hi, e]
    db1 = inp("db1", (T, D), BF16)               # db1/(SX*SW)
    db2 = inp("db2", (T, D), BF16)
    b1t = inp("b1t", (128, KT), F32)             # b1/(SX*SW) [j_local, jt]
    b2t = inp("b2t", (128, KT), F32)             # b2 [e_local, et]
    b2r = inp("b2r", (BC, D), F32)               # b2 replicated over samples
    mw1 = inp("mw1", (128, KT, HM), BF16)
    mb1r = inp("mb1r", (BC, HM), BF16)           # mb1 replicated over samples
    mw2 = inp("mw2", (128, 2, T), BF16)          # [h_local, g, t], g=1 padded
    mb2r = inp("mb2r", (BC, T), F32)             # mb2 replicated over samples
    iexp16 = inp("iexp16", (T, 128), F32)        # 16*repeat(eye(8),16,axis=1)
    mask32 = inp("mask32", (128, 2, 32), BF16)   # [(t,slo), shi, s'32]
    i8 = inp("i8", (T, T), F32)                  # eye(8)
    i8bf = inp("i8bf", (T, T), BF16)             # eye(8) bf16

    out = nc.dram_tensor("out", (BC, D), F32, kind="ExternalOutput")
    if debug:
        for nm, shp, dt in [
                ("dbg_poolb", (128, KT, BC), F32), ("dbg_coefsB", (BC, T), F32),
                ("dbg_crep", (128, T), F32),
                ("dbg_mxcb0", (128, 4, KTD, 2, D), F8),
                ("dbg_pooln", (128, KT, BC), F32),
                ("dbg_vst", (BC, T, D), BF16)]:
            din[nm] = nc.dram_tensor(nm, shp, dt, kind="ExternalOutput")

    with tile.TileContext(nc) as tc:
        with (
            tc.tile_pool(name="big", bufs=1) as big,
            tc.tile_pool(name="sm", bufs=1) as sm,
            tc.tile_pool(name="scr", bufs=2) as scr,
        ):
            # ---------- persistent loads ----------
            w1_sb = big.tile([128, KTD, 2, D], F8, tag="w1")
            nc.sync.dma_start(w1_sb[:], w1[:])
            xt_sb = big.tile([128, KTD, 2, NB], F8, tag="xt")
            nc.sync.dma_start(xt_sb[:, :, :, 0:784], xt[:, :, :, 0:784])
            nc.sync.dma_start(xt_sb[:, :, :, 784:NB], xt[:, :, :, 784:NB])
            b1t_sb = sm.tile([128, KT], F32, tag="b1t")
            nc.sync.dma_start(b1t_sb[:], b1t[:])
            b2t_sb = sm.tile([128, KT], F32, tag="b2t")
            nc.sync.dma_start(b2t_sb[:], b2t[:])
            b2r_sb = sm.tile([BC, D], F32, tag="b2r")
            nc.sync.dma_start(b2r_sb[:], b2r[:])
            mw1_sb = sm.tile([128, KT, HM], BF16, tag="mw1")
            nc.sync.dma_start(mw1_sb[:], mw1[:])
            mb1r_sb = sm.tile([BC, HM], BF16, tag="mb1r")
            nc.sync.dma_start(mb1r_sb[:], mb1r[:])
            mw2_sb = sm.tile([128, 2, T], BF16, tag="mw2")
            nc.sync.dma_start(mw2_sb[:], mw2[:])
            mb2r_sb = sm.tile([BC, T], F32, tag="mb2r")
            nc.sync.dma_start(mb2r_sb[:], mb2r[:])
            iexp16_sb = sm.tile([T, 128], F32, tag="iexp16")
            nc.sync.dma_start(iexp16_sb[:], iexp16[:])
            mask32_sb = sm.tile([128, 2, 32], BF16, tag="mask32")
            nc.sync.dma_start(mask32_sb[:], mask32[:])
            i8_sb = sm.tile([T, T], F32, tag="i8")
            nc.sync.dma_start(i8_sb[:], i8[:])
            i8bf_sb = sm.tile([T, T], BF16, tag="i8bf")
            nc.sync.dma_start(i8bf_sb[:], i8bf[:])
            db1_sb = sm.tile([T, D], BF16, tag="db1")
            nc.sync.dma_start(db1_sb[:], db1[:])
            db2_sb = sm.tile([T, D], BF16, tag="db2")
            nc.sync.dma_start(db2_sb[:], db2[:])
            # scalar queue: w2 (metanet), then mixing/ph5 tensors
            w2_sb = big.tile([128, KT, D], BF16, tag="w2")
            nc.scalar.dma_start(w2_sb[:], w2[:])
            dw1_sb = big.tile([128, 24, 2, D], F8, tag="dw1")
            nc.scalar.dma_start(dw1_sb[:], dw1[:])
            w1i_sb = big.tile([128, 4, 4, D], F8, tag="w1i")
            nc.scalar.dma_start(w1i_sb[:], w1i[:])
            dw2_sb = big.tile([128, T, KTD, 2, D], F8, tag="dw2")
            nc.scalar.dma_start(dw2_sb[:], dw2[:])

            poolb = sm.tile([128, KT, BC], F32, tag="poolb")
            pooln = sm.tile([128, KT, BC], F32, tag="pooln")
            zeros_sb = sm.tile([128, NPAT], F8, tag="zeros")
            nc.vector.memset(zeros_sb[:], 0.0)

            def relu_pool(pa, bi, jt, b, dst):
                """relu(pa_slice + b1') summed into dst column; ACT or DVE."""
                sl = pa[:, bi * NPAT:(bi + 1) * NPAT]
                if b % 2 == 0:
                    ro = scr.tile([128, NPAT], F8, tag="ro")
                    nc.scalar.activation(
                        ro[:], sl, RELU, bias=b1t_sb[:, jt:jt + 1],
                        accum_out=dst)
                else:
                    ro = scr.tile([128, NPAT], F8, tag="ro")
                    nc.vector.scalar_tensor_tensor(
                        ro[:], sl, b1t_sb[:, jt:jt + 1], zeros_sb[:],
                        op0=ADD, op1=MAX, accum_out=dst)

            # ---------- phase 1: base pass (fp8 DR) ----------
            with tc.tile_pool(name="psA", bufs=4, space="PSUM") as psA:
                for jt in range(KT):
                    for ch in range(4):      # chunks of 392 = 2 samples
                        pa = psA.tile([128, 392], F32, tag="a")
                        for ktd in range(KTD):
                            nc.tensor.matmul(
                                pa[:],
                                w1_sb[:, ktd, :, jt * 128:(jt + 1) * 128],
                                xt_sb[:, ktd, :, ch * 392:(ch + 1) * 392],
                                start=(ktd == 0), stop=(ktd == KTD - 1),
                                perf_mode=DR)
                        for bi in range(2):
                            b = ch * 2 + bi
                            relu_pool(pa, bi, jt, b, poolb[:, jt, b:b + 1])

            # ---------- phase 2: MetaNet ----------
            with tc.tile_pool(name="psB", bufs=1, space="PSUM") as psB:
                poolb_bf = sm.tile([128, KT, BC], BF16, tag="poolbbf")
                nc.scalar.mul(poolb_bf[:], poolb[:], SXW / NPAT)

                # base2 = pooled @ W2 : [8, 768] (small stationary)
                pb2 = psB.tile([8, 2, 512], F32, tag="b2big")
                for eh in range(2):
                    for kt in range(KT):
                        nc.tensor.matmul(
                            pb2[:, eh, 0:384], poolb_bf[:, kt, :],
                            w2_sb[:, kt, eh * 384:(eh + 1) * 384],
                            start=(kt == 0), stop=(kt == KT - 1))
                base2b = sm.tile([BC, D], BF16, tag="base2b")
                nc.vector.tensor_copy(
                    base2b[:].rearrange("b (eh e) -> b eh e", eh=2),
                    pb2[:, :, 0:384])

                # transpose to [e_local, et, b] and add b2 bias
                base2T = sm.tile([128, KT, BC], BF16, tag="base2T")
                for et in range(KT):
                    ptp = psB.tile([128, BC], BF16, tag="tp")
                    nc.tensor.transpose(
                        ptp[:], base2b[:, et * 128:(et + 1) * 128], i8bf_sb[:])
                    nc.vector.tensor_scalar_add(
                        base2T[:, et, :], ptp[:], b2t_sb[:, et:et + 1])

                # mh = relu(base2 @ mw1 + mb1) : [8, 192]
                pmh = psB.tile([BC, HM], F32, tag="mh")
                for et in range(KT):
                    nc.tensor.matmul(
                        pmh[:], base2T[:, et, :], mw1_sb[:, et, :],
                        start=(et == 0), stop=(et == KT - 1))
                mh_sb = sm.tile([BC, HM], BF16, tag="mhsb")
                nc.vector.scalar_tensor_tensor(
                    mh_sb[:], pmh[:], 1.0, mb1r_sb[:], op0=MULT, op1=ADD)
                nc.vector.tensor_scalar_max(mh_sb[:], mh_sb[:], 0.0)

                # transpose mh -> [h_local, b] chunks
                mhT0 = sm.tile([128, BC], BF16, tag="mhT0")
                mhT1 = sm.tile([64, BC], BF16, tag="mhT1")
                ptp0 = psB.tile([128, BC], BF16, tag="tp")
                nc.tensor.transpose(ptp0[:], mh_sb[:, 0:128], i8bf_sb[:])
                nc.vector.tensor_copy(mhT0[:], ptp0[:])
                ptp1 = psB.tile([64, BC], BF16, tag="tp")
                nc.tensor.transpose(ptp1[:], mh_sb[:, 128:192], i8bf_sb[:])
                nc.vector.tensor_copy(mhT1[:], ptp1[:])

                # coefs[b, t]
                pcB = psB.tile([BC, T], F32, tag="sm8")
                nc.tensor.matmul(pcB[:], mhT0[:], mw2_sb[:, 0, :],
                                 start=True, stop=False)
                nc.tensor.matmul(pcB[:], mhT1[:], mw2_sb[0:64, 1, :],
                                 start=False, stop=True)
                coefsB = sm.tile([BC, T], F32, tag="coefsB")
                nc.vector.tensor_tensor(coefsB[:], pcB[:], mb2r_sb[:], op=ADD)

                # coefsT[t, b] via PE transpose
                ptc = psB.tile([T, T], F32, tag="sm8")
                nc.tensor.transpose(ptc[:], coefsB[:], i8_sb[:])
                coefsT = sm.tile([T, T], F32, tag="coefsT")
                nc.vector.tensor_copy(coefsT[:], ptc[:])
                coefsT_bf = sm.tile([T, T], BF16, tag="coefsTbf")
                nc.vector.tensor_copy(coefsT_bf[:], ptc[:])

                # crep16[(t,slo), b] = 16*c[t,b]
                pcr = psB.tile([128, T], F32, tag="sm8")
                nc.tensor.matmul(pcr[:], iexp16_sb[:], coefsT[:],
                                 start=True, stop=True)
                crep = sm.tile([128, T], F32, tag="crepsb")
                nc.vector.tensor_copy(crep[:], pcr[:])

                # mixing stationaries cb2_g[(t,slo), shi, (bg,s'32)] fp8
                cb2_0 = sm.tile([128, 2, 128], F8, tag="cb2_0")
                cb2_1 = sm.tile([128, 2, 128], F8, tag="cb2_1")
                cb2 = [cb2_0, cb2_1]
                for g in range(2):
                    for bg in range(4):
                        nc.vector.tensor_scalar_mul(
                            cb2[g][:, :, bg * 32:(bg + 1) * 32],
                            mask32_sb[:], crep[:, g * 4 + bg:g * 4 + bg + 1])

                # cdiag[b', t, b] for layer-2 coef application
                cdiag = sm.tile([T, T, T], BF16, tag="cdiag")
                for t in range(T):
                    nc.vector.tensor_scalar_mul(
                        cdiag[:, t, :], i8bf_sb[:], coefsB[:, t:t + 1])

                # nb1t[j_local, jt, b] = (b1 + coefs @ db1)/(SX*SW)
                nb1t = sm.tile([128, KT, BC], F32, tag="nb1t")
                for jt in range(KT):
                    pb = psB.tile([128, T], F32, tag="sm8")
                    nc.tensor.matmul(pb[:], db1_sb[:, jt * 128:(jt + 1) * 128],
                                     coefsT_bf[:], start=True, stop=True)
                    nc.vector.tensor_scalar_add(
                        nb1t[:, jt, :], pb[:], b1t_sb[:, jt:jt + 1])

            # ---------- phase 3: mixing (fp8 DR, 4 samples x 32 rows) ----------
            # mxg[g][(bg,s'), ph, kh, j] = M[(g,bg), i=(kh*4+ph)*32+s', j]/SW
            mxgp = tc.alloc_tile_pool(name="mxgp", bufs=1)
            mxg_0 = mxgp.tile([128, 4, KT, D], F8, tag="mxg0")
            mxg_1 = mxgp.tile([128, 4, KT, D], F8, tag="mxg1")
            mxg = [mxg_0, mxg_1]
            mxcb_0 = big.tile([128, 4, KTD, 2, D], F8, tag="mxcb0")
            mxcb_1 = big.tile([128, 4, KTD, 2, D], F8, tag="mxcb1")
            mxcb = [mxcb_0, mxcb_1]
            with tc.tile_pool(name="psM", bufs=4, space="PSUM") as psM:
                for g in range(2):
                    for ph in range(4):
                        for kh in (0, 4, 1, 5, 2, 3):
                            iblk = kh * 4 + ph
                            pm = psM.tile([128, 2, 512], F32, tag="m")
                            for jh in range(2):
                                nc.tensor.matmul(
                                    pm[:, jh, 0:384], cb2[g][:],
                                    dw1_sb[:, iblk, :, jh * 384:(jh + 1) * 384],
                                    start=True, stop=True, perf_mode=DR)
                            dst = mxg[g][:, ph, kh, :].rearrange(
                                "p (jh j) -> p jh j", jh=2)
                            if kh < 4:
                                # fold W1 in: (pm/16) + W1/SW  (ktd 0,1)
                                nc.vector.scalar_tensor_tensor(
                                    dst, pm[:, :, 0:384], 1.0 / 16.0,
                                    w1i_sb[:, ph, kh, :].rearrange(
                                        "p (jh j) -> p jh j", jh=2),
                                    op0=MULT, op1=ADD)
                            else:
                                nc.scalar.mul(dst, pm[:, :, 0:384], 1.0 / 16.0)
                        # de-interleave gathers for this (g, ph) column
                        for bg in range(4):
                            deq = nc.sync
                            deq.dma_start(
                                mxcb[g][ph * 32:(ph + 1) * 32, bg, :, :, :],
                                mxg[g][bg * 32:(bg + 1) * 32, ph, :, :])

            mxgp.release()

            # ---------- phase 4: final per-sample pass ----------
            if True:
                with tc.tile_pool(name="psF", bufs=6, space="PSUM") as psF:
                    for b in range(BC):
                        g, bg = b // 4, b % 4
                        for jt in range(KT):
                            pf = psF.tile([128, NPAT], F32, tag="f")
                            for ktd in range(KTD):
                                nc.tensor.matmul(
                                    pf[:],
                                    mxcb[g][:, bg, ktd, :,
                                            jt * 128:(jt + 1) * 128],
                                    xt_sb[:, ktd, :, b * NPAT:(b + 1) * NPAT],
                                    start=(ktd == 0), stop=False,
                                    perf_mode=DR)
                            nc.tensor.matmul(
                                pf[:],
                                w1_sb[:, 2, :, jt * 128:(jt + 1) * 128],
                                xt_sb[:, 2, :, b * NPAT:(b + 1) * NPAT],
                                start=False, stop=True, perf_mode=DR)
                            ro = scr.tile([128, NPAT], F8, tag="ro")
                            if (b + jt) % 2 == 0:
                                nc.scalar.activation(
                                    ro[:], pf[:], RELU,
                                    bias=nb1t[:, jt, b:b + 1],
                                    accum_out=pooln[:, jt, b:b + 1])
                            else:
                                nc.vector.scalar_tensor_tensor(
                                    ro[:], pf[:], nb1t[:, jt, b:b + 1],
                                    zeros_sb[:], op0=ADD, op1=MAX,
                                    accum_out=pooln[:, jt, b:b + 1])

            # ---------- phase 5: layer 2 ----------
            pooln_f8 = sm.tile([128, KTD, 2, 32], F8, tag="poolnf8")
            nc.gpsimd.memset(pooln_f8[:], 0.0)
            nc.scalar.mul(
                pooln_f8[:, :, :, 0:BC].rearrange("p k h b -> p (k h) b"),
                pooln[:], SXW / NPAT)
            pooln_bf = sm.tile([128, KT, BC], BF16, tag="poolnbf")
            nc.gpsimd.tensor_scalar_mul(pooln_bf[:], pooln[:], SXW / NPAT)

            vst = sm.tile([BC, T, D], BF16, tag="vst")
            psV = tc.alloc_tile_pool(name="psV", bufs=2, space="PSUM")
            for t in range(T):
                pv = psV.tile([32, 2, 512], F32, tag="v")
                for eh in range(2):
                    for ktd in range(KTD):
                        nc.tensor.matmul(
                            pv[:, eh, 0:384], pooln_f8[:, ktd, :, :],
                            dw2_sb[:, t, ktd, :, eh * 384:(eh + 1) * 384],
                            start=(ktd == 0), stop=(ktd == KTD - 1),
                            perf_mode=DR)
                if t % 2 == 0:
                    nc.vector.tensor_scalar_mul(
                        vst[:, t, :].rearrange("b (eh e) -> b eh e", eh=2),
                        pv[0:BC, :, 0:384], SW)
                else:
                    nc.scalar.mul(
                        vst[:, t, :].rearrange("b (eh e) -> b eh e", eh=2),
                        pv[0:BC, :, 0:384], SW)

            out_sb = sm.tile([BC, D], F32, tag="out")
            po = psV.tile([BC, 2, 512], F32, tag="v")
            for eh in range(2):
                for kt in range(KT):
                    nc.tensor.matmul(
                        po[:, eh, 0:384], pooln_bf[:, kt, :],
                        w2_sb[:, kt, eh * 384:(eh + 1) * 384],
                        start=(kt == 0), stop=False)
                for t in range(T):
                    nc.tensor.matmul(
                        po[:, eh, 0:384], cdiag[:, t, :],
                        vst[:, t, eh * 384:(eh + 1) * 384],
                        start=False, stop=False)
                nc.tensor.matmul(
                    po[:, eh, 0:384], coefsT_bf[:],
                    db2_sb[:, eh * 384:(eh + 1) * 384],
                    start=False, stop=True)
            nc.vector.tensor_tensor(
                out_sb[:].rearrange("b (eh e) -> b eh e", eh=2),
                po[:, :, 0:384], b2r_sb[:].rearrange("b (eh e) -> b eh e", eh=2),
                op=ADD)
            nc.sync.dma_start(out[:], out_sb[:])
            if debug:
                nc.sync.dma_start(din["dbg_poolb"][:], poolb[:])
                nc.sync.dma_start(din["dbg_coefsB"][:], coefsB[:])
                nc.sync.dma_start(din["dbg_crep"][:], crep[:])
                nc.sync.dma_start(din["dbg_mxcb0"][:], mxcb[0][:])
                nc.sync.dma_start(din["dbg_pooln"][:], pooln[:])
                nc.sync.dma_start(din["dbg_vst"][:], vst[:])
            psV.release()

    if split_waits:
        _split_multi_waits(nc)
    return nc


def prep_inputs(x, W1, b1, W2, b2, dW1, db1, dW2, db2, mw1, mb1, mw2, mb2):
    """Host-side layout prep. Returns per-core in_maps."""
    bf = ml_dtypes.bfloat16
    f8 = ml_dtypes.float8_e4m3
    f32 = np.float32
    x = np.asarray(x, f32); W1 = np.asarray(W1, f32); W2 = np.asarray(W2, f32)
    b1 = np.asarray(b1, f32); b2 = np.asarray(b2, f32)
    dW1 = np.asarray(dW1, f32); dW2 = np.asarray(dW2, f32)
    db1 = np.asarray(db1, f32); db2 = np.asarray(db2, f32)
    mw1 = np.asarray(mw1, f32); mb1 = np.asarray(mb1, f32)
    mw2 = np.asarray(mw2, f32); mb2 = np.asarray(mb2, f32)

    # patches^T: [B, D, NPAT]
    pt = x.reshape(B, 3, 14, P, 14, P).transpose(0, 1, 3, 5, 2, 4)
    pt = np.ascontiguousarray(pt).reshape(B, D, NPAT)

    # shared (replicated) tensors
    w1_c = np.ascontiguousarray(
        (W1 / SW).reshape(KTD, 2, 128, D).transpose(2, 0, 1, 3)).astype(f8)
    w2_c = np.ascontiguousarray(
        W2.reshape(KT, 128, D).transpose(1, 0, 2)).astype(bf)
    # dw1[(t,slo), iblk, shi, j] = dW1[t, iblk*32+shi*16+slo, j]/SW
    d = (dW1 / SW).reshape(T, 24, 2, P, D)       # [t, iblk, shi, slo, j]
    dw1_c = np.ascontiguousarray(
        d.transpose(0, 3, 1, 2, 4).reshape(128, 24, 2, D)).astype(f8)
    # w1i[(bg,s'), ph, kh, j] = W1[(kh*4+ph)*32+s', j]/SW  (kh<4: ktd 0,1)
    w1i_c = np.zeros((128, 4, 4, D), np.float32)
    for ph in range(4):
        for kh in range(4):
            blk = (W1 / SW)[(kh * 4 + ph) * 32:(kh * 4 + ph) * 32 + 32, :]
            for bg in range(4):
                w1i_c[bg * 32:(bg + 1) * 32, ph, kh, :] = blk
    w1i_c = w1i_c.astype(f8)
    # dw2[k_local, t, ktd, hi, e] = dW2[t, ktd*256+hi*128+k_local, e]/SW
    dw2_c = np.ascontiguousarray(
        (dW2 / SW).reshape(T, KTD, 2, 128, D).transpose(3, 0, 1, 2, 4)
    ).astype(f8)
    db1_c = (db1 / SXW).astype(bf)
    db2_c = db2.astype(bf)
    b1t_c = np.ascontiguousarray((b1 / SXW).reshape(KT, 128).T).astype(f32)
    b2t_c = np.ascontiguousarray(b2.reshape(KT, 128).T).astype(f32)
    b2r_c = np.tile(b2, (BC, 1))
    mw1_c = np.ascontiguousarray(
        mw1.reshape(KT, 128, HM).transpose(1, 0, 2)).astype(bf)
    mb1r_c = np.tile(mb1, (BC, 1)).astype(bf)
    mw2_c = np.zeros((128, 2, T), f32)
    mw2_c[:, 0, :] = mw2[:128]
    mw2_c[:64, 1, :] = mw2[128:]
    mw2_c = mw2_c.astype(bf)
    mb2r_c = np.tile(mb2, (BC, 1)).astype(f32)
    iexp16_c = np.repeat(np.eye(T, dtype=f32) * 16.0, P, axis=1)
    # mask32[(t,slo), shi, s'] = (s' == shi*16+slo)
    m32 = np.zeros((P, 2, 32), f32)
    for slo in range(P):
        for shi in range(2):
            m32[slo, shi, shi * P + slo] = 1.0
    mask32_c = np.tile(m32, (T, 1, 1)).astype(bf)
    i8_c = np.eye(T, dtype=f32)

    shared = dict(
        w1=w1_c, w1i=w1i_c, w2=w2_c, dw1=dw1_c, dw2=dw2_c, db1=db1_c,
        db2=db2_c,
        b1t=b1t_c, b2t=b2t_c, b2r=b2r_c, mw1=mw1_c, mb1r=mb1r_c,
        mw2=mw2_c, mb2r=mb2r_c,
        iexp16=iexp16_c, mask32=mask32_c, i8=i8_c, i8bf=i8_c.astype(bf),
    )

    in_maps = []
    for c in range(NCORES):
        ptc = pt[c * BC:(c + 1) * BC]                  # [BC, D, NPAT]
        # xt[p, ktd, hi, (b,n)] = ptc[b, ktd*256+hi*128+p, n]/SX
        xt_c = np.ascontiguousarray(
            (ptc / SX).reshape(BC, KTD, 2, 128, NPAT).transpose(3, 1, 2, 0, 4)
        ).reshape(128, KTD, 2, NB).astype(f8)
        m = dict(shared)
        m["xt"] = xt_c
        in_maps.append(m)
    return in_maps


_NC_CACHE = {}


def kernel(**inputs) -> np.ndarray:
    _apply_tile_patch()
    if "nc" not in _NC_CACHE:
        _NC_CACHE["nc"] = build_kernel()
    nc = _NC_CACHE["nc"]
    in_maps = prep_inputs(**inputs)
    res = run_bass_kernel_spmd(nc, in_maps, core_ids=list(range(NCORES)))
    return np.concatenate([r["out"] for r in res.results], axis=0)


# revision 20
# speedup vs baseline: 1.8711x; 1.2166x over previous
"""MetaNetImageEncoder Trainium2 kernel — fp8 DoubleRow edition.

Data-parallel over batch: 8 samples per NeuronCore x 8 cores.

Per core (sample-local b in 0..7, D=768, N=196 patches, T=8 tasks):
  1. base pass:   A = P @ W1 as fp8 DoubleRow matmuls (K=256 per instr),
                  pooled_b = sum_n relu(A + b1') via ACT/DVE accum_out
                  (b1' = b1/(SX*SW); the fp8 scale folds into later muls)
  2. MetaNet:     coefs via small-stationary matmuls + PE transposes
  3. mixing:      M_b = sum_t c[t,b] dW1[t] with a (t,s32)-packed fp8
                  DoubleRow stationary; 4 samples x 32 i-rows per pass;
                  PSUM evacuated by DVE/GpSimd/ACT round-robin (x1/16)
  4. de-interleave: wide 32-partition DMAs regroup mixing output into
                  per-sample DR-layout stationary tiles
  5. final pass:  pf = P@W1 + P@M_b (6 fp8 DR matmuls, one PSUM chain),
                  relu-pool split between ACT and DVE
  6. layer 2:     out = pooled@W2 (bf16) + sum_t cdiag (pooled@dW2 fp8 DR)
                  + coefs@db2 + b2
"""
import numpy as np
import ml_dtypes

import concourse.bass as bass
import concourse.mybir as mybir
import concourse.tile as tile
from concourse.vector_clock import ScopedClock
from concourse.bass_utils import run_bass_kernel_spmd

F32 = mybir.dt.float32
BF16 = mybir.dt.bfloat16
F8 = mybir.dt.float8e4
RELU = mybir.ActivationFunctionType.Relu
DR = mybir.MatmulPerfMode.DoubleRow
ADD = mybir.AluOpType.add
MAX = mybir.AluOpType.max
MULT = mybir.AluOpType.mult

P = 16
D = 768
T = 8
HM = 192
NPAT = 196          # 14*14 patches
B = 64
NCORES = 8
BC = B // NCORES    # 8 samples per core
NB = BC * NPAT      # 1568
KT = D // 128       # 6 k-tiles
KTD = 3             # 3 double k-tiles

SX = 3.0            # patch fp8 scale
SW = 0.06           # weight fp8 scale
SXW = SX * SW

_PATCHED = False


def _apply_tile_patch():
    """This container's walrus allows only one sem wait per instruction;
    TileContext's exit drain attaches one wait per live semaphore. Split
    them onto standalone single-wait nops."""
    global _PATCHED
    if _PATCHED:
        return
    _PATCHED = True

    def _patched(self, tick_clock, wait_clock):
        carrier = self.nc.sync.nop(nofuse=True, hint="drain_waits")
        wait_clock.add_sem_waits(
            carrier.ins, ScopedClock({None: tick_clock.global_clock})
        )
        si = carrier.ins.sync_info
        waits = list(si.on_wait) if si else []
        if len(waits) > 1:
            carrier.ins.sync_info = mybir.SyncInfo(on_wait=[waits[0]], on_update=[])
            for w in waits[1:]:
                extra = self.nc.sync.nop(nofuse=True, hint="drain_waits")
                extra.ins.sync_info = mybir.SyncInfo(on_wait=[w], on_update=[])
        self.nc.sync.drain()
        self.nc.all_engine_barrier()
        popped = self.nc._tile_sem_poison_stack.pop()
        assert popped is self._sem_poison
        self.nc.clear_and_free_semaphores(list(self.sems.allocated().values()))
        self.nc.all_engine_barrier()

    tile.TileContext._drain_and_barrier = _patched


def _split_multi_waits(nc, max_waits: int = 1):
    """Hoist extra sem waits onto same-engine InstNoOp carriers."""
    for f in nc.m.functions:
        for blk in f.blocks:
            out = []
            for inst in blk.instructions:
                si = inst.sync_info
                if si is not None and len(si.on_wait) > max_waits:
                    waits = list(si.on_wait)
                    for i, w in enumerate(waits[:-max_waits]):
                        out.append(mybir.InstNoOp(
                            name=f"{inst.name}-w{i}",
                            sync_info=mybir.SyncInfo(on_wait=[w], on_update=[]),
                            bass_nofuse=True,
                            engine=inst.engine,
                        ))
                    inst.sync_info = mybir.SyncInfo(
                        on_wait=waits[-max_waits:], on_update=list(si.on_update)
                    )
                out.append(inst)
            blk.instructions = out


def build_kernel(split_waits=True, debug=False):
    nc = bass.Bass(target_bir_lowering=False, trn_type="TRN2")

    din = {}
    def inp(name, shape, dt):
        din[name] = nc.dram_tensor(name, shape, dt, kind="ExternalInput")
        return din[name]

    xt = inp("xt", (128, KTD, 2, NB), F8)        # patches^T/SX, DR layout
    w1 = inp("w1", (128, KTD, 2, D), F8)         # W1/SW, DR layout
    w2 = inp("w2", (128, KT, D), BF16)           # W2 [k_local, kt, e]
    dw1 = inp("dw1", (128, 24, 2, D), F8)        # [(t,slo), iblk, shi, j]
    w1i = inp("w1i", (128, 4, 2, D), F8)         # W1/SW mixing-layout, ktd0
    dw2 = inp("dw2", (128, T, KTD, 2, D), F8)    # [k_local, t, ktd, hi, e]
    db1 = inp("db1", (T, D), BF16)               # db1/(SX*SW)
    db2 = inp("db2", (T, D), BF16)
    b1t = inp("b1t", (128, KT), F32)             # b1/(SX*SW) [j_local, jt]
    b2t = inp("b2t", (128, KT), F32)             # b2 [e_local, et]
    b2r = inp("b2r", (BC, D), F32)               # b2 replicated over samples
    mw1 = inp("mw1", (128, KT, HM), BF16)
    mb1r = inp("mb1r", (BC, HM), BF16)           # mb1 replicated over samples
    mw2 = inp("mw2", (128, 2, T), BF16)          # [h_local, g, t], g=1 padded
    mb2r = inp("mb2r", (BC, T), F32)             # mb2 replicated over samples
    iexp16 = inp("iexp16", (T, 128), F32)        # 16*repeat(eye(8),16,axis=1)
    mask32 = inp("mask32", (128, 2, 32), BF16)   # [(t,slo), shi, s'32]
    i8 = inp("i8", (T, T), F32)                  # eye(8)
    i8bf = inp("i8bf", (T, T), BF16)             # eye(8) bf16

    out = nc.dram_tensor("out", (BC, D), F32, kind="ExternalOutput")
    if debug:
        for nm, shp, dt in [
                ("dbg_poolb", (128, KT, BC), F32), ("dbg_coefsB", (BC, T), F32),
                ("dbg_crep", (128, T), F32),
                ("dbg_mxcb0", (128, 4, KTD, 2, D), F8),
                ("dbg_pooln", (128, KT, BC), F32),
                ("dbg_vst", (BC, T, D), BF16)]:
            din[nm] = nc.dram_tensor(nm, shp, dt, kind="ExternalOutput")

    with tile.TileContext(nc) as tc:
        with (
            tc.tile_pool(name="big", bufs=1) as big,
            tc.tile_pool(name="sm", bufs=1) as sm,
            tc.tile_pool(name="scr", bufs=2) as scr,
        ):
            # ---------- persistent loads ----------
            w1_sb = big.tile([128, KTD, 2, D], F8, tag="w1")
            nc.sync.dma_start(w1_sb[:], w1[:])
            xt_sb = big.tile([128, KTD, 2, NB], F8, tag="xt")
            nc.sync.dma_start(xt_sb[:], xt[:])
            b1t_sb = sm.tile([128, KT], F32, tag="b1t")
            nc.sync.dma_start(b1t_sb[:], b1t[:])
            b2t_sb = sm.tile([128, KT], F32, tag="b2t")
            nc.sync.dma_start(b2t_sb[:], b2t[:])
            b2r_sb = sm.tile([BC, D], F32, tag="b2r")
            nc.sync.dma_start(b2r_sb[:], b2r[:])
            mw1_sb = sm.tile([128, KT, HM], BF16, tag="mw1")
            nc.sync.dma_start(mw1_sb[:], mw1[:])
            mb1r_sb = sm.tile([BC, HM], BF16, tag="mb1r")
            nc.sync.dma_start(mb1r_sb[:], mb1r[:])
            mw2_sb = sm.tile([128, 2, T], BF16, tag="mw2")
            nc.sync.dma_start(mw2_sb[:], mw2[:])
            mb2r_sb = sm.tile([BC, T], F32, tag="mb2r")
            nc.sync.dma_start(mb2r_sb[:], mb2r[:])
            iexp16_sb = sm.tile([T, 128], F32, tag="iexp16")
            nc.sync.dma_start(iexp16_sb[:], iexp16[:])
            mask32_sb = sm.tile([128, 2, 32], BF16, tag="mask32")
            nc.sync.dma_start(mask32_sb[:], mask32[:])
            i8_sb = sm.tile([T, T], F32, tag="i8")
            nc.sync.dma_start(i8_sb[:], i8[:])
            i8bf_sb = sm.tile([T, T], BF16, tag="i8bf")
            nc.sync.dma_start(i8bf_sb[:], i8bf[:])
            db1_sb = sm.tile([T, D], BF16, tag="db1")
            nc.sync.dma_start(db1_sb[:], db1[:])
            db2_sb = sm.tile([T, D], BF16, tag="db2")
            nc.sync.dma_start(db2_sb[:], db2[:])
            # scalar queue: w2 (metanet), then mixing/ph5 tensors
            w2_sb = big.tile([128, KT, D], BF16, tag="w2")
            nc.scalar.dma_start(w2_sb[:], w2[:])
            dw1_sb = big.tile([128, 24, 2, D], F8, tag="dw1")
            nc.scalar.dma_start(dw1_sb[:], dw1[:])
            w1i_sb = big.tile([128, 4, 2, D], F8, tag="w1i")
            nc.scalar.dma_start(w1i_sb[:], w1i[:])
            dw2_sb = big.tile([128, T, KTD, 2, D], F8, tag="dw2")
            nc.scalar.dma_start(dw2_sb[:], dw2[:])

            poolb = sm.tile([128, KT, BC], F32, tag="poolb")
            pooln = sm.tile([128, KT, BC], F32, tag="pooln")
            zeros_sb = sm.tile([128, NPAT], F8, tag="zeros")
            nc.vector.memset(zeros_sb[:], 0.0)

            def relu_pool(pa, bi, jt, b, dst):
                """relu(pa_slice + b1') summed into dst column; ACT or DVE."""
                sl = pa[:, bi * NPAT:(bi + 1) * NPAT]
                if b % 2 == 0:
                    ro = scr.tile([128, NPAT], F8, tag="ro")
                    nc.scalar.activation(
                        ro[:], sl, RELU, bias=b1t_sb[:, jt:jt + 1],
                        accum_out=dst)
                else:
                    ro = scr.tile([128, NPAT], F8, tag="ro")
                    nc.vector.scalar_tensor_tensor(
                        ro[:], sl, b1t_sb[:, jt:jt + 1], zeros_sb[:],
                        op0=ADD, op1=MAX, accum_out=dst)

            # ---------- phase 1: base pass (fp8 DR) ----------
            with tc.tile_pool(name="psA", bufs=4, space="PSUM") as psA:
                for jt in range(KT):
                    for ch in range(4):      # chunks of 392 = 2 samples
                        pa = psA.tile([128, 392], F32, tag="a")
                        for ktd in range(KTD):
                            nc.tensor.matmul(
                                pa[:],
                                w1_sb[:, ktd, :, jt * 128:(jt + 1) * 128],
                                xt_sb[:, ktd, :, ch * 392:(ch + 1) * 392],
                                start=(ktd == 0), stop=(ktd == KTD - 1),
                                perf_mode=DR)
                        for bi in range(2):
                            b = ch * 2 + bi
                            relu_pool(pa, bi, jt, b, poolb[:, jt, b:b + 1])

            # ---------- phase 2: MetaNet ----------
            with tc.tile_pool(name="psB", bufs=1, space="PSUM") as psB:
                poolb_bf = sm.tile([128, KT, BC], BF16, tag="poolbbf")
                nc.scalar.mul(poolb_bf[:], poolb[:], SXW / NPAT)

                # base2 = pooled @ W2 : [8, 768] (small stationary)
                pb2 = psB.tile([8, 2, 512], F32, tag="b2big")
                for eh in range(2):
                    for kt in range(KT):
                        nc.tensor.matmul(
                            pb2[:, eh, 0:384], poolb_bf[:, kt, :],
                            w2_sb[:, kt, eh * 384:(eh + 1) * 384],
                            start=(kt == 0), stop=(kt == KT - 1))
                base2b = sm.tile([BC, D], BF16, tag="base2b")
                nc.vector.tensor_copy(
                    base2b[:].rearrange("b (eh e) -> b eh e", eh=2),
                    pb2[:, :, 0:384])

                # transpose to [e_local, et, b] and add b2 bias
                base2T = sm.tile([128, KT, BC], BF16, tag="base2T")
                for et in range(KT):
                    ptp = psB.tile([128, BC], BF16, tag="tp")
                    nc.tensor.transpose(
                        ptp[:], base2b[:, et * 128:(et + 1) * 128], i8bf_sb[:])
                    nc.vector.tensor_scalar_add(
                        base2T[:, et, :], ptp[:], b2t_sb[:, et:et + 1])

                # mh = relu(base2 @ mw1 + mb1) : [8, 192]
                pmh = psB.tile([BC, HM], F32, tag="mh")
                for et in range(KT):
                    nc.tensor.matmul(
                        pmh[:], base2T[:, et, :], mw1_sb[:, et, :],
                        start=(et == 0), stop=(et == KT - 1))
                mh_sb = sm.tile([BC, HM], BF16, tag="mhsb")
                nc.vector.scalar_tensor_tensor(
                    mh_sb[:], pmh[:], 1.0, mb1r_sb[:], op0=MULT, op1=ADD)
                nc.vector.tensor_scalar_max(mh_sb[:], mh_sb[:], 0.0)

                # transpose mh -> [h_local, b] chunks
                mhT0 = sm.tile([128, BC], BF16, tag="mhT0")
                mhT1 = sm.tile([64, BC], BF16, tag="mhT1")
                ptp0 = psB.tile([128, BC], BF16, tag="tp")
                nc.tensor.transpose(ptp0[:], mh_sb[:, 0:128], i8bf_sb[:])
                nc.vector.tensor_copy(mhT0[:], ptp0[:])
                ptp1 = psB.tile([64, BC], BF16, tag="tp")
                nc.tensor.transpose(ptp1[:], mh_sb[:, 128:192], i8bf_sb[:])
                nc.vector.tensor_copy(mhT1[:], ptp1[:])

                # coefs[b, t]
                pcB = psB.tile([BC, T], F32, tag="sm8")
                nc.tensor.matmul(pcB[:], mhT0[:], mw2_sb[:, 0, :],
                                 start=True, stop=False)
                nc.tensor.matmul(pcB[:], mhT1[:], mw2_sb[0:64, 1, :],
                                 start=False, stop=True)
                coefsB = sm.tile([BC, T], F32, tag="coefsB")
                nc.vector.tensor_tensor(coefsB[:], pcB[:], mb2r_sb[:], op=ADD)

                # coefsT[t, b] via PE transpose
                ptc = psB.tile([T, T], F32, tag="sm8")
                nc.tensor.transpose(ptc[:], coefsB[:], i8_sb[:])
                coefsT = sm.tile([T, T], F32, tag="coefsT")
                nc.vector.tensor_copy(coefsT[:], ptc[:])
                coefsT_bf = sm.tile([T, T], BF16, tag="coefsTbf")
                nc.vector.tensor_copy(coefsT_bf[:], ptc[:])

                # crep16[(t,slo), b] = 16*c[t,b]
                pcr = psB.tile([128, T], F32, tag="sm8")
                nc.tensor.matmul(pcr[:], iexp16_sb[:], coefsT[:],
                                 start=True, stop=True)
                crep = sm.tile([128, T], F32, tag="crepsb")
                nc.vector.tensor_copy(crep[:], pcr[:])

                # mixing stationaries cb2_g[(t,slo), shi, (bg,s'32)] fp8
                cb2_0 = sm.tile([128, 2, 128], F8, tag="cb2_0")
                cb2_1 = sm.tile([128, 2, 128], F8, tag="cb2_1")
                cb2 = [cb2_0, cb2_1]
                for g in range(2):
                    for bg in range(4):
                        nc.vector.tensor_scalar_mul(
                            cb2[g][:, :, bg * 32:(bg + 1) * 32],
                            mask32_sb[:], crep[:, g * 4 + bg:g * 4 + bg + 1])

                # cdiag[b', t, b] for layer-2 coef application
                cdiag = sm.tile([T, T, T], BF16, tag="cdiag")
                for t in range(T):
                    nc.vector.tensor_scalar_mul(
                        cdiag[:, t, :], i8bf_sb[:], coefsB[:, t:t + 1])

                # nb1t[j_local, jt, b] = (b1 + coefs @ db1)/(SX*SW)
                nb1t = sm.tile([128, KT, BC], F32, tag="nb1t")
                for jt in range(KT):
                    pb = psB.tile([128, T], F32, tag="sm8")
                    nc.tensor.matmul(pb[:], db1_sb[:, jt * 128:(jt + 1) * 128],
                                     coefsT_bf[:], start=True, stop=True)
                    nc.vector.tensor_scalar_add(
                        nb1t[:, jt, :], pb[:], b1t_sb[:, jt:jt + 1])

            # ---------- phase 3: mixing (fp8 DR, 4 samples x 32 rows) ----------
            # mxg[g][(bg,s'), ph, kh, j] = M[(g,bg), i=(kh*4+ph)*32+s', j]/SW
            mxgp = tc.alloc_tile_pool(name="mxgp", bufs=1)
            mxg_0 = mxgp.tile([128, 4, KT, D], F8, tag="mxg0")
            mxg_1 = mxgp.tile([128, 4, KT, D], F8, tag="mxg1")
            mxg = [mxg_0, mxg_1]
            mxcb_0 = big.tile([128, 4, KTD, 2, D], F8, tag="mxcb0")
            mxcb_1 = big.tile([128, 4, KTD, 2, D], F8, tag="mxcb1")
            mxcb = [mxcb_0, mxcb_1]
            with tc.tile_pool(name="psM", bufs=4, space="PSUM") as psM:
                for g in range(2):
                    for ph in range(4):
                        for kh in (0, 2, 1, 3, 4, 5):
                            iblk = kh * 4 + ph
                            pm = psM.tile([128, 2, 512], F32, tag="m")
                            for jh in range(2):
                                nc.tensor.matmul(
                                    pm[:, jh, 0:384], cb2[g][:],
                                    dw1_sb[:, iblk, :, jh * 384:(jh + 1) * 384],
                                    start=True, stop=True, perf_mode=DR)
                            dst = mxg[g][:, ph, kh, :].rearrange(
                                "p (jh j) -> p jh j", jh=2)
                            if kh < 2:
                                # fold W1 in: (pm/16) + W1/SW  (ktd0)
                                nc.vector.scalar_tensor_tensor(
                                    dst, pm[:, :, 0:384], 1.0 / 16.0,
                                    w1i_sb[:, ph, kh, :].rearrange(
                                        "p (jh j) -> p jh j", jh=2),
                                    op0=MULT, op1=ADD)
                            elif kh == 4:
                                nc.vector.tensor_scalar_mul(
                                    dst, pm[:, :, 0:384], 1.0 / 16.0)
                            else:
                                nc.scalar.mul(dst, pm[:, :, 0:384], 1.0 / 16.0)
                        # de-interleave gathers for this (g, ph) column
                        for bg in range(4):
                            deq = nc.sync
                            deq.dma_start(
                                mxcb[g][ph * 32:(ph + 1) * 32, bg, :, :, :],
                                mxg[g][bg * 32:(bg + 1) * 32, ph, :, :])

            mxgp.release()

            # ---------- phase 4: final per-sample pass ----------
            if True:
                with tc.tile_pool(name="psF", bufs=6, space="PSUM") as psF:
                    for b in range(BC):
                        g, bg = b // 4, b % 4
                        for jt in range(KT):
                            pf = psF.tile([128, NPAT], F32, tag="f")
                            for ktd in range(KTD):
                                nc.tensor.matmul(
                                    pf[:],
                                    mxcb[g][:, bg, ktd, :,
                                            jt * 128:(jt + 1) * 128],
                                    xt_sb[:, ktd, :, b * NPAT:(b + 1) * NPAT],
                                    start=(ktd == 0), stop=False,
                                    perf_mode=DR)
                            nc.tensor.matmul(
                                pf[:],
                                w1_sb[:, 1, :, jt * 128:(jt + 1) * 128],
                                xt_sb[:, 1, :, b * NPAT:(b + 1) * NPAT],
                                start=False, stop=False, perf_mode=DR)
                            nc.tensor.matmul(
                                pf[:],
                                w1_sb[:, 2, :, jt * 128:(jt + 1) * 128],
                                xt_sb[:, 2, :, b * NPAT:(b + 1) * NPAT],
                                start=False, stop=True, perf_mode=DR)
                            ro = scr.tile([128, NPAT], F8, tag="ro")
                            if (b + jt) % 2 == 0:
                                nc.scalar.activation(
                                    ro[:], pf[:], RELU,
                                    bias=nb1t[:, jt, b:b + 1],
                                    accum_out=pooln[:, jt, b:b + 1])
                            else:
                                nc.vector.scalar_tensor_tensor(
                                    ro[:], pf[:], nb1t[:, jt, b:b + 1],
                                    zeros_sb[:], op0=ADD, op1=MAX,
                                    accum_out=pooln[:, jt, b:b + 1])

            # ---------- phase 5: layer 2 ----------
            pooln_f8 = sm.tile([128, KTD, 2, 32], F8, tag="poolnf8")
            nc.gpsimd.memset(pooln_f8[:], 0.0)
            nc.scalar.mul(
                pooln_f8[:, :, :, 0:BC].rearrange("p k h b -> p (k h) b"),
                pooln[:], SXW / NPAT)
            pooln_bf = sm.tile([128, KT, BC], BF16, tag="poolnbf")
            nc.gpsimd.tensor_scalar_mul(pooln_bf[:], pooln[:], SXW / NPAT)

            vst = sm.tile([BC, T, D], BF16, tag="vst")
            psV = tc.alloc_tile_pool(name="psV", bufs=2, space="PSUM")
            for t in range(T):
                pv = psV.tile([32, 2, 512], F32, tag="v")
                for eh in range(2):
                    for ktd in range(KTD):
                        nc.tensor.matmul(
                            pv[:, eh, 0:384], pooln_f8[:, ktd, :, :],
                            dw2_sb[:, t, ktd, :, eh * 384:(eh + 1) * 384],
                            start=(ktd == 0), stop=(ktd == KTD - 1),
                            perf_mode=DR)
                if t % 2 == 0:
                    nc.vector.tensor_scalar_mul(
                        vst[:, t, :].rearrange("b (eh e) -> b eh e", eh=2),
                        pv[0:BC, :, 0:384], SW)
                else:
                    nc.scalar.mul(
                        vst[:, t, :].rearrange("b (eh e) -> b eh e", eh=2),
                        pv[0:BC, :, 0:384], SW)

            out_sb = sm.tile([BC, D], F32, tag="out")
            po = psV.tile([BC, 2, 512], F32, tag="v")
            for eh in range(2):
                for kt in range(KT):
                    nc.tensor.matmul(
                        po[:, eh, 0:384], pooln_bf[:, kt, :],
                        w2_sb[:, kt, eh * 384:(eh + 1) * 384],
                        start=(kt == 0), stop=False)
                for t in range(T):
                    nc.tensor.matmul(
                        po[:, eh, 0:384], cdiag[:, t, :],
                        vst[:, t, eh * 384:(eh + 1) * 384],
                        start=False, stop=False)
                nc.tensor.matmul(
                    po[:, eh, 0:384], coefsT_bf[:],
                    db2_sb[:, eh * 384:(eh + 1) * 384],
                    start=False, stop=True)
            nc.vector.tensor_tensor(
                out_sb[:].rearrange("b (eh e) -> b eh e", eh=2),
                po[:, :, 0:384], b2r_sb[:].rearrange("b (eh e) -> b eh e", eh=2),
                op=ADD)
            nc.sync.dma_start(out[:], out_sb[:])
            if debug:
                nc.sync.dma_start(din["dbg_poolb"][:], poolb[:])
                nc.sync.dma_start(din["dbg_coefsB"][:], coefsB[:])
                nc.sync.dma_start(din["dbg_crep"][:], crep[:])
                nc.sync.dma_start(din["dbg_mxcb0"][:], mxcb[0][:])
                nc.sync.dma_start(din["dbg_pooln"][:], pooln[:])
                nc.sync.dma_start(din["dbg_vst"][:], vst[:])
            psV.release()

    if split_waits:
        _split_multi_waits(nc)
    return nc


def prep_inputs(x, W1, b1, W2, b2, dW1, db1, dW2, db2, mw1, mb1, mw2, mb2):
    """Host-side layout prep. Returns per-core in_maps."""
    bf = ml_dtypes.bfloat16
    f8 = ml_dtypes.float8_e4m3
    f32 = np.float32
    x = np.asarray(x, f32); W1 = np.asarray(W1, f32); W2 = np.asarray(W2, f32)
    b1 = np.asarray(b1, f32); b2 = np.asarray(b2, f32)
    dW1 = np.asarray(dW1, f32); dW2 = np.asarray(dW2, f32)
    db1 = np.asarray(db1, f32); db2 = np.asarray(db2, f32)
    mw1 = np.asarray(mw1, f32); mb1 = np.asarray(mb1, f32)
    mw2 = np.asarray(mw2, f32); mb2 = np.asarray(mb2, f32)

    # patches^T: [B, D, NPAT]
    pt = x.reshape(B, 3, 14, P, 14, P).transpose(0, 1, 3, 5, 2, 4)
    pt = np.ascontiguousarray(pt).reshape(B, D, NPAT)

    # shared (replicated) tensors
    w1_c = np.ascontiguousarray(
        (W1 / SW).reshape(KTD, 2, 128, D).transpose(2, 0, 1, 3)).astype(f8)
    w2_c = np.ascontiguousarray(
        W2.reshape(KT, 128, D).transpose(1, 0, 2)).astype(bf)
    # dw1[(t,slo), iblk, shi, j] = dW1[t, iblk*32+shi*16+slo, j]/SW
    d = (dW1 / SW).reshape(T, 24, 2, P, D)       # [t, iblk, shi, slo, j]
    dw1_c = np.ascontiguousarray(
        d.transpose(0, 3, 1, 2, 4).reshape(128, 24, 2, D)).astype(f8)
    # w1i[(bg,s'), ph, kh, j] = W1[(kh*4+ph)*32+s', j]/SW  (kh<2: ktd0)
    w1i_c = np.zeros((128, 4, 2, D), np.float32)
    for ph in range(4):
        for kh in range(2):
            blk = (W1 / SW)[(kh * 4 + ph) * 32:(kh * 4 + ph) * 32 + 32, :]
            for bg in range(4):
                w1i_c[bg * 32:(bg + 1) * 32, ph, kh, :] = blk
    w1i_c = w1i_c.astype(f8)
    # dw2[k_local, t, ktd, hi, e] = dW2[t, ktd*256+hi*128+k_local, e]/SW
    dw2_c = np.ascontiguousarray(
        (dW2 / SW).reshape(T, KTD, 2, 128, D).transpose(3, 0, 1, 2, 4)
    ).astype(f8)
    db1_c = (db1 / SXW).astype(bf)
    db2_c = db2.astype(bf)
    b1t_c = np.ascontiguousarray((b1 / SXW).reshape(KT, 128).T).astype(f32)
    b2t_c = np.ascontiguousarray(b2.reshape(KT, 128).T).astype(f32)
    b2r_c = np.tile(b2, (BC, 1))
    mw1_c = np.ascontiguousarray(
        mw1.reshape(KT, 128, HM).transpose(1, 0, 2)).astype(bf)
    mb1r_c = np.tile(mb1, (BC, 1)).astype(bf)
    mw2_c = np.zeros((128, 2, T), f32)
    mw2_c[:, 0, :] = mw2[:128]
    mw2_c[:64, 1, :] = mw2[128:]
    mw2_c = mw2_c.astype(bf)
    mb2r_c = np.tile(mb2, (BC, 1)).astype(f32)
    iexp16_c = np.repeat(np.eye(T, dtype=f32) * 16.0, P, axis=1)
    # mask32[(t,slo), shi, s'] = (s' == shi*16+slo)
    m32 = np.zeros((P, 2, 32), f32)
    for slo in range(P):
        for shi in range(2):
            m32[slo, shi, shi * P + slo] = 1.0
    mask32_c = np.tile(m32, (T, 1, 1)).astype(bf)
    i8_c = np.eye(T, dtype=f32)

    shared = dict(
        w1=w1_c, w1i=w1i_c, w2=w2_c, dw1=dw1_c, dw2=dw2_c, db1=db1_c,
        db2=db2_c,
        b1t=b1t_c, b2t=b2t_c, b2r=b2r_c, mw1=mw1_c, mb1r=mb1r_c,
        mw2=mw2_c, mb2r=mb2r_c,
        iexp16=iexp16_c, mask32=mask32_c, i8=i8_c, i8bf=i8_c.astype(bf),
    )

    in_maps = []
    for c in range(NCORES):
        ptc = pt[c * BC:(c + 1) * BC]                  # [BC, D, NPAT]
        # xt[p, ktd, hi, (b,n)] = ptc[b, ktd*256+hi*128+p, n]/SX
        xt_c = np.ascontiguousarray(
            (ptc / SX).reshape(BC, KTD, 2, 128, NPAT).transpose(3, 1, 2, 0, 4)
        ).reshape(128, KTD, 2, NB).astype(f8)
        m = dict(shared)
        m["xt"] = xt_c
        in_maps.append(m)
    return in_maps


_NC_CACHE = {}


def kernel(**inputs) -> np.ndarray:
    _apply_tile_patch()
    if "nc" not in _NC_CACHE:
        _NC_CACHE["nc"] = build_kernel()
    nc = _NC_CACHE["nc"]
    in_maps = prep_inputs(**inputs)
    res = run_bass_kernel_spmd(nc, in_maps, core_ids=list(range(NCORES)))
    return np.concatenate([r["out"] for r in res.results], axis=0)


# revision 21
# speedup vs baseline: 1.9414x; 1.0376x over previous
"""MetaNetImageEncoder Trainium2 kernel — fp8 DoubleRow edition.

Data-parallel over batch: 8 samples per NeuronCore x 8 cores.

Per core (sample-local b in 0..7, D=768, N=196 patches, T=8 tasks):
  1. base pass:   A = P @ W1 as fp8 DoubleRow matmuls (K=256 per instr),
                  pooled_b = sum_n relu(A + b1') via ACT/DVE accum_out
                  (b1' = b1/(SX*SW); the fp8 scale folds into later muls)
  2. MetaNet:     coefs via small-stationary matmuls + PE transposes
  3. mixing:      M_b = sum_t c[t,b] dW1[t] with a (t,s32)-packed fp8
                  DoubleRow stationary; 4 samples x 32 i-rows per pass;
                  PSUM evacuated by DVE/GpSimd/ACT round-robin (x1/16)
  4. de-interleave: wide 32-partition DMAs regroup mixing output into
                  per-sample DR-layout stationary tiles
  5. final pass:  pf = P@W1 + P@M_b (6 fp8 DR matmuls, one PSUM chain),
                  relu-pool split between ACT and DVE
  6. layer 2:     out = pooled@W2 (bf16) + sum_t cdiag (pooled@dW2 fp8 DR)
                  + coefs@db2 + b2
"""
import numpy as np
import ml_dtypes

import concourse.bass as bass
import concourse.mybir as mybir
import concourse.tile as tile
from concourse.vector_clock import ScopedClock
from concourse.bass_utils import run_bass_kernel_spmd

F32 = mybir.dt.float32
BF16 = mybir.dt.bfloat16
F8 = mybir.dt.float8e4
RELU = mybir.ActivationFunctionType.Relu
DR = mybir.MatmulPerfMode.DoubleRow
ADD = mybir.AluOpType.add
MAX = mybir.AluOpType.max
MULT = mybir.AluOpType.mult

P = 16
D = 768
T = 8
HM = 192
NPAT = 196          # 14*14 patches
B = 64
NCORES = 8
BC = B // NCORES    # 8 samples per core
NB = BC * NPAT      # 1568
KT = D // 128       # 6 k-tiles
KTD = 3             # 3 double k-tiles

SX = 3.0            # patch fp8 scale
SW = 0.06           # weight fp8 scale
SXW = SX * SW

_PATCHED = False


def _apply_tile_patch():
    """This container's walrus allows only one sem wait per instruction;
    TileContext's exit drain attaches one wait per live semaphore. Split
    them onto standalone single-wait nops."""
    global _PATCHED
    if _PATCHED:
        return
    _PATCHED = True

    def _patched(self, tick_clock, wait_clock):
        carrier = self.nc.sync.nop(nofuse=True, hint="drain_waits")
        wait_clock.add_sem_waits(
            carrier.ins, ScopedClock({None: tick_clock.global_clock})
        )
        si = carrier.ins.sync_info
        waits = list(si.on_wait) if si else []
        if len(waits) > 1:
            carrier.ins.sync_info = mybir.SyncInfo(on_wait=[waits[0]], on_update=[])
            for w in waits[1:]:
                extra = self.nc.sync.nop(nofuse=True, hint="drain_waits")
                extra.ins.sync_info = mybir.SyncInfo(on_wait=[w], on_update=[])
        self.nc.sync.drain()
        self.nc.all_engine_barrier()
        popped = self.nc._tile_sem_poison_stack.pop()
        assert popped is self._sem_poison
        self.nc.clear_and_free_semaphores(list(self.sems.allocated().values()))
        self.nc.all_engine_barrier()

    tile.TileContext._drain_and_barrier = _patched


def _split_multi_waits(nc, max_waits: int = 1):
    """Hoist extra sem waits onto same-engine InstNoOp carriers."""
    for f in nc.m.functions:
        for blk in f.blocks:
            out = []
            for inst in blk.instructions:
                si = inst.sync_info
                if si is not None and len(si.on_wait) > max_waits:
                    waits = list(si.on_wait)
                    for i, w in enumerate(waits[:-max_waits]):
                        out.append(mybir.InstNoOp(
                            name=f"{inst.name}-w{i}",
                            sync_info=mybir.SyncInfo(on_wait=[w], on_update=[]),
                            bass_nofuse=True,
                            engine=inst.engine,
                        ))
                    inst.sync_info = mybir.SyncInfo(
                        on_wait=waits[-max_waits:], on_update=list(si.on_update)
                    )
                out.append(inst)
            blk.instructions = out


def build_kernel(split_waits=True, debug=False):
    nc = bass.Bass(target_bir_lowering=False, trn_type="TRN2")

    din = {}
    def inp(name, shape, dt):
        din[name] = nc.dram_tensor(name, shape, dt, kind="ExternalInput")
        return din[name]

    xt = inp("xt", (128, KTD, 2, NB), F8)        # patches^T/SX, DR layout
    w1 = inp("w1", (128, KTD, 2, D), F8)         # W1/SW, DR layout
    w2 = inp("w2", (128, KT, D), BF16)           # W2 [k_local, kt, e]
    dw1 = inp("dw1", (128, 24, 2, D), F8)        # [(t,slo), iblk, shi, j]
    w1i = inp("w1i", (128, 4, 2, D), F8)         # W1/SW mixing-layout, ktd0
    dw2 = inp("dw2", (128, T, KTD, 2, D), F8)    # [k_local, t, ktd, hi, e]
    db1 = inp("db1", (T, D), BF16)               # db1/(SX*SW)
    db2 = inp("db2", (T, D), BF16)
    b1t = inp("b1t", (128, KT), F32)             # b1/(SX*SW) [j_local, jt]
    b2t = inp("b2t", (128, KT), F32)             # b2 [e_local, et]
    b2r = inp("b2r", (BC, D), F32)               # b2 replicated over samples
    mw1 = inp("mw1", (128, KT, HM), BF16)
    mb1r = inp("mb1r", (BC, HM), BF16)           # mb1 replicated over samples
    mw2 = inp("mw2", (128, 2, T), BF16)          # [h_local, g, t], g=1 padded
    mb2r = inp("mb2r", (BC, T), F32)             # mb2 replicated over samples
    iexp16 = inp("iexp16", (T, 128), F32)        # 16*repeat(eye(8),16,axis=1)
    mask32 = inp("mask32", (128, 2, 32), BF16)   # [(t,slo), shi, s'32]
    i8 = inp("i8", (T, T), F32)                  # eye(8)
    i8bf = inp("i8bf", (T, T), BF16)             # eye(8) bf16

    out = nc.dram_tensor("out", (BC, D), F32, kind="ExternalOutput")
    if debug:
        for nm, shp, dt in [
                ("dbg_poolb", (128, KT, BC), F32), ("dbg_coefsB", (BC, T), F32),
                ("dbg_crep", (128, T), F32),
                ("dbg_mxcb0", (128, 4, KTD, 2, D), F8),
                ("dbg_pooln", (128, KT, BC), F32),
                ("dbg_vst", (BC, T, D), BF16)]:
            din[nm] = nc.dram_tensor(nm, shp, dt, kind="ExternalOutput")

    with tile.TileContext(nc) as tc:
        with (
            tc.tile_pool(name="big", bufs=1) as big,
            tc.tile_pool(name="sm", bufs=1) as sm,
            tc.tile_pool(name="scr", bufs=2) as scr,
        ):
            # ---------- persistent loads ----------
            w1_sb = big.tile([128, KTD, 2, D], F8, tag="w1")
            nc.sync.dma_start(w1_sb[:], w1[:])
            xt_sb = big.tile([128, KTD, 2, NB], F8, tag="xt")
            nc.scalar.dma_start(xt_sb[:], xt[:])
            b1t_sb = sm.tile([128, KT], F32, tag="b1t")
            nc.sync.dma_start(b1t_sb[:], b1t[:])
            b2t_sb = sm.tile([128, KT], F32, tag="b2t")
            nc.sync.dma_start(b2t_sb[:], b2t[:])
            b2r_sb = sm.tile([BC, D], F32, tag="b2r")
            nc.sync.dma_start(b2r_sb[:], b2r[:])
            mw1_sb = sm.tile([128, KT, HM], BF16, tag="mw1")
            nc.sync.dma_start(mw1_sb[:], mw1[:])
            mb1r_sb = sm.tile([BC, HM], BF16, tag="mb1r")
            nc.sync.dma_start(mb1r_sb[:], mb1r[:])
            mw2_sb = sm.tile([128, 2, T], BF16, tag="mw2")
            nc.sync.dma_start(mw2_sb[:], mw2[:])
            mb2r_sb = sm.tile([BC, T], F32, tag="mb2r")
            nc.sync.dma_start(mb2r_sb[:], mb2r[:])
            iexp16_sb = sm.tile([T, 128], F32, tag="iexp16")
            nc.sync.dma_start(iexp16_sb[:], iexp16[:])
            mask32_sb = sm.tile([128, 2, 32], BF16, tag="mask32")
            nc.sync.dma_start(mask32_sb[:], mask32[:])
            i8_sb = sm.tile([T, T], F32, tag="i8")
            nc.sync.dma_start(i8_sb[:], i8[:])
            i8bf_sb = sm.tile([T, T], BF16, tag="i8bf")
            nc.sync.dma_start(i8bf_sb[:], i8bf[:])
            db1_sb = sm.tile([T, D], BF16, tag="db1")
            nc.sync.dma_start(db1_sb[:], db1[:])
            db2_sb = sm.tile([T, D], BF16, tag="db2")
            nc.sync.dma_start(db2_sb[:], db2[:])
            # scalar queue: w2 (metanet), then mixing/ph5 tensors
            w2_sb = big.tile([128, KT, D], BF16, tag="w2")
            nc.scalar.dma_start(w2_sb[:], w2[:])
            dw1_sb = big.tile([128, 24, 2, D], F8, tag="dw1")
            nc.scalar.dma_start(dw1_sb[:], dw1[:])
            w1i_sb = big.tile([128, 4, 2, D], F8, tag="w1i")
            nc.scalar.dma_start(w1i_sb[:], w1i[:])
            dw2_sb = big.tile([128, T, KTD, 2, D], F8, tag="dw2")
            nc.scalar.dma_start(dw2_sb[:], dw2[:])

            poolb = sm.tile([128, KT, BC], F32, tag="poolb")
            pooln = sm.tile([128, KT, BC], F32, tag="pooln")
            zeros_sb = sm.tile([128, NPAT], F8, tag="zeros")
            nc.vector.memset(zeros_sb[:], 0.0)

            def relu_pool(pa, bi, jt, b, dst):
                """relu(pa_slice + b1') summed into dst column; ACT or DVE."""
                sl = pa[:, bi * NPAT:(bi + 1) * NPAT]
                if b % 2 == 0:
                    ro = scr.tile([128, NPAT], F8, tag="ro")
                    nc.scalar.activation(
                        ro[:], sl, RELU, bias=b1t_sb[:, jt:jt + 1],
                        accum_out=dst)
                else:
                    ro = scr.tile([128, NPAT], F8, tag="ro")
                    nc.vector.scalar_tensor_tensor(
                        ro[:], sl, b1t_sb[:, jt:jt + 1], zeros_sb[:],
                        op0=ADD, op1=MAX, accum_out=dst)

            # ---------- phase 1: base pass (fp8 DR) ----------
            with tc.tile_pool(name="psA", bufs=4, space="PSUM") as psA:
                for jt in range(KT):
                    for ch in range(4):      # chunks of 392 = 2 samples
                        pa = psA.tile([128, 392], F32, tag="a")
                        for ktd in range(KTD):
                            nc.tensor.matmul(
                                pa[:],
                                w1_sb[:, ktd, :, jt * 128:(jt + 1) * 128],
                                xt_sb[:, ktd, :, ch * 392:(ch + 1) * 392],
                                start=(ktd == 0), stop=(ktd == KTD - 1),
                                perf_mode=DR)
                        ro = scr.tile([128, 2, NPAT], BF16, tag="rr")
                        if ch % 4 != 3:   # 18 ACT / 6 DVE relus
                            nc.scalar.activation(
                                ro[:], pa[:].rearrange("p (b n) -> p b n", b=2),
                                RELU, bias=b1t_sb[:, jt:jt + 1])
                        else:
                            nc.vector.tensor_scalar(
                                ro[:], pa[:].rearrange("p (b n) -> p b n", b=2),
                                b1t_sb[:, jt:jt + 1], 0.0, op0=ADD, op1=MAX)
                        nc.vector.tensor_reduce(
                            poolb[:, jt, ch * 2:(ch + 1) * 2], ro[:],
                            axis=mybir.AxisListType.X, op=ADD)

            # ---------- phase 2: MetaNet ----------
            with tc.tile_pool(name="psB", bufs=1, space="PSUM") as psB:
                poolb_bf = sm.tile([128, KT, BC], BF16, tag="poolbbf")
                nc.scalar.mul(poolb_bf[:], poolb[:], SXW / NPAT)

                # base2 = pooled @ W2 : [8, 768] (small stationary)
                pb2 = psB.tile([8, 2, 512], F32, tag="b2big")
                for eh in range(2):
                    for kt in range(KT):
                        nc.tensor.matmul(
                            pb2[:, eh, 0:384], poolb_bf[:, kt, :],
                            w2_sb[:, kt, eh * 384:(eh + 1) * 384],
                            start=(kt == 0), stop=(kt == KT - 1))
                base2b = sm.tile([BC, D], BF16, tag="base2b")
                nc.vector.tensor_copy(
                    base2b[:].rearrange("b (eh e) -> b eh e", eh=2),
                    pb2[:, :, 0:384])

                # transpose to [e_local, et, b] and add b2 bias
                base2T = sm.tile([128, KT, BC], BF16, tag="base2T")
                for et in range(KT):
                    ptp = psB.tile([128, BC], BF16, tag="tp")
                    nc.tensor.transpose(
                        ptp[:], base2b[:, et * 128:(et + 1) * 128], i8bf_sb[:])
                    nc.vector.tensor_scalar_add(
                        base2T[:, et, :], ptp[:], b2t_sb[:, et:et + 1])

                # mh = relu(base2 @ mw1 + mb1) : [8, 192]
                pmh = psB.tile([BC, HM], F32, tag="mh")
                for et in range(KT):
                    nc.tensor.matmul(
                        pmh[:], base2T[:, et, :], mw1_sb[:, et, :],
                        start=(et == 0), stop=(et == KT - 1))
                mh_sb = sm.tile([BC, HM], BF16, tag="mhsb")
                nc.vector.scalar_tensor_tensor(
                    mh_sb[:], pmh[:], 1.0, mb1r_sb[:], op0=MULT, op1=ADD)
                nc.vector.tensor_scalar_max(mh_sb[:], mh_sb[:], 0.0)

                # transpose mh -> [h_local, b] chunks
                mhT0 = sm.tile([128, BC], BF16, tag="mhT0")
                mhT1 = sm.tile([64, BC], BF16, tag="mhT1")
                ptp0 = psB.tile([128, BC], BF16, tag="tp")
                nc.tensor.transpose(ptp0[:], mh_sb[:, 0:128], i8bf_sb[:])
                nc.vector.tensor_copy(mhT0[:], ptp0[:])
                ptp1 = psB.tile([64, BC], BF16, tag="tp")
                nc.tensor.transpose(ptp1[:], mh_sb[:, 128:192], i8bf_sb[:])
                nc.vector.tensor_copy(mhT1[:], ptp1[:])

                # coefs[b, t]
                pcB = psB.tile([BC, T], F32, tag="sm8")
                nc.tensor.matmul(pcB[:], mhT0[:], mw2_sb[:, 0, :],
                                 start=True, stop=False)
                nc.tensor.matmul(pcB[:], mhT1[:], mw2_sb[0:64, 1, :],
                                 start=False, stop=True)
                coefsB = sm.tile([BC, T], F32, tag="coefsB")
                nc.vector.tensor_tensor(coefsB[:], pcB[:], mb2r_sb[:], op=ADD)

                # coefsT[t, b] via PE transpose
                ptc = psB.tile([T, T], F32, tag="sm8")
                nc.tensor.transpose(ptc[:], coefsB[:], i8_sb[:])
                coefsT = sm.tile([T, T], F32, tag="coefsT")
                nc.vector.tensor_copy(coefsT[:], ptc[:])
                coefsT_bf = sm.tile([T, T], BF16, tag="coefsTbf")
                nc.vector.tensor_copy(coefsT_bf[:], ptc[:])

                # crep16[(t,slo), b] = 16*c[t,b]
                pcr = psB.tile([128, T], F32, tag="sm8")
                nc.tensor.matmul(pcr[:], iexp16_sb[:], coefsT[:],
                                 start=True, stop=True)
                crep = sm.tile([128, T], F32, tag="crepsb")
                nc.vector.tensor_copy(crep[:], pcr[:])

                # mixing stationaries cb2_g[(t,slo), shi, (bg,s'32)] fp8
                cb2_0 = sm.tile([128, 2, 128], F8, tag="cb2_0")
                cb2_1 = sm.tile([128, 2, 128], F8, tag="cb2_1")
                cb2 = [cb2_0, cb2_1]
                for g in range(2):
                    for bg in range(4):
                        nc.vector.tensor_scalar_mul(
                            cb2[g][:, :, bg * 32:(bg + 1) * 32],
                            mask32_sb[:], crep[:, g * 4 + bg:g * 4 + bg + 1])

                # cdiag[b', t, b] for layer-2 coef application
                cdiag = sm.tile([T, T, T], BF16, tag="cdiag")
                for t in range(T):
                    nc.vector.tensor_scalar_mul(
                        cdiag[:, t, :], i8bf_sb[:], coefsB[:, t:t + 1])

                # nb1t[j_local, jt, b] = (b1 + coefs @ db1)/(SX*SW)
                nb1t = sm.tile([128, KT, BC], F32, tag="nb1t")
                for jt in range(KT):
                    pb = psB.tile([128, T], F32, tag="sm8")
                    nc.tensor.matmul(pb[:], db1_sb[:, jt * 128:(jt + 1) * 128],
                                     coefsT_bf[:], start=True, stop=True)
                    nc.vector.tensor_scalar_add(
                        nb1t[:, jt, :], pb[:], b1t_sb[:, jt:jt + 1])

            # ---------- phase 3: mixing (fp8 DR, 4 samples x 32 rows) ----------
            # mxg[g][(bg,s'), ph, kh, j] = M[(g,bg), i=(kh*4+ph)*32+s', j]/SW
            mxgp = tc.alloc_tile_pool(name="mxgp", bufs=1)
            mxg_0 = mxgp.tile([128, 4, KT, D], F8, tag="mxg0")
            mxg_1 = mxgp.tile([128, 4, KT, D], F8, tag="mxg1")
            mxg = [mxg_0, mxg_1]
            mxcb_0 = big.tile([128, 4, KTD, 2, D], F8, tag="mxcb0")
            mxcb_1 = big.tile([128, 4, KTD, 2, D], F8, tag="mxcb1")
            mxcb = [mxcb_0, mxcb_1]
            with tc.tile_pool(name="psM", bufs=4, space="PSUM") as psM:
                for g in range(2):
                    for ph in range(4):
                        for kh in (0, 2, 1, 3, 4, 5):
                            iblk = kh * 4 + ph
                            pm = psM.tile([128, 2, 512], F32, tag="m")
                            for jh in range(2):
                                nc.tensor.matmul(
                                    pm[:, jh, 0:384], cb2[g][:],
                                    dw1_sb[:, iblk, :, jh * 384:(jh + 1) * 384],
                                    start=True, stop=True, perf_mode=DR)
                            dst = mxg[g][:, ph, kh, :].rearrange(
                                "p (jh j) -> p jh j", jh=2)
                            if kh < 2:
                                # fold W1 in: (pm/16) + W1/SW  (ktd0)
                                nc.vector.scalar_tensor_tensor(
                                    dst, pm[:, :, 0:384], 1.0 / 16.0,
                                    w1i_sb[:, ph, kh, :].rearrange(
                                        "p (jh j) -> p jh j", jh=2),
                                    op0=MULT, op1=ADD)
                            elif kh == 4:
                                nc.vector.tensor_scalar_mul(
                                    dst, pm[:, :, 0:384], 1.0 / 16.0)
                            else:
                                nc.scalar.mul(dst, pm[:, :, 0:384], 1.0 / 16.0)
                        # de-interleave gathers for this (g, ph) column
                        for bg in range(4):
                            deq = nc.sync
                            deq.dma_start(
                                mxcb[g][ph * 32:(ph + 1) * 32, bg, :, :, :],
                                mxg[g][bg * 32:(bg + 1) * 32, ph, :, :])

            mxgp.release()

            # ---------- phase 4: final per-sample pass ----------
            if True:
                with tc.tile_pool(name="psF", bufs=6, space="PSUM") as psF:
                    for b in range(BC):
                        g, bg = b // 4, b % 4
                        for jt in range(KT):
                            pf = psF.tile([128, NPAT], F32, tag="f")
                            for ktd in range(KTD):
                                nc.tensor.matmul(
                                    pf[:],
                                    mxcb[g][:, bg, ktd, :,
                                            jt * 128:(jt + 1) * 128],
                                    xt_sb[:, ktd, :, b * NPAT:(b + 1) * NPAT],
                                    start=(ktd == 0), stop=False,
                                    perf_mode=DR)
                            nc.tensor.matmul(
                                pf[:],
                                w1_sb[:, 1, :, jt * 128:(jt + 1) * 128],
                                xt_sb[:, 1, :, b * NPAT:(b + 1) * NPAT],
                                start=False, stop=False, perf_mode=DR)
                            nc.tensor.matmul(
                                pf[:],
                                w1_sb[:, 2, :, jt * 128:(jt + 1) * 128],
                                xt_sb[:, 2, :, b * NPAT:(b + 1) * NPAT],
                                start=False, stop=True, perf_mode=DR)
                            ro = scr.tile([128, NPAT], F8, tag="ro")
                            if (b * KT + jt) % 8 < 3:   # 18 ACT / 30 DVE
                                nc.scalar.activation(
                                    ro[:], pf[:], RELU,
                                    bias=nb1t[:, jt, b:b + 1],
                                    accum_out=pooln[:, jt, b:b + 1])
                            else:
                                nc.vector.scalar_tensor_tensor(
                                    ro[:], pf[:], nb1t[:, jt, b:b + 1],
                                    zeros_sb[:], op0=ADD, op1=MAX,
                                    accum_out=pooln[:, jt, b:b + 1])

            # ---------- phase 5: layer 2 ----------
            pooln_f8 = sm.tile([128, KTD, 2, 32], F8, tag="poolnf8")
            nc.gpsimd.memset(pooln_f8[:], 0.0)
            nc.scalar.mul(
                pooln_f8[:, :, :, 0:BC].rearrange("p k h b -> p (k h) b"),
                pooln[:], SXW / NPAT)
            pooln_bf = sm.tile([128, KT, BC], BF16, tag="poolnbf")
            nc.gpsimd.tensor_scalar_mul(pooln_bf[:], pooln[:], SXW / NPAT)

            vst = sm.tile([BC, T, D], BF16, tag="vst")
            psV = tc.alloc_tile_pool(name="psV", bufs=2, space="PSUM")
            for t in range(T):
                pv = psV.tile([32, 2, 512], F32, tag="v")
                for eh in range(2):
                    for ktd in range(KTD):
                        nc.tensor.matmul(
                            pv[:, eh, 0:384], pooln_f8[:, ktd, :, :],
                            dw2_sb[:, t, ktd, :, eh * 384:(eh + 1) * 384],
                            start=(ktd == 0), stop=(ktd == KTD - 1),
                            perf_mode=DR)
                if t % 2 == 0:
                    nc.vector.tensor_scalar_mul(
                        vst[:, t, :].rearrange("b (eh e) -> b eh e", eh=2),
                        pv[0:BC, :, 0:384], SW)
                else:
                    nc.scalar.mul(
                        vst[:, t, :].rearrange("b (eh e) -> b eh e", eh=2),
                        pv[0:BC, :, 0:384], SW)

            out_sb = sm.tile([BC, D], F32, tag="out")
            po = psV.tile([BC, 2, 512], F32, tag="v")
            for eh in range(2):
                for kt in range(KT):
                    nc.tensor.matmul(
                        po[:, eh, 0:384], pooln_bf[:, kt, :],
                        w2_sb[:, kt, eh * 384:(eh + 1) * 384],
                        start=(kt == 0), stop=False)
                for t in range(T):
                    nc.tensor.matmul(
                        po[:, eh, 0:384], cdiag[:, t, :],
                        vst[:, t, eh * 384:(eh + 1) * 384],
                        start=False, stop=False)
                nc.tensor.matmul(
                    po[:, eh, 0:384], coefsT_bf[:],
                    db2_sb[:, eh * 384:(eh + 1) * 384],
                    start=False, stop=True)
            nc.vector.tensor_tensor(
                out_sb[:].rearrange("b (eh e) -> b eh e", eh=2),
                po[:, :, 0:384], b2r_sb[:].rearrange("b (eh e) -> b eh e", eh=2),
                op=ADD)
            nc.sync.dma_start(out[:], out_sb[:])
            if debug:
                nc.sync.dma_start(din["dbg_poolb"][:], poolb[:])
                nc.sync.dma_start(din["dbg_coefsB"][:], coefsB[:])
                nc.sync.dma_start(din["dbg_crep"][:], crep[:])
                nc.sync.dma_start(din["dbg_mxcb0"][:], mxcb[0][:])
                nc.sync.dma_start(din["dbg_pooln"][:], pooln[:])
                nc.sync.dma_start(din["dbg_vst"][:], vst[:])
            psV.release()

    if split_waits:
        _split_multi_waits(nc)
    return nc


def prep_inputs(x, W1, b1, W2, b2, dW1, db1, dW2, db2, mw1, mb1, mw2, mb2):
    """Host-side layout prep. Returns per-core in_maps."""
    bf = ml_dtypes.bfloat16
    f8 = ml_dtypes.float8_e4m3
    f32 = np.float32
    x = np.asarray(x, f32); W1 = np.asarray(W1, f32); W2 = np.asarray(W2, f32)
    b1 = np.asarray(b1, f32); b2 = np.asarray(b2, f32)
    dW1 = np.asarray(dW1, f32); dW2 = np.asarray(dW2, f32)
    db1 = np.asarray(db1, f32); db2 = np.asarray(db2, f32)
    mw1 = np.asarray(mw1, f32); mb1 = np.asarray(mb1, f32)
    mw2 = np.asarray(mw2, f32); mb2 = np.asarray(mb2, f32)

    # patches^T: [B, D, NPAT]
    pt = x.reshape(B, 3, 14, P, 14, P).transpose(0, 1, 3, 5, 2, 4)
    pt = np.ascontiguousarray(pt).reshape(B, D, NPAT)

    # shared (replicated) tensors
    w1_c = np.ascontiguousarray(
        (W1 / SW).reshape(KTD, 2, 128, D).transpose(2, 0, 1, 3)).astype(f8)
    w2_c = np.ascontiguousarray(
        W2.reshape(KT, 128, D).transpose(1, 0, 2)).astype(bf)
    # dw1[(t,slo), iblk, shi, j] = dW1[t, iblk*32+shi*16+slo, j]/SW
    d = (dW1 / SW).reshape(T, 24, 2, P, D)       # [t, iblk, shi, slo, j]
    dw1_c = np.ascontiguousarray(
        d.transpose(0, 3, 1, 2, 4).reshape(128, 24, 2, D)).astype(f8)
    # w1i[(bg,s'), ph, kh, j] = W1[(kh*4+ph)*32+s', j]/SW  (kh<2: ktd0)
    w1i_c = np.zeros((128, 4, 2, D), np.float32)
    for ph in range(4):
        for kh in range(2):
            blk = (W1 / SW)[(kh * 4 + ph) * 32:(kh * 4 + ph) * 32 + 32, :]
            for bg in range(4):
                w1i_c[bg * 32:(bg + 1) * 32, ph, kh, :] = blk
    w1i_c = w1i_c.astype(f8)
    # dw2[k_local, t, ktd, hi, e] = dW2[t, ktd*256+hi*128+k_local, e]/SW
    dw2_c = np.ascontiguousarray(
        (dW2 / SW).reshape(T, KTD, 2, 128, D).transpose(3, 0, 1, 2, 4)
    ).astype(f8)
    db1_c = (db1 / SXW).astype(bf)
    db2_c = db2.astype(bf)
    b1t_c = np.ascontiguousarray((b1 / SXW).reshape(KT, 128).T).astype(f32)
    b2t_c = np.ascontiguousarray(b2.reshape(KT, 128).T).astype(f32)
    b2r_c = np.tile(b2, (BC, 1))
    mw1_c = np.ascontiguousarray(
        mw1.reshape(KT, 128, HM).transpose(1, 0, 2)).astype(bf)
    mb1r_c = np.tile(mb1, (BC, 1)).astype(bf)
    mw2_c = np.zeros((128, 2, T), f32)
    mw2_c[:, 0, :] = mw2[:128]
    mw2_c[:64, 1, :] = mw2[128:]
    mw2_c = mw2_c.astype(bf)
    mb2r_c = np.tile(mb2, (BC, 1)).astype(f32)
    iexp16_c = np.repeat(np.eye(T, dtype=f32) * 16.0, P, axis=1)
    # mask32[(t,slo), shi, s'] = (s' == shi*16+slo)
    m32 = np.zeros((P, 2, 32), f32)
    for slo in range(P):
        for shi in range(2):
            m32[slo, shi, shi * P + slo] = 1.0
    mask32_c = np.tile(m32, (T, 1, 1)).astype(bf)
    i8_c = np.eye(T, dtype=f32)

    shared = dict(
        w1=w1_c, w1i=w1i_c, w2=w2_c, dw1=dw1_c, dw2=dw2_c, db1=db1_c,
        db2=db2_c,
        b1t=b1t_c, b2t=b2t_c, b2r=b2r_c, mw1=mw1_c, mb1r=mb1r_c,
        mw2=mw2_c, mb2r=mb2r_c,
        iexp16=iexp16_c, mask32=mask32_c, i8=i8_c, i8bf=i8_c.astype(bf),
    )

    in_maps = []
    for c in range(NCORES):
        ptc = pt[c * BC:(c + 1) * BC]                  # [BC, D, NPAT]
        # xt[p, ktd, hi, (b,n)] = ptc[b, ktd*256+hi*128+p, n]/SX
        xt_c = np.ascontiguousarray(
            (ptc / SX).reshape(BC, KTD, 2, 128, NPAT).transpose(3, 1, 2, 0, 4)
        ).reshape(128, KTD, 2, NB).astype(f8)
        m = dict(shared)
        m["xt"] = xt_c
        in_maps.append(m)
    return in_maps


_NC_CACHE = {}


def kernel(**inputs) -> np.ndarray:
    _apply_tile_patch()
    if "nc" not in _NC_CACHE:
        _NC_CACHE["nc"] = build_kernel()
    nc = _NC_CACHE["nc"]
    in_maps = prep_inputs(**inputs)
    res = run_bass_kernel_spmd(nc, in_maps, core_ids=list(range(NCORES)))
    return np.concatenate([r["out"] for r in res.results], axis=0)


# revision 23
# speedup vs baseline: 1.9645x; 1.0119x over previous
"""MetaNetImageEncoder Trainium2 kernel — fp8 DoubleRow edition.

Data-parallel over batch: 8 samples per NeuronCore x 8 cores.

Per core (sample-local b in 0..7, D=768, N=196 patches, T=8 tasks):
  1. base pass:   A = P @ W1 as fp8 DoubleRow matmuls (K=256 per instr),
                  pooled_b = sum_n relu(A + b1') via ACT/DVE accum_out
                  (b1' = b1/(SX*SW); the fp8 scale folds into later muls)
  2. MetaNet:     coefs via small-stationary matmuls + PE transposes
  3. mixing:      M_b = sum_t c[t,b] dW1[t] with a (t,s32)-packed fp8
                  DoubleRow stationary; 4 samples x 32 i-rows per pass;
                  PSUM evacuated by DVE/GpSimd/ACT round-robin (x1/16)
  4. de-interleave: wide 32-partition DMAs regroup mixing output into
                  per-sample DR-layout stationary tiles
  5. final pass:  pf = P@W1 + P@M_b (6 fp8 DR matmuls, one PSUM chain),
                  relu-pool split between ACT and DVE
  6. layer 2:     out = pooled@W2 (bf16) + sum_t cdiag (pooled@dW2 fp8 DR)
                  + coefs@db2 + b2
"""
import numpy as np
import ml_dtypes

import concourse.bass as bass
import concourse.mybir as mybir
import concourse.tile as tile
from concourse.vector_clock import ScopedClock
from concourse.bass_utils import run_bass_kernel_spmd

F32 = mybir.dt.float32
BF16 = mybir.dt.bfloat16
F8 = mybir.dt.float8e4
RELU = mybir.ActivationFunctionType.Relu
DR = mybir.MatmulPerfMode.DoubleRow
ADD = mybir.AluOpType.add
MAX = mybir.AluOpType.max
MULT = mybir.AluOpType.mult

P = 16
D = 768
T = 8
HM = 192
NPAT = 196          # 14*14 patches
B = 64
NCORES = 8
BC = B // NCORES    # 8 samples per core
NB = BC * NPAT      # 1568
KT = D // 128       # 6 k-tiles
KTD = 3             # 3 double k-tiles

SX = 3.0            # patch fp8 scale
SW = 0.06           # weight fp8 scale
SXW = SX * SW

_PATCHED = False


def _apply_tile_patch():
    """This container's walrus allows only one sem wait per instruction;
    TileContext's exit drain attaches one wait per live semaphore. Split
    them onto standalone single-wait nops."""
    global _PATCHED
    if _PATCHED:
        return
    _PATCHED = True

    def _patched(self, tick_clock, wait_clock):
        carrier = self.nc.sync.nop(nofuse=True, hint="drain_waits")
        wait_clock.add_sem_waits(
            carrier.ins, ScopedClock({None: tick_clock.global_clock})
        )
        si = carrier.ins.sync_info
        waits = list(si.on_wait) if si else []
        if len(waits) > 1:
            carrier.ins.sync_info = mybir.SyncInfo(on_wait=[waits[0]], on_update=[])
            for w in waits[1:]:
                extra = self.nc.sync.nop(nofuse=True, hint="drain_waits")
                extra.ins.sync_info = mybir.SyncInfo(on_wait=[w], on_update=[])
        self.nc.sync.drain()
        self.nc.all_engine_barrier()
        popped = self.nc._tile_sem_poison_stack.pop()
        assert popped is self._sem_poison
        self.nc.clear_and_free_semaphores(list(self.sems.allocated().values()))
        self.nc.all_engine_barrier()

    tile.TileContext._drain_and_barrier = _patched


def _split_multi_waits(nc, max_waits: int = 1):
    """Hoist extra sem waits onto same-engine InstNoOp carriers."""
    for f in nc.m.functions:
        for blk in f.blocks:
            out = []
            for inst in blk.instructions:
                si = inst.sync_info
                if si is not None and len(si.on_wait) > max_waits:
                    waits = list(si.on_wait)
                    for i, w in enumerate(waits[:-max_waits]):
                        out.append(mybir.InstNoOp(
                            name=f"{inst.name}-w{i}",
                            sync_info=mybir.SyncInfo(on_wait=[w], on_update=[]),
                            bass_nofuse=True,
                            engine=inst.engine,
                        ))
                    inst.sync_info = mybir.SyncInfo(
                        on_wait=waits[-max_waits:], on_update=list(si.on_update)
                    )
                out.append(inst)
            blk.instructions = out


def build_kernel(split_waits=True, debug=False):
    nc = bass.Bass(target_bir_lowering=False, trn_type="TRN2")

    din = {}
    def inp(name, shape, dt):
        din[name] = nc.dram_tensor(name, shape, dt, kind="ExternalInput")
        return din[name]

    xt = inp("xt", (128, KTD, 2, NB), F8)        # patches^T/SX, DR layout
    w1 = inp("w1", (128, KTD, 2, D), F8)         # W1/SW, DR layout
    w2 = inp("w2", (128, KT, D), BF16)           # W2 [k_local, kt, e]
    dw1 = inp("dw1", (128, 24, 2, D), F8)        # [(t,slo), iblk, shi, j]
    w1i = inp("w1i", (128, 4, 2, D), F8)         # W1/SW mixing-layout, ktd0
    dw2 = inp("dw2", (128, T, KTD, 2, D), F8)    # [k_local, t, ktd, hi, e]
    db1 = inp("db1", (T, D), BF16)               # db1/(SX*SW)
    db2 = inp("db2", (T, D), BF16)
    b1t = inp("b1t", (128, KT), F32)             # b1/(SX*SW) [j_local, jt]
    b2t = inp("b2t", (128, KT), F32)             # b2 [e_local, et]
    b2r = inp("b2r", (BC, D), F32)               # b2 replicated over samples
    mw1 = inp("mw1", (128, KT, HM), BF16)
    mb1t = inp("mb1t", (128, 2), F32)            # mb1 [h_local, chunk]
    mw2 = inp("mw2", (128, 2, T), BF16)          # [h_local, g, t], g=1 padded
    mb2r = inp("mb2r", (BC, T), F32)             # mb2 replicated over samples
    iexp16 = inp("iexp16", (T, 128), F32)        # 16*repeat(eye(8),16,axis=1)
    mask32 = inp("mask32", (128, 2, 32), BF16)   # [(t,slo), shi, s'32]
    i8 = inp("i8", (T, T), F32)                  # eye(8)
    i8bf = inp("i8bf", (T, T), BF16)             # eye(8) bf16

    out = nc.dram_tensor("out", (BC, D), F32, kind="ExternalOutput")
    if debug:
        for nm, shp, dt in [
                ("dbg_poolb", (128, KT, BC), F32), ("dbg_coefsB", (BC, T), F32),
                ("dbg_crep", (128, T), F32),
                ("dbg_mxcb0", (128, 4, KTD, 2, D), F8),
                ("dbg_pooln", (128, KT, BC), F32),
                ("dbg_vst", (BC, T, D), BF16)]:
            din[nm] = nc.dram_tensor(nm, shp, dt, kind="ExternalOutput")

    with tile.TileContext(nc) as tc:
        with (
            tc.tile_pool(name="big", bufs=1) as big,
            tc.tile_pool(name="sm", bufs=1) as sm,
            tc.tile_pool(name="scr", bufs=2) as scr,
        ):
            # ---------- persistent loads ----------
            w1_sb = big.tile([128, KTD, 2, D], F8, tag="w1")
            nc.sync.dma_start(w1_sb[:], w1[:])
            xt_sb = big.tile([128, KTD, 2, NB], F8, tag="xt")
            nc.scalar.dma_start(xt_sb[:], xt[:])
            b1t_sb = sm.tile([128, KT], F32, tag="b1t")
            nc.sync.dma_start(b1t_sb[:], b1t[:])
            b2t_sb = sm.tile([128, KT], F32, tag="b2t")
            nc.sync.dma_start(b2t_sb[:], b2t[:])
            b2r_sb = sm.tile([BC, D], F32, tag="b2r")
            nc.sync.dma_start(b2r_sb[:], b2r[:])
            mw1_sb = sm.tile([128, KT, HM], BF16, tag="mw1")
            nc.sync.dma_start(mw1_sb[:], mw1[:])
            mb1t_sb = sm.tile([128, 2], F32, tag="mb1t")
            nc.sync.dma_start(mb1t_sb[:], mb1t[:])
            mw2_sb = sm.tile([128, 2, T], BF16, tag="mw2")
            nc.sync.dma_start(mw2_sb[:], mw2[:])
            mb2r_sb = sm.tile([BC, T], F32, tag="mb2r")
            nc.sync.dma_start(mb2r_sb[:], mb2r[:])
            iexp16_sb = sm.tile([T, 128], F32, tag="iexp16")
            nc.sync.dma_start(iexp16_sb[:], iexp16[:])
            mask32_sb = sm.tile([128, 2, 32], BF16, tag="mask32")
            nc.sync.dma_start(mask32_sb[:], mask32[:])
            i8_sb = sm.tile([T, T], F32, tag="i8")
            nc.sync.dma_start(i8_sb[:], i8[:])
            i8bf_sb = sm.tile([T, T], BF16, tag="i8bf")
            nc.sync.dma_start(i8bf_sb[:], i8bf[:])
            db1_sb = sm.tile([T, D], BF16, tag="db1")
            nc.sync.dma_start(db1_sb[:], db1[:])
            db2_sb = sm.tile([T, D], BF16, tag="db2")
            nc.sync.dma_start(db2_sb[:], db2[:])
            # scalar queue: w2 (metanet), then mixing/ph5 tensors
            w2_sb = big.tile([128, KT, D], BF16, tag="w2")
            nc.scalar.dma_start(w2_sb[:], w2[:])
            dw1_sb = big.tile([128, 24, 2, D], F8, tag="dw1")
            nc.scalar.dma_start(dw1_sb[:], dw1[:])
            w1i_sb = big.tile([128, 4, 2, D], F8, tag="w1i")
            nc.scalar.dma_start(w1i_sb[:], w1i[:])
            dw2_sb = big.tile([128, T, KTD, 2, D], F8, tag="dw2")
            nc.scalar.dma_start(dw2_sb[:], dw2[:])

            poolb = sm.tile([128, KT, BC], F32, tag="poolb")
            pooln = sm.tile([128, KT, BC], F32, tag="pooln")
            zeros_sb = sm.tile([128, NPAT], F8, tag="zeros")
            nc.vector.memset(zeros_sb[:], 0.0)
            zeros32_sb = sm.tile([128, 2, 32], F8, tag="zeros32")
            nc.vector.memset(zeros32_sb[:], 0.0)

            def relu_pool(pa, bi, jt, b, dst):
                """relu(pa_slice + b1') summed into dst column; ACT or DVE."""
                sl = pa[:, bi * NPAT:(bi + 1) * NPAT]
                if b % 2 == 0:
                    ro = scr.tile([128, NPAT], F8, tag="ro")
                    nc.scalar.activation(
                        ro[:], sl, RELU, bias=b1t_sb[:, jt:jt + 1],
                        accum_out=dst)
                else:
                    ro = scr.tile([128, NPAT], F8, tag="ro")
                    nc.vector.scalar_tensor_tensor(
                        ro[:], sl, b1t_sb[:, jt:jt + 1], zeros_sb[:],
                        op0=ADD, op1=MAX, accum_out=dst)

            # ---------- phase 1: base pass (fp8 DR) ----------
            with tc.tile_pool(name="psA", bufs=4, space="PSUM") as psA:
                for jt in range(KT):
                    for ch in range(4):      # chunks of 392 = 2 samples
                        pa = psA.tile([128, 392], F32, tag="a")
                        for ktd in range(KTD):
                            nc.tensor.matmul(
                                pa[:],
                                w1_sb[:, ktd, :, jt * 128:(jt + 1) * 128],
                                xt_sb[:, ktd, :, ch * 392:(ch + 1) * 392],
                                start=(ktd == 0), stop=(ktd == KTD - 1),
                                perf_mode=DR)
                        ro = scr.tile([128, 2, NPAT], BF16, tag="rr")
                        if ch % 4 != 3:   # 18 ACT / 6 DVE relus
                            nc.scalar.activation(
                                ro[:], pa[:].rearrange("p (b n) -> p b n", b=2),
                                RELU, bias=b1t_sb[:, jt:jt + 1])
                        else:
                            nc.vector.tensor_scalar(
                                ro[:], pa[:].rearrange("p (b n) -> p b n", b=2),
                                b1t_sb[:, jt:jt + 1], 0.0, op0=ADD, op1=MAX)
                        nc.vector.tensor_reduce(
                            poolb[:, jt, ch * 2:(ch + 1) * 2], ro[:],
                            axis=mybir.AxisListType.X, op=ADD)

            # ---------- phase 2: MetaNet ----------
            with tc.tile_pool(name="psB", bufs=1, space="PSUM") as psB:
                poolb_bf = sm.tile([128, KT, BC], BF16, tag="poolbbf")
                nc.scalar.mul(poolb_bf[:], poolb[:], SXW / NPAT)

                # base2 = pooled @ W2 : [8, 768] (small stationary)
                pb2 = psB.tile([8, 2, 512], F32, tag="b2big")
                for eh in range(2):
                    for kt in range(KT):
                        nc.tensor.matmul(
                            pb2[:, eh, 0:384], poolb_bf[:, kt, :],
                            w2_sb[:, kt, eh * 384:(eh + 1) * 384],
                            start=(kt == 0), stop=(kt == KT - 1))
                base2b = sm.tile([BC, D], BF16, tag="base2b")
                nc.vector.tensor_copy(
                    base2b[:].rearrange("b (eh e) -> b eh e", eh=2),
                    pb2[:, :, 0:384])

                # transpose to [e_local, et, b] and add b2 bias
                base2T = sm.tile([128, KT, BC], BF16, tag="base2T")
                for et in range(KT):
                    ptp = psB.tile([128, BC], BF16, tag="tp")
                    nc.tensor.transpose(
                        ptp[:], base2b[:, et * 128:(et + 1) * 128], i8bf_sb[:])
                    nc.vector.tensor_scalar_add(
                        base2T[:, et, :], ptp[:], b2t_sb[:, et:et + 1])

                # mh^T[h, b] directly: lhsT = mw1 tiles, rhs = base2T cols
                pmh0 = psB.tile([128, BC], F32, tag="mh")
                pmh1 = psB.tile([64, BC], F32, tag="mh1")
                for et in range(KT):
                    nc.tensor.matmul(
                        pmh0[:], mw1_sb[:, et, 0:128], base2T[:, et, :],
                        start=(et == 0), stop=(et == KT - 1))
                for et in range(KT):
                    nc.tensor.matmul(
                        pmh1[:], mw1_sb[:, et, 128:192], base2T[:, et, :],
                        start=(et == 0), stop=(et == KT - 1))
                mhT0 = sm.tile([128, BC], BF16, tag="mhT0")
                mhT1 = sm.tile([64, BC], BF16, tag="mhT1")
                nc.vector.tensor_scalar(
                    mhT0[:], pmh0[:], mb1t_sb[:, 0:1], 0.0, op0=ADD, op1=MAX)
                nc.vector.tensor_scalar(
                    mhT1[:], pmh1[:], mb1t_sb[0:64, 1:2], 0.0, op0=ADD, op1=MAX)

                # coefs[b, t]
                pcB = psB.tile([BC, T], F32, tag="sm8")
                nc.tensor.matmul(pcB[:], mhT0[:], mw2_sb[:, 0, :],
                                 start=True, stop=False)
                nc.tensor.matmul(pcB[:], mhT1[:], mw2_sb[0:64, 1, :],
                                 start=False, stop=True)
                coefsB = sm.tile([BC, T], F32, tag="coefsB")
                nc.vector.tensor_tensor(coefsB[:], pcB[:], mb2r_sb[:], op=ADD)

                # coefsT[t, b] via PE transpose
                ptc = psB.tile([T, T], F32, tag="sm8")
                nc.tensor.transpose(ptc[:], coefsB[:], i8_sb[:])
                coefsT = sm.tile([T, T], F32, tag="coefsT")
                nc.vector.tensor_copy(coefsT[:], ptc[:])
                coefsT_bf = sm.tile([T, T], BF16, tag="coefsTbf")
                nc.vector.tensor_copy(coefsT_bf[:], ptc[:])

                # crep16[(t,slo), b] = 16*c[t,b]
                pcr = psB.tile([128, T], F32, tag="sm8")
                nc.tensor.matmul(pcr[:], iexp16_sb[:], coefsT[:],
                                 start=True, stop=True)
                crep = sm.tile([128, T], F32, tag="crepsb")
                nc.vector.tensor_copy(crep[:], pcr[:])

                # mixing stationaries cb2_g[(t,slo), shi, (bg,s'32)] fp8
                cb2_0 = sm.tile([128, 2, 128], F8, tag="cb2_0")
                cb2_1 = sm.tile([128, 2, 128], F8, tag="cb2_1")
                cb2 = [cb2_0, cb2_1]
                for g in range(2):
                    for bg in range(4):
                        nc.vector.tensor_scalar_mul(
                            cb2[g][:, :, bg * 32:(bg + 1) * 32],
                            mask32_sb[:], crep[:, g * 4 + bg:g * 4 + bg + 1])

                # cdiag[b', t, b] for layer-2 coef application
                cdiag = sm.tile([T, T, T], BF16, tag="cdiag")
                for t in range(T):
                    nc.vector.tensor_scalar_mul(
                        cdiag[:, t, :], i8bf_sb[:], coefsB[:, t:t + 1])

                # nb1t[j_local, jt, b] = (b1 + coefs @ db1)/(SX*SW)
                nb1t = sm.tile([128, KT, BC], F32, tag="nb1t")
                for jt in range(KT):
                    pb = psB.tile([128, T], F32, tag="sm8")
                    nc.tensor.matmul(pb[:], db1_sb[:, jt * 128:(jt + 1) * 128],
                                     coefsT_bf[:], start=True, stop=True)
                    nc.vector.tensor_scalar_add(
                        nb1t[:, jt, :], pb[:], b1t_sb[:, jt:jt + 1])

            # ---------- phase 3: mixing (fp8 DR, 4 samples x 32 rows) ----------
            # mxg[g][(bg,s'), ph, kh, j] = M[(g,bg), i=(kh*4+ph)*32+s', j]/SW
            mxgp = tc.alloc_tile_pool(name="mxgp", bufs=1)
            mxg_0 = mxgp.tile([128, 4, KT, D], F8, tag="mxg0")
            mxg_1 = mxgp.tile([128, 4, KT, D], F8, tag="mxg1")
            mxg = [mxg_0, mxg_1]
            mxcb_0 = big.tile([128, 4, KTD, 2, D], F8, tag="mxcb0")
            mxcb_1 = big.tile([128, 4, KTD, 2, D], F8, tag="mxcb1")
            mxcb = [mxcb_0, mxcb_1]
            nb1t = sm.tile([128, KT, BC], F32, tag="nb1t")
            cdiag = sm.tile([T, T, T], BF16, tag="cdiag")
            with (tc.tile_pool(name="psM", bufs=3, space="PSUM") as psM,
                  tc.tile_pool(name="psN", bufs=1, space="PSUM") as psN):
                for g in range(2):
                    if g == 1:
                        # deferred small work, off the coefs critical path
                        for t in range(T):
                            nc.vector.tensor_scalar_mul(
                                cdiag[:, t, :], i8bf_sb[:],
                                coefsB[:, t:t + 1])
                        for jt in range(KT):
                            pb = psN.tile([128, T], F32, tag="nb1")
                            nc.tensor.matmul(
                                pb[:], db1_sb[:, jt * 128:(jt + 1) * 128],
                                coefsT_bf[:], start=True, stop=True)
                            nc.vector.tensor_scalar_add(
                                nb1t[:, jt, :], pb[:], b1t_sb[:, jt:jt + 1])
                    for ph in range(4):
                        for kh in (0, 2, 1, 3, 4, 5):
                            iblk = kh * 4 + ph
                            pm = psM.tile([128, 2, 512], F32, tag="m")
                            for jh in range(2):
                                nc.tensor.matmul(
                                    pm[:, jh, 0:384], cb2[g][:],
                                    dw1_sb[:, iblk, :, jh * 384:(jh + 1) * 384],
                                    start=True, stop=True, perf_mode=DR)
                            dst = mxg[g][:, ph, kh, :].rearrange(
                                "p (jh j) -> p jh j", jh=2)
                            if kh < 2:
                                # fold W1 in: (pm/16) + W1/SW  (ktd0)
                                nc.vector.scalar_tensor_tensor(
                                    dst, pm[:, :, 0:384], 1.0 / 16.0,
                                    w1i_sb[:, ph, kh, :].rearrange(
                                        "p (jh j) -> p jh j", jh=2),
                                    op0=MULT, op1=ADD)
                            else:
                                nc.scalar.mul(dst, pm[:, :, 0:384], 1.0 / 16.0)
                        # de-interleave gathers for this (g, ph) column
                        for bg in range(4):
                            deq = nc.sync
                            deq.dma_start(
                                mxcb[g][ph * 32:(ph + 1) * 32, bg, :, :, :],
                                mxg[g][bg * 32:(bg + 1) * 32, ph, :, :])

            mxgp.release()

            # ---------- phase 4: final per-sample pass ----------
            if True:
                with tc.tile_pool(name="psF", bufs=6, space="PSUM") as psF:
                    for b in range(BC):
                        g, bg = b // 4, b % 4
                        for jt in range(KT):
                            pf = psF.tile([128, NPAT], F32, tag="f")
                            for ktd in range(KTD):
                                nc.tensor.matmul(
                                    pf[:],
                                    mxcb[g][:, bg, ktd, :,
                                            jt * 128:(jt + 1) * 128],
                                    xt_sb[:, ktd, :, b * NPAT:(b + 1) * NPAT],
                                    start=(ktd == 0), stop=False,
                                    perf_mode=DR)
                            nc.tensor.matmul(
                                pf[:],
                                w1_sb[:, 1, :, jt * 128:(jt + 1) * 128],
                                xt_sb[:, 1, :, b * NPAT:(b + 1) * NPAT],
                                start=False, stop=False, perf_mode=DR)
                            nc.tensor.matmul(
                                pf[:],
                                w1_sb[:, 2, :, jt * 128:(jt + 1) * 128],
                                xt_sb[:, 2, :, b * NPAT:(b + 1) * NPAT],
                                start=False, stop=True, perf_mode=DR)
                            ro = scr.tile([128, NPAT], F8, tag="ro")
                            if (b * KT + jt) % 8 < 3:   # 18 ACT / 30 DVE
                                nc.scalar.activation(
                                    ro[:], pf[:], RELU,
                                    bias=nb1t[:, jt, b:b + 1],
                                    accum_out=pooln[:, jt, b:b + 1])
                            else:
                                nc.vector.scalar_tensor_tensor(
                                    ro[:], pf[:], nb1t[:, jt, b:b + 1],
                                    zeros_sb[:], op0=ADD, op1=MAX,
                                    accum_out=pooln[:, jt, b:b + 1])

            # ---------- phase 5: layer 2 ----------
            pooln_f8 = sm.tile([128, KTD, 2, 32], F8, tag="poolnf8")
            nc.gpsimd.memset(pooln_f8[:], 0.0)
            nc.scalar.mul(
                pooln_f8[:, :, :, 0:BC].rearrange("p k h b -> p (k h) b"),
                pooln[:], SXW / NPAT)
            pooln_bf = sm.tile([128, KT, BC], BF16, tag="poolnbf")
            nc.gpsimd.tensor_scalar_mul(pooln_bf[:], pooln[:], SXW / NPAT)

            vst = sm.tile([BC, T, D], BF16, tag="vst")
            psV = tc.alloc_tile_pool(name="psV", bufs=2, space="PSUM")
            for t in range(T):
                pv = psV.tile([32, 2, 512], F32, tag="v")
                for eh in range(2):
                    for ktd in range(KTD):
                        nc.tensor.matmul(
                            pv[:, eh, 0:384], pooln_f8[:, ktd, :, :],
                            dw2_sb[:, t, ktd, :, eh * 384:(eh + 1) * 384],
                            start=(ktd == 0), stop=(ktd == KTD - 1),
                            perf_mode=DR)
                if t % 2 == 0:
                    nc.vector.tensor_scalar_mul(
                        vst[:, t, :].rearrange("b (eh e) -> b eh e", eh=2),
                        pv[0:BC, :, 0:384], SW)
                else:
                    nc.scalar.mul(
                        vst[:, t, :].rearrange("b (eh e) -> b eh e", eh=2),
                        pv[0:BC, :, 0:384], SW)

            out_sb = sm.tile([BC, D], F32, tag="out")
            po = psV.tile([BC, 2, 512], F32, tag="v")
            for eh in range(2):
                for kt in range(KT):
                    nc.tensor.matmul(
                        po[:, eh, 0:384], pooln_bf[:, kt, :],
                        w2_sb[:, kt, eh * 384:(eh + 1) * 384],
                        start=(kt == 0), stop=False)
                for t in range(T):
                    nc.tensor.matmul(
                        po[:, eh, 0:384], cdiag[:, t, :],
                        vst[:, t, eh * 384:(eh + 1) * 384],
                        start=False, stop=False)
                nc.tensor.matmul(
                    po[:, eh, 0:384], coefsT_bf[:],
                    db2_sb[:, eh * 384:(eh + 1) * 384],
                    start=False, stop=True)
            nc.vector.tensor_tensor(
                out_sb[:].rearrange("b (eh e) -> b eh e", eh=2),
                po[:, :, 0:384], b2r_sb[:].rearrange("b (eh e) -> b eh e", eh=2),
                op=ADD)
            nc.sync.dma_start(out[:], out_sb[:])
            if debug:
                nc.sync.dma_start(din["dbg_poolb"][:], poolb[:])
                nc.sync.dma_start(din["dbg_coefsB"][:], coefsB[:])
                nc.sync.dma_start(din["dbg_crep"][:], crep[:])
                nc.sync.dma_start(din["dbg_mxcb0"][:], mxcb[0][:])
                nc.sync.dma_start(din["dbg_pooln"][:], pooln[:])
                nc.sync.dma_start(din["dbg_vst"][:], vst[:])
            psV.release()

    if split_waits:
        _split_multi_waits(nc)
    return nc


def prep_inputs(x, W1, b1, W2, b2, dW1, db1, dW2, db2, mw1, mb1, mw2, mb2):
    """Host-side layout prep. Returns per-core in_maps."""
    bf = ml_dtypes.bfloat16
    f8 = ml_dtypes.float8_e4m3
    f32 = np.float32
    x = np.asarray(x, f32); W1 = np.asarray(W1, f32); W2 = np.asarray(W2, f32)
    b1 = np.asarray(b1, f32); b2 = np.asarray(b2, f32)
    dW1 = np.asarray(dW1, f32); dW2 = np.asarray(dW2, f32)
    db1 = np.asarray(db1, f32); db2 = np.asarray(db2, f32)
    mw1 = np.asarray(mw1, f32); mb1 = np.asarray(mb1, f32)
    mw2 = np.asarray(mw2, f32); mb2 = np.asarray(mb2, f32)

    # patches^T: [B, D, NPAT]
    pt = x.reshape(B, 3, 14, P, 14, P).transpose(0, 1, 3, 5, 2, 4)
    pt = np.ascontiguousarray(pt).reshape(B, D, NPAT)

    # shared (replicated) tensors
    w1_c = np.ascontiguousarray(
        (W1 / SW).reshape(KTD, 2, 128, D).transpose(2, 0, 1, 3)).astype(f8)
    w2_c = np.ascontiguousarray(
        W2.reshape(KT, 128, D).transpose(1, 0, 2)).astype(bf)
    # dw1[(t,slo), iblk, shi, j] = dW1[t, iblk*32+shi*16+slo, j]/SW
    d = (dW1 / SW).reshape(T, 24, 2, P, D)       # [t, iblk, shi, slo, j]
    dw1_c = np.ascontiguousarray(
        d.transpose(0, 3, 1, 2, 4).reshape(128, 24, 2, D)).astype(f8)
    # w1i[(bg,s'), ph, kh, j] = W1[(kh*4+ph)*32+s', j]/SW  (kh<2: ktd0)
    w1i_c = np.zeros((128, 4, 2, D), np.float32)
    for ph in range(4):
        for kh in range(2):
            blk = (W1 / SW)[(kh * 4 + ph) * 32:(kh * 4 + ph) * 32 + 32, :]
            for bg in range(4):
                w1i_c[bg * 32:(bg + 1) * 32, ph, kh, :] = blk
    w1i_c = w1i_c.astype(f8)
    # dw2[k_local, t, ktd, hi, e] = dW2[t, ktd*256+hi*128+k_local, e]/SW
    dw2_c = np.ascontiguousarray(
        (dW2 / SW).reshape(T, KTD, 2, 128, D).transpose(3, 0, 1, 2, 4)
    ).astype(f8)
    db1_c = (db1 / SXW).astype(bf)
    db2_c = db2.astype(bf)
    b1t_c = np.ascontiguousarray((b1 / SXW).reshape(KT, 128).T).astype(f32)
    b2t_c = np.ascontiguousarray(b2.reshape(KT, 128).T).astype(f32)
    b2r_c = np.tile(b2, (BC, 1))
    mw1_c = np.ascontiguousarray(
        mw1.reshape(KT, 128, HM).transpose(1, 0, 2)).astype(bf)
    mb1t_c = np.zeros((128, 2), f32)
    mb1t_c[:, 0] = mb1[:128]
    mb1t_c[:64, 1] = mb1[128:]
    mw2_c = np.zeros((128, 2, T), f32)
    mw2_c[:, 0, :] = mw2[:128]
    mw2_c[:64, 1, :] = mw2[128:]
    mw2_c = mw2_c.astype(bf)
    mb2r_c = np.tile(mb2, (BC, 1)).astype(f32)
    iexp16_c = np.repeat(np.eye(T, dtype=f32) * 16.0, P, axis=1)
    # mask32[(t,slo), shi, s'] = (s' == shi*16+slo)
    m32 = np.zeros((P, 2, 32), f32)
    for slo in range(P):
        for shi in range(2):
            m32[slo, shi, shi * P + slo] = 1.0
    mask32_c = np.tile(m32, (T, 1, 1)).astype(bf)
    i8_c = np.eye(T, dtype=f32)

    shared = dict(
        w1=w1_c, w1i=w1i_c, w2=w2_c, dw1=dw1_c, dw2=dw2_c, db1=db1_c,
        db2=db2_c,
        b1t=b1t_c, b2t=b2t_c, b2r=b2r_c, mw1=mw1_c, mb1t=mb1t_c,
        mw2=mw2_c, mb2r=mb2r_c,
        iexp16=iexp16_c, mask32=mask32_c, i8=i8_c, i8bf=i8_c.astype(bf),
    )

    in_maps = []
    for c in range(NCORES):
        ptc = pt[c * BC:(c + 1) * BC]                  # [BC, D, NPAT]
        # xt[p, ktd, hi, (b,n)] = ptc[b, ktd*256+hi*128+p, n]/SX
        xt_c = np.ascontiguousarray(
            (ptc / SX).reshape(BC, KTD, 2, 128, NPAT).transpose(3, 1, 2, 0, 4)
        ).reshape(128, KTD, 2, NB).astype(f8)
        m = dict(shared)
        m["xt"] = xt_c
        in_maps.append(m)
    return in_maps


_NC_CACHE = {}


def kernel(**inputs) -> np.ndarray:
    _apply_tile_patch()
    if "nc" not in _NC_CACHE:
        _NC_CACHE["nc"] = build_kernel()
    nc = _NC_CACHE["nc"]
    in_maps = prep_inputs(**inputs)
    res = run_bass_kernel_spmd(nc, in_maps, core_ids=list(range(NCORES)))
    return np.concatenate([r["out"] for r in res.results], axis=0)
